# revision 19
# baseline (speedup 1.0000x reference)
"""Trainium2 Bass kernel for nn_AGNN (3-frame attentional GNN + ConvGRU).

Self-contained: builds an 8-core SPMD Bass graph (sequence-parallel over the
48x48 spatial tokens, 6 rows per core), runs it via run_bass_kernel_spmd,
and reassembles the full output.

Sharding: each core owns 6 rows (288 tokens) of every frame. Per iteration:
  AllGather h (bf16, ch-major + tok-major layouts) -> each core computes
  attention for its 288 query tokens against all 2304 keys of each frame
  (9 ordered frame pairs), gated aggregation, then a 4-row boundary
  AllGather (magg + h) feeds the halo rows of the 3x3 ConvGRU which each
  core evaluates for its own rows.  Readout convs are local (v computed
  with halo from the raw frames; h halo from a final boundary exchange).

Precision: attention matmuls in bf16 (fp32 PSUM accumulation), everything
else float32r (tf32 matmul mode).  Validated ~5.5e-4 rel error vs the
fp32 reference in simulation.
"""
import sys
import numpy as np

NF = 3          # frames
C = 256         # channels
HF = WF = 48    # feature map
P = HF * WF     # 2304 tokens/frame
NCORES = 8
RW = 6          # rows per core
PL = RW * WF    # 288 tokens per core
K_ITERS = 3

_CACHE = {}


def _build_graph():
    sys.path.insert(0, '/opt/trn_rl_repo')
    import concourse.bass as bass
    import concourse.mybir as mybir
    import concourse.tile as tile
    from concourse import bacc

    dt = mybir.dt
    f32 = dt.float32
    f32r = dt.float32r
    bf16 = dt.bfloat16
    AF = mybir.ActivationFunctionType
    OP = mybir.AluOpType
    RG = [list(range(NCORES))]

    nc = bacc.Bacc()

    # ---------------- external IO ----------------
    def ein(name, shape, dtype=None):
        return nc.dram_tensor(name, list(shape), dtype or f32, kind="ExternalInput")

    patches = ein("patches", (192, NF, 480), f32r)       # host patch-extract, rows 6k-2..6k+7
    bbw0 = ein("bbw0", (128, C), f32r)
    bbw1 = ein("bbw1", (64, C), f32r)
    bbb = ein("bbb", (128, 2))
    bbb_top = ein("bbb_top", (128, 2))
    bbb_bot = ein("bbb_bot", (128, 2))
    w_int = ein("w_int", (128, 2, C), f32r)              # W_inter [c, d] swizzled
    w_inta = ein("w_inta", (128, 2, C), f32r)
    gw_t = ein("gw_t", (128, 2, C), f32r)                # gate_w^T [i, o] swizzled
    gb_neg = ein("gb_neg", (128, 2))                       # -gate_b
    wz = ein("wz", (9, 128, 2, 4, 128), f32r)            # [tap, kp, mt, kt, mp]
    wr = ein("wr", (9, 128, 2, 4, 128), f32r)
    wh = ein("wh", (9, 128, 2, 4, 128), f32r)
    bz = ein("bz", (128, 2))
    br = ein("br", (128, 2))
    bh = ein("bh", (128, 2))
    ro1 = ein("ro1", (9, 128, 2, 4, 128), f32r)
    rb1 = ein("rb1", (128, 2))
    rb1_top = ein("rb1_top", (128, 2))
    rb1_bot = ein("rb1_bot", (128, 2))
    ro2 = ein("ro2", (128, 2, 9), f32r)
    rb2 = ein("rb2", (1, 1))
    sel_top = ein("sel_top", (128, 8))                   # one-hot rank k-1 (zeros at core 0)
    sel_bot = ein("sel_bot", (128, 8))                   # one-hot rank k+1 (zeros at core 7)
    sc_top = ein("sc_top", (128, 1))                     # 0.0 on core 0 else 1.0
    sc_bot = ein("sc_bot", (128, 1))                     # 0.0 on core 7 else 1.0
    ident_in = ein("ident_in", (128, 128), f32r)

    out_ext = nc.dram_tensor("out", [NF, RW, WF], f32, kind="ExternalOutput")

    SH = NF * C * PL            # 221184 elements per layout shard

    with tile.TileContext(nc) as tc:
        with (
            tc.tile_pool(name="pers", bufs=1) as pers,
            tc.tile_pool(name="dram", bufs=1, space="DRAM") as dram,
            tc.tile_pool(name="psum", bufs=1, space="PSUM") as psum,
            tc.tile_pool(name="work", bufs=1) as work,
        ):
            # ---------------- persistent SBUF ----------------
            ident = pers.tile([128, 128], f32r)
            nc.sync.dma_start(ident[...], ident_in.ap())

            def load_pers(name, ext, shape, view=None):
                t = pers.tile(list(shape), f32r, name=name)
                src = ext.ap() if view is None else view
                nc.sync.dma_start(t[...], src)
                return t

            # W_inter/W_intra/gate_w^T as [128, 2ct, 256]
            wint_t = load_pers("wint_t", w_int, (128, 2, C))
            winta_t = load_pers("winta_t", w_inta, (128, 2, C))
            gw_tt = load_pers("gw_tt", gw_t, (128, 2, C))
            bbw0_t = load_pers("bbw0_t", bbw0, (128, C))
            bbw1_t = load_pers("bbw1_t", bbw1, (64, C))
            ro2_t = load_pers("ro2_t", ro2, (128, 2, 9))

            def load_bias(name, ext):
                t = pers.tile([128, 2], f32, name=name)
                nc.sync.dma_start(t[...], ext.ap())
                return t

            bbb_t = load_bias("bbb_t", bbb)
            bbbtop_t = load_bias("bbbtop_t", bbb_top)
            bbbbot_t = load_bias("bbbbot_t", bbb_bot)
            gnb_t = load_bias("gnb_t", gb_neg)
            bz_t = load_bias("bz_t", bz)
            br_t = load_bias("br_t", br)
            bh_t = load_bias("bh_t", bh)
            rb1_t = load_bias("rb1_t", rb1)
            rb1top_t = load_bias("rb1top_t", rb1_top)
            rb1bot_t = load_bias("rb1bot_t", rb1_bot)
            rb2_t = pers.tile([1, 1], f32)
            nc.sync.dma_start(rb2_t[...], rb2.ap())
            seltop_t = pers.tile([128, 8], f32)
            nc.sync.dma_start(seltop_t[...], sel_top.ap())
            selbot_t = pers.tile([128, 8], f32)
            nc.sync.dma_start(selbot_t[...], sel_bot.ap())
            sctop_t = pers.tile([128, 1], f32)
            nc.sync.dma_start(sctop_t[...], sc_top.ap())
            scbot_t = pers.tile([128, 1], f32)
            nc.sync.dma_start(scbot_t[...], sc_bot.ap())

            # big persistent state
            v_sb = pers.tile([128, 2, NF, 10 * WF], f32)      # v rows 6k-2..6k+7
            h_loc = pers.tile([128, 2, NF, PL], f32r)          # own rows, ch-major
            magg = pers.tile([128, 2, NF, PL], f32r)
            t_sb = pers.tile([128, 2, 6, PL], bf16)           # t^T (3 inter + 3 intra)

            def r32(ap):
                return ap.bitcast(f32r)

            # ---------------- backbone ----------------
            with tc.tile_pool(name="bb", bufs=1) as bb:
                pk0 = bb.tile([128, NF, 480], f32r, name="pk0")
                pk1 = bb.tile([64, NF, 480], f32r, name="pk1")
                nc.sync.dma_start(pk0[...], patches[0:128])
                nc.sync.dma_start(pk1[...], patches[128:192])
                for f in range(NF):
                    vps = psum.tile([128, 2, 512], f32, tag="pe", bufs=2, name="vps")
                    for mt in range(2):
                        nc.tensor.matmul(vps[:, mt, 0:480],
                                         bbw0_t[:, mt * 128:(mt + 1) * 128],
                                         pk0[:, f, :], start=True, stop=False)
                        nc.tensor.matmul(vps[:, mt, 0:480],
                                         bbw1_t[0:64, mt * 128:(mt + 1) * 128],
                                         pk1[0:64, f, :], start=False, stop=True)
                    for mt in range(2):
                        # rows 0-1 / 2-7 / 8-9 with edge masking (v=0 outside image)
                        nc.scalar.activation(v_sb[:, mt, f, 0:96], vps[:, mt, 0:96],
                                             AF.Relu, bias=bbbtop_t[:, mt:mt + 1], scale=sctop_t[:, :])
                        nc.scalar.activation(v_sb[:, mt, f, 96:384], vps[:, mt, 96:384],
                                             AF.Relu, bias=bbb_t[:, mt:mt + 1])
                        nc.scalar.activation(v_sb[:, mt, f, 384:480], vps[:, mt, 384:480],
                                             AF.Relu, bias=bbbbot_t[:, mt:mt + 1], scale=scbot_t[:, :])
                        # h0 = v own rows (rows 2..8 of the 10-row window)
                        nc.vector.tensor_copy(h_loc[:, mt, f, :], v_sb[:, mt, f, 96:384])

            # ---------------- iterations ----------------
            for it in range(K_ITERS):
                # ---- write AG input: ch-major + tok-major (bf16) ----
                agi = dram.tile([2, NF, SH // NF], bf16, tag="agi", bufs=2, name="agi")
                hloc16 = work.tile([128, 2, NF, PL], bf16, tag="hloc16", bufs=1, name="hloc16")
                nc.vector.tensor_copy(hloc16[...], h_loc[...])
                for f in range(NF):
                    for ct in range(2):
                        nc.sync.dma_start(
                            agi[0, f].rearrange("(c t) -> c t", t=PL)[ct * 128:(ct + 1) * 128, :],
                            hloc16[:, ct, f, :])
                    # tok-major via TensorE transposes: [c,96tok] -> [96tok, 128c]
                    hlt = work.tile([96, 3, C], bf16, tag="hlt", bufs=2, name="hlt")
                    for ct in range(2):
                        for ps in range(3):
                            tp = psum.tile([96, 128], f32r, tag="aux", bufs=1, name="tp")
                            nc.tensor.transpose(
                                tp[0:96, 0:128],
                                h_loc[:, ct, f, ps * 96:(ps + 1) * 96],
                                ident[:, :])
                            nc.vector.tensor_copy(hlt[0:96, ps, ct * 128:(ct + 1) * 128],
                                                  tp[0:96, 0:128])
                    nc.sync.dma_start(
                        agi[1, f].rearrange("(t c) -> t c", c=C)
                        .rearrange("(ps p) c -> p ps c", p=96),
                        hlt[0:96, :, :])

                ago = dram.tile([NCORES, 2, NF, SH // NF], bf16, tag="ago", bufs=2,
                                addr_space="Shared", name="ago")
                nc.gpsimd.collective_compute(
                    "AllGather", OP.bypass, replica_groups=RG,
                    ins=[agi.opt()], outs=[ago.opt()])

                # ---- t = h_loc @ W (both kinds), bf16 out ----
                for i in range(NF):
                    for kind in range(2):       # 0 inter, 1 intra
                        wsel = wint_t if kind == 0 else winta_t
                        for dct in range(2):
                            tps = psum.tile([128, 2, 512], f32, tag="pe", bufs=2, name="tps")
                            for ct in range(2):
                                nc.tensor.matmul(
                                    tps[:, 0, 0:288],
                                    r32(wsel[:, ct, dct * 128:(dct + 1) * 128]),
                                    r32(h_loc[:, ct, i, :]),
                                    start=(ct == 0), stop=(ct == 1))
                            nc.vector.tensor_copy(t_sb[:, dct, kind * 3 + i, :],
                                                  tps[:, 0, 0:288])

                # ---- boundary AG input: magg written later; h part now ----
                bdi = dram.tile([2, NF, C, 4 * WF], f32r, tag="bdi", bufs=2, name="bdi")
                for f in range(NF):
                    for ct in range(2):
                        nc.sync.dma_start(
                            bdi[1, f, ct * 128:(ct + 1) * 128, 0:96], h_loc[:, ct, f, 0:96])
                        nc.sync.dma_start(
                            bdi[1, f, ct * 128:(ct + 1) * 128, 96:192], h_loc[:, ct, f, 192:288])

                # ---- attention over j (keys) and i (queries) ----
                for j in range(NF):
                    # stage frame j contiguously in DRAM (rank chunks are 288
                    # tokens; 128-token tiles cross rank boundaries otherwise)
                    stg = dram.tile([P, C], bf16, tag="stg", bufs=2, name="stg")
                    nc.sync.dma_start(stg[:, :], ago[:, 1, j])
                    stgc = dram.tile([C, P], bf16, tag="stgc", bufs=2, name="stgc")
                    nc.sync.dma_start(
                        stgc[:, :].rearrange("c (r t) -> r c t", r=NCORES),
                        ago[:, 0, j])
                    hch = []
                    for hh in range(2):
                        t_ = work.tile([128, 2, 9, 128], bf16, tag="hch", bufs=2, name="hch")
                        for ct in range(2):
                            nc.sync.dma_start(
                                t_[:, ct, :, :],
                                stgc[ct * 128:(ct + 1) * 128, :]
                                .rearrange("p (q x) -> p q x", x=128)
                                [:, hh * 9:(hh + 1) * 9, :])
                        hch.append(t_)
                    htok = []
                    for hh in range(2):
                        t_ = work.tile([128, 9, 257], bf16, tag="htok", bufs=2, name="htok")
                        nc.vector.memset(t_[...], 1.0)
                        nc.sync.dma_start(
                            t_[:, :, 0:256],
                            stg[:, :].rearrange("(q p) c -> p q c", p=128)
                            [:, hh * 9:(hh + 1) * 9, :])
                        htok.append(t_)

                    for i in range(NF):
                        tix = (3 + i) if i == j else i
                        attn = work.tile([128, 18, 288], bf16, tag="attn", bufs=1, name="attn")
                        mu0 = psum.tile([128, 2, 512], f32, tag="mu0", bufs=1, name="mu0")
                        mu2 = psum.tile([32, 257], f32, tag="mu2", bufs=1, name="mu2")
                        # software-pipelined: e/exp group g, then m-matmuls of g-1
                        for g in range(10):
                            if g < 9:
                                e2 = psum.tile([128, 2, 512], f32, tag="pe", bufs=2, name="e2")
                                for u in range(2):
                                    q = g * 2 + u
                                    for ct in range(2):
                                        nc.tensor.matmul(
                                            e2[:, u, 0:288],
                                            hch[q // 9][:, ct, q % 9, :],
                                            t_sb[:, ct, tix, :],
                                            start=(ct == 0), stop=(ct == 1))
                                nc.scalar.activation(attn[:, g * 2:g * 2 + 2, :],
                                                     e2[:, :, 0:288], AF.Exp)
                            if g >= 1:
                                for u in range(2):
                                    q = (g - 1) * 2 + u
                                    st = (q == 0)
                                    sp = (q == 17)
                                    mv = htok[q // 9][:, q % 9, :]
                                    nc.tensor.matmul(mu0[:, 0, 0:257], attn[:, q, 0:128],
                                                     mv, start=st, stop=sp)
                                    nc.tensor.matmul(mu0[:, 1, 0:257], attn[:, q, 128:256],
                                                     mv, start=st, stop=sp)
                                    nc.tensor.matmul(mu2[0:32, 0:257], attn[:, q, 256:288],
                                                     mv, start=st, stop=sp)
                        # normalize m (softmax denominator = col 256)
                        mnorm = work.tile([128, 3, 256], f32r, tag="mnorm", bufs=1, name="mnorm")
                        rs = work.tile([128, 3, 1], f32, tag="rs", bufs=2, name="rs")
                        for s in range(3):
                            mus = mu0[:, s, :] if s < 2 else mu2[0:32, :]
                            pp = 128 if s < 2 else 32
                            nc.vector.reciprocal(rs[0:pp, s, :], mus[0:pp, 256:257])
                            nc.vector.tensor_scalar(mnorm[0:pp, s, :], mus[0:pp, 0:256],
                                                    rs[0:pp, s, :], None, OP.mult)
                        # transpose m -> ch-major
                        mT = work.tile([128, 2, 288], f32r, tag="mT", bufs=2, name="mT")
                        for ct in range(2):
                            tps = psum.tile([128, 288], f32r, tag="aux", bufs=1, name="mtp")
                            for s in range(3):
                                pp = 128 if s < 2 else 32
                                nc.tensor.transpose(
                                    tps[:, s * 128:s * 128 + pp],
                                    mnorm[0:pp, s, ct * 128:(ct + 1) * 128],
                                    ident[0:pp, 0:pp])
                            nc.vector.tensor_copy(mT[:, ct, :], tps[:, 0:288])
                        # gate: g = sigmoid(gate_w m + b) via exp (stay on exp table)
                        gps = psum.tile([128, 2, 512], f32, tag="mu0", bufs=1, name="gps")
                        for oct in range(2):
                            for ict in range(2):
                                nc.tensor.matmul(
                                    gps[:, oct, 0:288],
                                    r32(gw_tt[:, ict, oct * 128:(oct + 1) * 128]),
                                    r32(mT[:, ict, :]),
                                    start=(ict == 0), stop=(ict == 1))
                        gtmp = work.tile([128, 2, 288], f32, tag="gtmp", bufs=1, name="gtmp")
                        for oct in range(2):
                            nc.scalar.activation(gtmp[:, oct, :], gps[:, oct, 0:288],
                                                 AF.Exp, bias=gnb_t[:, oct:oct + 1], scale=-1.0)
                        nc.vector.tensor_scalar(gtmp[...], gtmp[...], 1.0, None, OP.add)
                        nc.vector.reciprocal(gtmp[...], gtmp[...])
                        nc.vector.tensor_tensor(gtmp[...], gtmp[...], mT[...], OP.mult)
                        if j == 0:
                            nc.vector.tensor_copy(magg[:, :, i, :], gtmp[...])
                        else:
                            nc.vector.tensor_tensor(magg[:, :, i, :], magg[:, :, i, :],
                                                    gtmp[...], OP.add)

                # ---- boundary AG (magg + h 2-row halos) ----
                for f in range(NF):
                    for ct in range(2):
                        nc.sync.dma_start(
                            bdi[0, f, ct * 128:(ct + 1) * 128, 0:96], magg[:, ct, f, 0:96])
                        nc.sync.dma_start(
                            bdi[0, f, ct * 128:(ct + 1) * 128, 96:192], magg[:, ct, f, 192:288])
                bdo = dram.tile([NCORES, 2, NF, C, 4 * WF], f32r, tag="bdo", bufs=2,
                                addr_space="Shared", name="bdo")
                nc.gpsimd.collective_compute(
                    "AllGather", OP.bypass, replica_groups=RG,
                    ins=[bdi.opt()], outs=[bdo.opt()])

                # ---- halo extraction via one-hot rank masks ----
                # halo[kind][ct]: top rows (from rank k-1 bottom seg) & bottom rows
                halo = work.tile([128, 2, 2, NF, 2, 96], f32r, tag="halo", bufs=1, name="halo")
                for kind in range(2):
                    for ct in range(2):
                        for rp in range(4):
                            ch = work.tile([128, 2, NF, 192], f32r, tag="hchk", bufs=1, name="hchk")
                            for _rr in range(2):
                                nc.sync.dma_start(
                                    ch[:, _rr, :, :],
                                    bdo[rp * 2 + _rr, kind, :, ct * 128:(ct + 1) * 128, :]
                                    .rearrange("f c x -> c f x"))
                            for rr in range(2):
                                r = rp * 2 + rr
                                for tb in range(2):
                                    sel = seltop_t if tb == 0 else selbot_t
                                    seg = ch[:, rr, :, 96:192] if tb == 0 else ch[:, rr, :, 0:96]
                                    dst = halo[:, kind, ct, :, tb, :]
                                    if r == 0:
                                        nc.vector.tensor_scalar(dst, seg, sel[:, 0:1],
                                                                None, OP.mult)
                                    else:
                                        nc.vector.scalar_tensor_tensor(
                                            dst, seg, sel[:, r:r + 1], dst,
                                            OP.mult, OP.add)

                # ---- ConvGRU ----
                mh = []
                for f in range(NF):
                    m_ = work.tile([128, 4, 512], f32r, tag="mh", bufs=3, name="mh")
                    nc.vector.memset(m_[...].bitcast(f32), 0.0)
                    for ct in range(2):
                        rows = m_[:, ct, 6:506].rearrange("p (r w) -> p r w", w=50)
                        nc.vector.tensor_copy(
                            rows[:, 2:8, 1:49],
                            magg[:, ct, f, :].rearrange("p (r w) -> p r w", w=48))
                        nc.vector.tensor_copy(
                            rows[:, 0:2, 1:49],
                            halo[:, 0, ct, f, 0, :].rearrange("p (r w) -> p r w", w=48))
                        nc.vector.tensor_copy(
                            rows[:, 8:10, 1:49],
                            halo[:, 0, ct, f, 1, :].rearrange("p (r w) -> p r w", w=48))
                        hrows = m_[:, 2 + ct, 6:506].rearrange("p (r w) -> p r w", w=50)
                        nc.vector.tensor_copy(
                            hrows[:, 2:8, 1:49],
                            h_loc[:, ct, f, :].rearrange("p (r w) -> p r w", w=48))
                        nc.vector.tensor_copy(
                            hrows[:, 0:2, 1:49],
                            halo[:, 1, ct, f, 0, :].rearrange("p (r w) -> p r w", w=48))
                        nc.vector.tensor_copy(
                            hrows[:, 8:10, 1:49],
                            halo[:, 1, ct, f, 1, :].rearrange("p (r w) -> p r w", w=48))
                    mh.append(m_)

                def conv(wext, psums, NOUT, row0, src_of):
                    """9-tap conv: psums[f][:, mt, 0:NOUT] += taps."""
                    for tap in range(9):
                        dy, dx = tap // 3 - 1, tap % 3 - 1
                        wt = work.tile([128, 2, 4, 128], f32r, tag="wtap", bufs=2, name="wtap")
                        nc.sync.dma_start(wt[...], wext[tap])
                        for f in range(NF):
                            for kt in range(4):
                                mvs = src_of(f, kt, dy, dx)
                                if mvs is None:
                                    continue
                                for mt in range(2):
                                    nc.tensor.matmul(
                                        psums[f][:, mt, 0:NOUT],
                                        wt[:, mt, kt, :], r32(mvs),
                                        start=(tap == 0 and kt == 0),
                                        stop=(tap == 8 and kt == 3))

                def conv_psums():
                    ps = []
                    for f in range(NF):
                        tag = "pe" if f < 2 else "mu0"
                        ps.append(psum.tile([128, 2, 512], f32, tag=tag, bufs=2 if f < 2 else 1,
                                            name=f"cps{f}"))
                    return ps

                # z conv: out rows W2..W7 (own), N=300
                zps = conv_psums()
                conv(wz, zps, 300, 2,
                     lambda f, kt, dy, dx: mh[f][:, kt, 6 + (2 + dy) * 50 + dx:
                                                 6 + (2 + dy) * 50 + dx + 300])
                zgs = []
                for f in range(NF):
                    z_ = work.tile([128, 2, 300], f32, tag=f"zgs{f}", bufs=1, name="zgs")
                    for mt in range(2):
                        nc.scalar.activation(z_[:, mt, :], zps[f][:, mt, 0:300],
                                             AF.Sigmoid, bias=bz_t[:, mt:mt + 1])
                    zgs.append(z_)
                # r conv: out rows W1..W8, N=400
                rps = conv_psums()
                conv(wr, rps, 400, 1,
                     lambda f, kt, dy, dx: mh[f][:, kt, 6 + (1 + dy) * 50 + dx:
                                                 6 + (1 + dy) * 50 + dx + 400])
                rgh = []
                for f in range(NF):
                    # rg sigmoid written at 6-offset, then rg*h in place
                    rh_ = work.tile([128, 2, 416], f32r, tag=f"rgh{f}", bufs=1, name="rgh")
                    nc.vector.memset(rh_[...].bitcast(f32), 0.0)
                    for mt in range(2):
                        nc.scalar.activation(rh_[:, mt, 6:406], rps[f][:, mt, 0:400],
                                             AF.Sigmoid, bias=br_t[:, mt:mt + 1])
                    nc.vector.tensor_tensor(
                        rh_[:, :, 6:406], rh_[:, :, 6:406],
                        mh[f][:, 2:4, 56:456], OP.mult)
                    rgh.append(rh_)
                # candidate conv: out rows W2..W7, N=300; inputs kt0-1 magg, kt2-3 rg*h
                hps = conv_psums()

                def hc_src(f, kt, dy, dx):
                    if kt < 2:
                        o = 6 + (2 + dy) * 50 + dx
                        return mh[f][:, kt, o:o + 300]
                    o = 6 + (1 + dy) * 50 + dx
                    return rgh[f][:, kt - 2, o:o + 300]

                conv(wh, hps, 300, 2, hc_src)
                for f in range(NF):
                    hc_ = work.tile([128, 2, 300], f32, tag="hcs", bufs=2, name="hcs")
                    for mt in range(2):
                        nc.scalar.activation(hc_[:, mt, :], hps[f][:, mt, 0:300],
                                             AF.Tanh, bias=bh_t[:, mt:mt + 1])
                    # h_new = h + z*(hc - h)
                    hold = mh[f][:, 2:4, 106:406]
                    nc.vector.tensor_tensor(hc_[...], hc_[...], hold, OP.subtract)
                    nc.vector.tensor_tensor(hc_[...], hc_[...], zgs[f][...], OP.mult)
                    for ct in range(2):
                        nc.vector.tensor_tensor(
                            h_loc[:, ct, f, :].rearrange("p (r w) -> p r w", w=48),
                            mh[f][:, 2 + ct, 106:406].rearrange("p (r w) -> p r w", w=50)[:, :, 1:49],
                            hc_[:, ct, :].rearrange("p (r w) -> p r w", w=50)[:, :, 1:49],
                            OP.add)

            # ---------------- readout ----------------
            bdi2 = dram.tile([NF, C, 4 * WF], f32r, tag="bdi", bufs=2, name="bdi2")
            for f in range(NF):
                for ct in range(2):
                    nc.sync.dma_start(bdi2[f, ct * 128:(ct + 1) * 128, 0:96],
                                      h_loc[:, ct, f, 0:96])
                    nc.sync.dma_start(bdi2[f, ct * 128:(ct + 1) * 128, 96:192],
                                      h_loc[:, ct, f, 192:288])
            bdo2 = dram.tile([NCORES, NF, C, 4 * WF], f32r, tag="bdo", bufs=2,
                             addr_space="Shared", name="bdo2")
            nc.gpsimd.collective_compute(
                "AllGather", OP.bypass, replica_groups=RG,
                ins=[bdi2.opt()], outs=[bdo2.opt()])
            halo2 = work.tile([128, 2, NF, 2, 96], f32r, tag="halo", bufs=1, name="halo2")
            for ct in range(2):
                for rp in range(4):
                    ch = work.tile([128, 2, NF, 192], f32r, tag="hchk", bufs=1, name="hchk2")
                    for _rr in range(2):
                        nc.sync.dma_start(
                            ch[:, _rr, :, :],
                            bdo2[rp * 2 + _rr, :, ct * 128:(ct + 1) * 128, :]
                            .rearrange("f c x -> c f x"))
                    for rr in range(2):
                        r = rp * 2 + rr
                        for tb in range(2):
                            sel = seltop_t if tb == 0 else selbot_t
                            seg = ch[:, rr, :, 96:192] if tb == 0 else ch[:, rr, :, 0:96]
                            dst = halo2[:, ct, :, tb, :]
                            if r == 0:
                                nc.vector.tensor_scalar(dst, seg, sel[:, 0:1], None, OP.mult)
                            else:
                                nc.vector.scalar_tensor_tensor(
                                    dst, seg, sel[:, r:r + 1], dst, OP.mult, OP.add)

            mask_sb = pers.tile([1, NF, RW * WF], f32)
            for f in range(NF):
                ro_in = work.tile([128, 4, 512], f32r, tag="mh", bufs=3, name="ro_in")
                nc.vector.memset(ro_in[...].bitcast(f32), 0.0)
                for ct in range(2):
                    hrows = ro_in[:, ct, 6:506].rearrange("p (r w) -> p r w", w=50)
                    nc.vector.tensor_copy(
                        hrows[:, 2:8, 1:49],
                        h_loc[:, ct, f, :].rearrange("p (r w) -> p r w", w=48))
                    nc.vector.tensor_copy(
                        hrows[:, 0:2, 1:49],
                        halo2[:, ct, f, 0, :].rearrange("p (r w) -> p r w", w=48))
                    nc.vector.tensor_copy(
                        hrows[:, 8:10, 1:49],
                        halo2[:, ct, f, 1, :].rearrange("p (r w) -> p r w", w=48))
                    vrows = ro_in[:, 2 + ct, 6:506].rearrange("p (r w) -> p r w", w=50)
                    nc.vector.tensor_copy(
                        vrows[:, 0:10, 1:49],
                        v_sb[:, ct, f, :].rearrange("p (r w) -> p r w", w=48))
                # y = relu(ro1 * cat) rows W1..W8 (N=400)
                yps = psum.tile([128, 2, 512], f32, tag="pe", bufs=2, name="yps")
                for tap in range(9):
                    dy, dx = tap // 3 - 1, tap % 3 - 1
                    wt = work.tile([128, 2, 4, 128], f32r, tag="wtap", bufs=2, name="wtap2")
                    nc.sync.dma_start(wt[...], ro1[tap])
                    o = 6 + (1 + dy) * 50 + dx
                    for kt in range(4):
                        for mt in range(2):
                            nc.tensor.matmul(
                                yps[:, mt, 0:400], wt[:, mt, kt, :],
                                r32(ro_in[:, kt, o:o + 400]),
                                start=(tap == 0 and kt == 0), stop=(tap == 8 and kt == 3))
                y_sb = work.tile([128, 2, 412], f32r, tag="y_sb", bufs=1, name="y_sb")
                nc.vector.memset(y_sb[...].bitcast(f32), 0.0)
                for mt in range(2):
                    ypr = yps[:, mt, 0:400].rearrange("p (r w) -> p r w", w=50)
                    ydst = y_sb[:, mt, 6:406].rearrange("p (r w) -> p r w", w=50)
                    # y rows: 0 (global 6k-1, invalid on core 0), 1..7, 7 (invalid on core 7)
                    nc.scalar.activation(ydst[:, 0:1, 1:49], ypr[:, 0:1, 1:49], AF.Relu,
                                         bias=rb1top_t[:, mt:mt + 1], scale=sctop_t[:, :])
                    nc.scalar.activation(ydst[:, 1:7, 1:49], ypr[:, 1:7, 1:49], AF.Relu,
                                         bias=rb1_t[:, mt:mt + 1])
                    nc.scalar.activation(ydst[:, 7:8, 1:49], ypr[:, 7:8, 1:49], AF.Relu,
                                         bias=rb1bot_t[:, mt:mt + 1], scale=scbot_t[:, :])
                # mask = ro2 * y + b2, own rows (N=300 in 50-col layout; y pad
                # cols are zero so the windows are safe)
                mps = psum.tile([1, 300], f32, tag="aux", bufs=1, name="mps")
                for tap in range(9):
                    dy, dx = tap // 3 - 1, tap % 3 - 1
                    o = 6 + (1 + dy) * 50 + dx
                    for ct in range(2):
                        nc.tensor.matmul(
                            mps[0:1, 0:300],
                            r32(ro2_t[:, ct, tap:tap + 1]),
                            r32(y_sb[:, ct, o:o + 300]),
                            start=(tap == 0 and ct == 0), stop=(tap == 8 and ct == 1))
                nc.scalar.activation(
                    mask_sb[0:1, f, :].rearrange("p (r w) -> p r w", w=48),
                    mps[0:1, :].rearrange("p (r w) -> p r w", w=50)[:, :, 1:49],
                    AF.Identity, bias=rb2_t[0:1, :])
            nc.sync.dma_start(out_ext.ap(), mask_sb[0:1, :, :])

    nc.finalize()
    return nc


def _prep_inputs(inputs):
    """Host-side weight prep + per-core shards."""
    frames = np.ascontiguousarray(inputs['frames'], dtype=np.float32)  # (1,3,3,384,384)
    bb_w = np.asarray(inputs['backbone_w'], dtype=np.float32)
    bb_b = np.asarray(inputs['backbone_b'], dtype=np.float32).reshape(C, 1)
    W_intra = np.asarray(inputs['W_intra'], dtype=np.float32)
    W_inter = np.asarray(inputs['W_inter'], dtype=np.float32)
    gate_w = np.asarray(inputs['gate_w'], dtype=np.float32)[:, :, 0, 0]
    gate_b = np.asarray(inputs['gate_b'], dtype=np.float32).reshape(C, 1)

    def taps(w):
        return np.ascontiguousarray(
            np.asarray(w, dtype=np.float32).transpose(2, 3, 1, 0).reshape(9, 2 * C, C))

    def swz(w):
        """[c, d] (256x256) -> [128, 2ct, d]"""
        return np.ascontiguousarray(w.reshape(2, 128, C).transpose(1, 0, 2))

    def bias2(b):
        """(256,1) -> (128, 2)"""
        return np.ascontiguousarray(b.reshape(2, 128).T)

    def conv_taps(w):
        """(O,I,3,3) -> [9, kp(128), mt, kt, mp(128)]"""
        t = taps(w)                                    # (9, 512, 256)
        t = t.reshape(9, 4, 128, 2, 128)               # tap, kt, kp, mt, mp
        return np.ascontiguousarray(t.transpose(0, 2, 3, 1, 4))

    bbw192 = bb_w.transpose(1, 2, 3, 0).reshape(192, C)
    ro2_flat = np.asarray(inputs['ro_w2'], dtype=np.float32
                          ).transpose(2, 3, 1, 0).reshape(9, C)      # tap, c
    common = {
        'bbw0': np.ascontiguousarray(bbw192[0:128]),
        'bbw1': np.ascontiguousarray(bbw192[128:192]),
        'bbb': bias2(bb_b),
        'w_int': swz(W_inter),
        'w_inta': swz(W_intra),
        'gw_t': swz(np.ascontiguousarray(gate_w.T)),
        'gb_neg': bias2(-gate_b),
        'wz': conv_taps(inputs['Wz']), 'wr': conv_taps(inputs['Wr']),
        'wh': conv_taps(inputs['Wh']),
        'bz': bias2(np.asarray(inputs['bz'], dtype=np.float32)),
        'br': bias2(np.asarray(inputs['br'], dtype=np.float32)),
        'bh': bias2(np.asarray(inputs['bh'], dtype=np.float32)),
        'ro1': conv_taps(inputs['ro_w1']),
        'rb1': bias2(np.asarray(inputs['ro_b1'], dtype=np.float32)),
        'ro2': np.ascontiguousarray(
            ro2_flat.T.reshape(2, 128, 9).transpose(1, 0, 2)),
        'rb2': np.asarray(inputs['ro_b2'], dtype=np.float32).reshape(1, 1),
        'ident_in': np.eye(128, dtype=np.float32),
    }

    fp = np.zeros((NF, 3, 384 + 32, 384), np.float32)
    fp[:, :, 16:400] = frames[0]

    in_maps = []
    for k in range(NCORES):
        m = dict(common)
        # patches [192=(ch,dy,dx), f, 10 rows x 48]
        pc = fp[:, :, 48 * k:48 * k + 80, :].reshape(NF, 3, 10, 8, 48, 8)
        m['patches'] = np.ascontiguousarray(
            pc.transpose(1, 3, 5, 0, 2, 4).reshape(192, NF, 480))
        st = np.zeros((128, 8), np.float32)
        sb = np.zeros((128, 8), np.float32)
        if k > 0:
            st[:, k - 1] = 1.0
        if k < NCORES - 1:
            sb[:, k + 1] = 1.0
        m['sel_top'] = st
        m['sel_bot'] = sb
        sct = np.full((128, 1), 0.0 if k == 0 else 1.0, np.float32)
        scb = np.full((128, 1), 0.0 if k == NCORES - 1 else 1.0, np.float32)
        m['sc_top'] = sct
        m['sc_bot'] = scb
        m['bbb_top'] = common['bbb'] * sct[0, 0]
        m['bbb_bot'] = common['bbb'] * scb[0, 0]
        m['rb1_top'] = common['rb1'] * sct[0, 0]
        m['rb1_bot'] = common['rb1'] * scb[0, 0]
        in_maps.append(m)
    return in_maps


def run_cores(inputs, trace=False):
    """Returns (per_core_results, BassKernelResults)."""
    sys.path.insert(0, '/opt/trn_rl_repo')
    from concourse.bass_utils import run_bass_kernel_spmd
    if 'nc' not in _CACHE:
        _CACHE['nc'] = _build_graph()
    nc = _CACHE['nc']
    in_maps = _prep_inputs(inputs)
    res = run_bass_kernel_spmd(nc, in_maps, core_ids=list(range(NCORES)), trace=trace)
    return res


def kernel(**inputs):
    res = run_cores(inputs, trace=False)
    out = np.zeros((1, NF, 1, HF, WF), np.float32)
    for k in range(NCORES):
        out[0, :, 0, RW * k:RW * (k + 1), :] = res.results[k]['out']
    return out


if __name__ == '__main__':
    data = np.load('/tmp/ref_inputs.npz')
    inputs = {k: data[k] for k in data.files}
    out = kernel(**inputs)
    ref = np.load('/tmp/ref_out.npy')
    rel = np.linalg.norm(out - ref) / np.linalg.norm(ref)
    print('rel l2 err:', rel)


# revision 20
# speedup vs baseline: 38.5669x; 38.5669x over previous
"""Trainium2 Bass kernel for nn_AGNN (3-frame attentional GNN + ConvGRU).

Self-contained: builds an 8-core SPMD Bass graph (sequence-parallel over the
48x48 spatial tokens, 6 rows per core), runs it via run_bass_kernel_spmd,
and reassembles the full output.

Sharding: each core owns 6 rows (288 tokens) of every frame. Per iteration:
  AllGather h (bf16, ch-major + tok-major layouts) -> each core computes
  attention for its 288 query tokens against all 2304 keys of each frame
  (9 ordered frame pairs), gated aggregation, then a 4-row boundary
  AllGather (magg + h) feeds the halo rows of the 3x3 ConvGRU which each
  core evaluates for its own rows.  Readout convs are local (v computed
  with halo from the raw frames; h halo from a final boundary exchange).

Precision: attention matmuls in bf16 (fp32 PSUM accumulation), everything
else float32r (tf32 matmul mode).  Validated ~5.5e-4 rel error vs the
fp32 reference in simulation.
"""
import sys
import numpy as np

NF = 3          # frames
C = 256         # channels
HF = WF = 48    # feature map
P = HF * WF     # 2304 tokens/frame
NCORES = 8
RW = 6          # rows per core
PL = RW * WF    # 288 tokens per core
K_ITERS = 3

_CACHE = {}


def _build_graph():
    sys.path.insert(0, '/opt/trn_rl_repo')
    import concourse.bass as bass
    import concourse.mybir as mybir
    import concourse.tile as tile
    from concourse import bacc

    dt = mybir.dt
    f32 = dt.float32
    f32r = dt.float32r
    bf16 = dt.bfloat16
    AF = mybir.ActivationFunctionType
    OP = mybir.AluOpType
    RG = [list(range(NCORES))]

    nc = bacc.Bacc()

    # ---------------- external IO ----------------
    def ein(name, shape, dtype=None):
        return nc.dram_tensor(name, list(shape), dtype or f32, kind="ExternalInput")

    patches = ein("patches", (192, NF, 480), f32r)       # host patch-extract, rows 6k-2..6k+7
    bbw0 = ein("bbw0", (128, C), f32r)
    bbw1 = ein("bbw1", (64, C), f32r)
    bbb = ein("bbb", (128, 2))
    bbb_top = ein("bbb_top", (128, 2))
    bbb_bot = ein("bbb_bot", (128, 2))
    w_int = ein("w_int", (128, 2, C), f32r)              # W_inter [c, d] swizzled
    w_inta = ein("w_inta", (128, 2, C), f32r)
    gw_t = ein("gw_t", (128, 2, C), f32r)                # gate_w^T [i, o] swizzled
    gb_neg = ein("gb_neg", (128, 2))                       # -gate_b
    wz = ein("wz", (9, 128, 2, 4, 128), f32r)            # [tap, kp, mt, kt, mp]
    wr = ein("wr", (9, 128, 2, 4, 128), f32r)
    wh = ein("wh", (9, 128, 2, 4, 128), f32r)
    bz = ein("bz", (128, 2))
    br = ein("br", (128, 2))
    bh = ein("bh", (128, 2))
    ro1 = ein("ro1", (9, 128, 2, 4, 128), f32r)
    rb1 = ein("rb1", (128, 2))
    rb1_top = ein("rb1_top", (128, 2))
    rb1_bot = ein("rb1_bot", (128, 2))
    ro2 = ein("ro2", (128, 2, 9), f32r)
    rb2 = ein("rb2", (1, 1))
    sel_top = ein("sel_top", (128, 8))                   # one-hot rank k-1 (zeros at core 0)
    sel_bot = ein("sel_bot", (128, 8))                   # one-hot rank k+1 (zeros at core 7)
    sc_top = ein("sc_top", (128, 1))                     # 0.0 on core 0 else 1.0
    sc_bot = ein("sc_bot", (128, 1))                     # 0.0 on core 7 else 1.0
    ident_in = ein("ident_in", (128, 128), f32r)

    out_ext = nc.dram_tensor("out", [NF, RW, WF], f32, kind="ExternalOutput")

    SH = NF * C * PL            # 221184 elements per layout shard

    with tile.TileContext(nc) as tc:
        with (
            tc.tile_pool(name="pers", bufs=1) as pers,
            tc.tile_pool(name="dram", bufs=1, space="DRAM") as dram,
            tc.tile_pool(name="psum", bufs=1, space="PSUM") as psum,
            tc.tile_pool(name="work", bufs=1) as work,
        ):
            # ---------------- persistent SBUF ----------------
            ident = pers.tile([128, 128], f32r)
            nc.sync.dma_start(ident[...], ident_in.ap())

            def load_pers(name, ext, shape, view=None):
                t = pers.tile(list(shape), f32r, name=name)
                src = ext.ap() if view is None else view
                nc.sync.dma_start(t[...], src)
                return t

            # W_inter/W_intra/gate_w^T as [128, 2ct, 256]
            wint_t = load_pers("wint_t", w_int, (128, 2, C))
            winta_t = load_pers("winta_t", w_inta, (128, 2, C))
            gw_tt = load_pers("gw_tt", gw_t, (128, 2, C))
            bbw0_t = load_pers("bbw0_t", bbw0, (128, C))
            bbw1_t = load_pers("bbw1_t", bbw1, (64, C))
            ro2_t = load_pers("ro2_t", ro2, (128, 2, 9))

            def load_bias(name, ext):
                t = pers.tile([128, 2], f32, name=name)
                nc.sync.dma_start(t[...], ext.ap())
                return t

            bbb_t = load_bias("bbb_t", bbb)
            bbbtop_t = load_bias("bbbtop_t", bbb_top)
            bbbbot_t = load_bias("bbbbot_t", bbb_bot)
            gnb_t = load_bias("gnb_t", gb_neg)
            bz_t = load_bias("bz_t", bz)
            br_t = load_bias("br_t", br)
            bh_t = load_bias("bh_t", bh)
            rb1_t = load_bias("rb1_t", rb1)
            rb1top_t = load_bias("rb1top_t", rb1_top)
            rb1bot_t = load_bias("rb1bot_t", rb1_bot)
            rb2_t = pers.tile([1, 1], f32)
            nc.sync.dma_start(rb2_t[...], rb2.ap())
            seltop_t = pers.tile([128, 8], f32)
            nc.sync.dma_start(seltop_t[...], sel_top.ap())
            selbot_t = pers.tile([128, 8], f32)
            nc.sync.dma_start(selbot_t[...], sel_bot.ap())
            sctop_t = pers.tile([128, 1], f32)
            nc.sync.dma_start(sctop_t[...], sc_top.ap())
            scbot_t = pers.tile([128, 1], f32)
            nc.sync.dma_start(scbot_t[...], sc_bot.ap())

            # big persistent state
            v_sb = pers.tile([128, 2, NF, 10 * WF], f32)      # v rows 6k-2..6k+7
            h_loc = pers.tile([128, 2, NF, PL], f32r)          # own rows, ch-major
            magg = pers.tile([128, 2, NF, PL], f32r)
            t_sb = pers.tile([128, 2, 6, PL], bf16)           # t^T (3 inter + 3 intra)

            def r32(ap):
                return ap.bitcast(f32r)

            # ---------------- backbone ----------------
            with tc.tile_pool(name="bb", bufs=1) as bb:
                pk0 = bb.tile([128, NF, 480], f32r, name="pk0")
                pk1 = bb.tile([64, NF, 480], f32r, name="pk1")
                nc.sync.dma_start(pk0[...], patches[0:128])
                nc.sync.dma_start(pk1[...], patches[128:192])
                for f in range(NF):
                    vps = psum.tile([128, 2, 512], f32, tag="pe", bufs=2, name="vps")
                    for mt in range(2):
                        nc.tensor.matmul(vps[:, mt, 0:480],
                                         bbw0_t[:, mt * 128:(mt + 1) * 128],
                                         pk0[:, f, :], start=True, stop=False)
                        nc.tensor.matmul(vps[:, mt, 0:480],
                                         bbw1_t[0:64, mt * 128:(mt + 1) * 128],
                                         pk1[0:64, f, :], start=False, stop=True)
                    for mt in range(2):
                        # rows 0-1 / 2-7 / 8-9 with edge masking (v=0 outside image)
                        nc.scalar.activation(v_sb[:, mt, f, 0:96], vps[:, mt, 0:96],
                                             AF.Relu, bias=bbbtop_t[:, mt:mt + 1], scale=sctop_t[:, :])
                        nc.scalar.activation(v_sb[:, mt, f, 96:384], vps[:, mt, 96:384],
                                             AF.Relu, bias=bbb_t[:, mt:mt + 1])
                        nc.scalar.activation(v_sb[:, mt, f, 384:480], vps[:, mt, 384:480],
                                             AF.Relu, bias=bbbbot_t[:, mt:mt + 1], scale=scbot_t[:, :])
                        # h0 = v own rows (rows 2..8 of the 10-row window)
                        nc.vector.tensor_copy(h_loc[:, mt, f, :], v_sb[:, mt, f, 96:384])

            # ---------------- iterations ----------------
            for it in range(K_ITERS):
                # ---- write AG input: ch-major + tok-major (bf16) ----
                agi = dram.tile([2, NF, SH // NF], bf16, tag="agi", bufs=2, name="agi")
                hloc16 = work.tile([128, 2, NF, PL], bf16, tag="hloc16", bufs=1, name="hloc16")
                nc.vector.tensor_copy(hloc16[...], h_loc[...])
                for f in range(NF):
                    for ct in range(2):
                        nc.sync.dma_start(
                            agi[0, f].rearrange("(c t) -> c t", t=PL)[ct * 128:(ct + 1) * 128, :],
                            hloc16[:, ct, f, :])
                    # tok-major via TensorE transposes: [c,96tok] -> [96tok, 128c]
                    hlt = work.tile([96, 3, C], bf16, tag="hlt", bufs=2, name="hlt")
                    for ct in range(2):
                        for ps in range(3):
                            tp = psum.tile([96, 128], f32r, tag="aux", bufs=1, name="tp")
                            nc.tensor.transpose(
                                tp[0:96, 0:128],
                                h_loc[:, ct, f, ps * 96:(ps + 1) * 96],
                                ident[:, :])
                            nc.vector.tensor_copy(hlt[0:96, ps, ct * 128:(ct + 1) * 128],
                                                  tp[0:96, 0:128])
                    nc.sync.dma_start(
                        agi[1, f].rearrange("(t c) -> t c", c=C)
                        .rearrange("(ps p) c -> p ps c", p=96),
                        hlt[0:96, :, :])

                ago = dram.tile([NCORES, 2, NF, SH // NF], bf16, tag="ago", bufs=2,
                                addr_space="Shared", name="ago")
                nc.gpsimd.collective_compute(
                    "AllGather", OP.bypass, replica_groups=RG,
                    ins=[agi.opt()], outs=[ago.opt()])

                # ---- t = h_loc @ W (both kinds), bf16 out ----
                for i in range(NF):
                    for kind in range(2):       # 0 inter, 1 intra
                        wsel = wint_t if kind == 0 else winta_t
                        for dct in range(2):
                            tps = psum.tile([128, 2, 512], f32, tag="pe", bufs=2, name="tps")
                            for ct in range(2):
                                nc.tensor.matmul(
                                    tps[:, 0, 0:288],
                                    r32(wsel[:, ct, dct * 128:(dct + 1) * 128]),
                                    r32(h_loc[:, ct, i, :]),
                                    start=(ct == 0), stop=(ct == 1))
                            nc.vector.tensor_copy(t_sb[:, dct, kind * 3 + i, :],
                                                  tps[:, 0, 0:288])

                # ---- boundary AG input: magg written later; h part now ----
                bdi = dram.tile([2, NF, C, 4 * WF], f32r, tag="bdi", bufs=2, name="bdi")
                for f in range(NF):
                    for ct in range(2):
                        nc.sync.dma_start(
                            bdi[1, f, ct * 128:(ct + 1) * 128, 0:96], h_loc[:, ct, f, 0:96])
                        nc.sync.dma_start(
                            bdi[1, f, ct * 128:(ct + 1) * 128, 96:192], h_loc[:, ct, f, 192:288])

                # ---- attention over j (keys) and i (queries) ----
                for j in range(NF):
                    # stage frame j contiguously in DRAM (rank chunks are 288
                    # tokens; 128-token tiles cross rank boundaries otherwise)
                    stg = dram.tile([P, C], bf16, tag="stg", bufs=2, name="stg")
                    nc.sync.dma_start(stg[:, :], ago[:, 1, j])
                    stgc = dram.tile([C, P], bf16, tag="stgc", bufs=2, name="stgc")
                    nc.sync.dma_start(
                        stgc[:, :].rearrange("c (r t) -> r c t", r=NCORES),
                        ago[:, 0, j])
                    hch = []
                    for hh in range(2):
                        t_ = work.tile([128, 2, 9, 128], bf16, tag="hch", bufs=2, name="hch")
                        for ct in range(2):
                            nc.sync.dma_start(
                                t_[:, ct, :, :],
                                stgc[ct * 128:(ct + 1) * 128, :]
                                .rearrange("p (q x) -> p q x", x=128)
                                [:, hh * 9:(hh + 1) * 9, :])
                        hch.append(t_)
                    htok = []
                    for hh in range(2):
                        t_ = work.tile([128, 9, 257], bf16, tag="htok", bufs=2, name="htok")
                        nc.gpsimd.memset(t_[...], 1.0)
                        nc.sync.dma_start(
                            t_[:, :, 0:256],
                            stg[:, :].rearrange("(q p) c -> p q c", p=128)
                            [:, hh * 9:(hh + 1) * 9, :])
                        htok.append(t_)

                    for i in range(NF):
                        tix = (3 + i) if i == j else i
                        attn = work.tile([128, 18, 288], bf16, tag="attn", bufs=2, name="attn")
                        mu0 = psum.tile([128, 2, 512], f32, tag="mu0", bufs=1, name="mu0")
                        mu2 = psum.tile([32, 257], f32, tag="mu2", bufs=1, name="mu2")
                        # software-pipelined: e/exp group g, then m-matmuls of g-1
                        for g in range(10):
                            if g < 9:
                                e2 = psum.tile([128, 2, 512], f32, tag="pe", bufs=2, name="e2")
                                for u in range(2):
                                    q = g * 2 + u
                                    for ct in range(2):
                                        nc.tensor.matmul(
                                            e2[:, u, 0:288],
                                            hch[q // 9][:, ct, q % 9, :],
                                            t_sb[:, ct, tix, :],
                                            start=(ct == 0), stop=(ct == 1))
                                nc.scalar.activation(attn[:, g * 2:g * 2 + 2, :],
                                                     e2[:, :, 0:288], AF.Exp)
                            if g >= 1:
                                for u in range(2):
                                    q = (g - 1) * 2 + u
                                    st = (q == 0)
                                    sp = (q == 17)
                                    mv = htok[q // 9][:, q % 9, :]
                                    nc.tensor.matmul(mu0[:, 0, 0:257], attn[:, q, 0:128],
                                                     mv, start=st, stop=sp)
                                    nc.tensor.matmul(mu0[:, 1, 0:257], attn[:, q, 128:256],
                                                     mv, start=st, stop=sp)
                                    nc.tensor.matmul(mu2[0:32, 0:257], attn[:, q, 256:288],
                                                     mv, start=st, stop=sp)
                        # normalize m (softmax denominator = col 256)
                        mnorm = work.tile([128, 3, 256], f32r, tag="mnorm", bufs=1, name="mnorm")
                        rs = work.tile([128, 3, 1], f32, tag="rs", bufs=2, name="rs")
                        for s in range(3):
                            mus = mu0[:, s, :] if s < 2 else mu2[0:32, :]
                            pp = 128 if s < 2 else 32
                            nc.vector.reciprocal(rs[0:pp, s, :], mus[0:pp, 256:257])
                            nc.vector.tensor_scalar(mnorm[0:pp, s, :], mus[0:pp, 0:256],
                                                    rs[0:pp, s, :], None, OP.mult)
                        # transpose m -> ch-major
                        mT = work.tile([128, 2, 288], f32r, tag="mT", bufs=2, name="mT")
                        for ct in range(2):
                            tps = psum.tile([128, 288], f32r, tag="aux", bufs=1, name="mtp")
                            for s in range(3):
                                pp = 128 if s < 2 else 32
                                nc.tensor.transpose(
                                    tps[:, s * 128:s * 128 + pp],
                                    mnorm[0:pp, s, ct * 128:(ct + 1) * 128],
                                    ident[0:pp, 0:pp])
                            nc.vector.tensor_copy(mT[:, ct, :], tps[:, 0:288])
                        # gate: g = sigmoid(gate_w m + b) via exp (stay on exp table)
                        gps = psum.tile([128, 2, 512], f32, tag="mu0", bufs=1, name="gps")
                        for oct in range(2):
                            for ict in range(2):
                                nc.tensor.matmul(
                                    gps[:, oct, 0:288],
                                    r32(gw_tt[:, ict, oct * 128:(oct + 1) * 128]),
                                    r32(mT[:, ict, :]),
                                    start=(ict == 0), stop=(ict == 1))
                        gtmp = work.tile([128, 2, 288], f32, tag="gtmp", bufs=1, name="gtmp")
                        for oct in range(2):
                            nc.scalar.activation(gtmp[:, oct, :], gps[:, oct, 0:288],
                                                 AF.Exp, bias=gnb_t[:, oct:oct + 1], scale=-1.0)
                        nc.vector.tensor_scalar(gtmp[...], gtmp[...], 1.0, None, OP.add)
                        nc.vector.reciprocal(gtmp[...], gtmp[...])
                        nc.vector.tensor_tensor(gtmp[...], gtmp[...], mT[...], OP.mult)
                        if j == 0:
                            nc.vector.tensor_copy(magg[:, :, i, :], gtmp[...])
                        else:
                            nc.vector.tensor_tensor(magg[:, :, i, :], magg[:, :, i, :],
                                                    gtmp[...], OP.add)

                # ---- boundary AG (magg + h 2-row halos) ----
                for f in range(NF):
                    for ct in range(2):
                        nc.sync.dma_start(
                            bdi[0, f, ct * 128:(ct + 1) * 128, 0:96], magg[:, ct, f, 0:96])
                        nc.sync.dma_start(
                            bdi[0, f, ct * 128:(ct + 1) * 128, 96:192], magg[:, ct, f, 192:288])
                bdo = dram.tile([NCORES, 2, NF, C, 4 * WF], f32r, tag="bdo", bufs=2,
                                addr_space="Shared", name="bdo")
                nc.gpsimd.collective_compute(
                    "AllGather", OP.bypass, replica_groups=RG,
                    ins=[bdi.opt()], outs=[bdo.opt()])

                # ---- halo extraction via one-hot rank masks ----
                # halo[kind][ct]: top rows (from rank k-1 bottom seg) & bottom rows
                halo = work.tile([128, 2, 2, NF, 2, 96], f32r, tag="halo", bufs=1, name="halo")
                for kind in range(2):
                    for ct in range(2):
                        for rp in range(4):
                            ch = work.tile([128, 2, NF, 192], f32r, tag="hchk", bufs=1, name="hchk")
                            for _rr in range(2):
                                nc.sync.dma_start(
                                    ch[:, _rr, :, :],
                                    bdo[rp * 2 + _rr, kind, :, ct * 128:(ct + 1) * 128, :]
                                    .rearrange("f c x -> c f x"))
                            for rr in range(2):
                                r = rp * 2 + rr
                                for tb in range(2):
                                    sel = seltop_t if tb == 0 else selbot_t
                                    seg = ch[:, rr, :, 96:192] if tb == 0 else ch[:, rr, :, 0:96]
                                    dst = halo[:, kind, ct, :, tb, :]
                                    if r == 0:
                                        nc.vector.tensor_scalar(dst, seg, sel[:, 0:1],
                                                                None, OP.mult)
                                    else:
                                        nc.vector.scalar_tensor_tensor(
                                            dst, seg, sel[:, r:r + 1], dst,
                                            OP.mult, OP.add)

                # ---- ConvGRU ----
                mh = []
                for f in range(NF):
                    m_ = work.tile([128, 4, 512], f32r, tag="mh", bufs=3, name="mh")
                    nc.gpsimd.memset(m_[...].bitcast(f32), 0.0)
                    for ct in range(2):
                        rows = m_[:, ct, 6:506].rearrange("p (r w) -> p r w", w=50)
                        nc.vector.tensor_copy(
                            rows[:, 2:8, 1:49],
                            magg[:, ct, f, :].rearrange("p (r w) -> p r w", w=48))
                        nc.vector.tensor_copy(
                            rows[:, 0:2, 1:49],
                            halo[:, 0, ct, f, 0, :].rearrange("p (r w) -> p r w", w=48))
                        nc.vector.tensor_copy(
                            rows[:, 8:10, 1:49],
                            halo[:, 0, ct, f, 1, :].rearrange("p (r w) -> p r w", w=48))
                        hrows = m_[:, 2 + ct, 6:506].rearrange("p (r w) -> p r w", w=50)
                        nc.vector.tensor_copy(
                            hrows[:, 2:8, 1:49],
                            h_loc[:, ct, f, :].rearrange("p (r w) -> p r w", w=48))
                        nc.vector.tensor_copy(
                            hrows[:, 0:2, 1:49],
                            halo[:, 1, ct, f, 0, :].rearrange("p (r w) -> p r w", w=48))
                        nc.vector.tensor_copy(
                            hrows[:, 8:10, 1:49],
                            halo[:, 1, ct, f, 1, :].rearrange("p (r w) -> p r w", w=48))
                    mh.append(m_)

                def conv(wext, psums, NOUT, row0, src_of):
                    """9-tap conv: psums[f][:, mt, 0:NOUT] += taps."""
                    for tap in range(9):
                        dy, dx = tap // 3 - 1, tap % 3 - 1
                        wt = work.tile([128, 2, 4, 128], f32r, tag="wtap", bufs=2, name="wtap")
                        nc.sync.dma_start(wt[...], wext[tap])
                        for f in range(NF):
                            for kt in range(4):
                                mvs = src_of(f, kt, dy, dx)
                                if mvs is None:
                                    continue
                                for mt in range(2):
                                    nc.tensor.matmul(
                                        psums[f][:, mt, 0:NOUT],
                                        wt[:, mt, kt, :], r32(mvs),
                                        start=(tap == 0 and kt == 0),
                                        stop=(tap == 8 and kt == 3))

                def conv_psums():
                    ps = []
                    for f in range(NF):
                        tag = "pe" if f < 2 else "mu0"
                        ps.append(psum.tile([128, 2, 512], f32, tag=tag, bufs=2 if f < 2 else 1,
                                            name=f"cps{f}"))
                    return ps

                # z conv: out rows W2..W7 (own), N=300
                zps = conv_psums()
                conv(wz, zps, 300, 2,
                     lambda f, kt, dy, dx: mh[f][:, kt, 6 + (2 + dy) * 50 + dx:
                                                 6 + (2 + dy) * 50 + dx + 300])
                zgs = []
                for f in range(NF):
                    z_ = work.tile([128, 2, 300], f32, tag=f"zgs{f}", bufs=1, name="zgs")
                    for mt in range(2):
                        nc.scalar.activation(z_[:, mt, :], zps[f][:, mt, 0:300],
                                             AF.Sigmoid, bias=bz_t[:, mt:mt + 1])
                    zgs.append(z_)
                # r conv: out rows W1..W8, N=400
                rps = conv_psums()
                conv(wr, rps, 400, 1,
                     lambda f, kt, dy, dx: mh[f][:, kt, 6 + (1 + dy) * 50 + dx:
                                                 6 + (1 + dy) * 50 + dx + 400])
                rgh = []
                for f in range(NF):
                    # rg sigmoid written at 6-offset, then rg*h in place
                    rh_ = work.tile([128, 2, 416], f32r, tag=f"rgh{f}", bufs=1, name="rgh")
                    nc.gpsimd.memset(rh_[...].bitcast(f32), 0.0)
                    for mt in range(2):
                        nc.scalar.activation(rh_[:, mt, 6:406], rps[f][:, mt, 0:400],
                                             AF.Sigmoid, bias=br_t[:, mt:mt + 1])
                    nc.vector.tensor_tensor(
                        rh_[:, :, 6:406], rh_[:, :, 6:406],
                        mh[f][:, 2:4, 56:456], OP.mult)
                    rgh.append(rh_)
                # candidate conv: out rows W2..W7, N=300; inputs kt0-1 magg, kt2-3 rg*h
                hps = conv_psums()

                def hc_src(f, kt, dy, dx):
                    if kt < 2:
                        o = 6 + (2 + dy) * 50 + dx
                        return mh[f][:, kt, o:o + 300]
                    o = 6 + (1 + dy) * 50 + dx
                    return rgh[f][:, kt - 2, o:o + 300]

                conv(wh, hps, 300, 2, hc_src)
                for f in range(NF):
                    hc_ = work.tile([128, 2, 300], f32, tag="hcs", bufs=2, name="hcs")
                    for mt in range(2):
                        nc.scalar.activation(hc_[:, mt, :], hps[f][:, mt, 0:300],
                                             AF.Tanh, bias=bh_t[:, mt:mt + 1])
                    # h_new = h + z*(hc - h)
                    hold = mh[f][:, 2:4, 106:406]
                    nc.vector.tensor_tensor(hc_[...], hc_[...], hold, OP.subtract)
                    nc.vector.tensor_tensor(hc_[...], hc_[...], zgs[f][...], OP.mult)
                    for ct in range(2):
                        nc.vector.tensor_tensor(
                            h_loc[:, ct, f, :].rearrange("p (r w) -> p r w", w=48),
                            mh[f][:, 2 + ct, 106:406].rearrange("p (r w) -> p r w", w=50)[:, :, 1:49],
                            hc_[:, ct, :].rearrange("p (r w) -> p r w", w=50)[:, :, 1:49],
                            OP.add)

            # ---------------- readout ----------------
            bdi2 = dram.tile([NF, C, 4 * WF], f32r, tag="bdi", bufs=2, name="bdi2")
            for f in range(NF):
                for ct in range(2):
                    nc.sync.dma_start(bdi2[f, ct * 128:(ct + 1) * 128, 0:96],
                                      h_loc[:, ct, f, 0:96])
                    nc.sync.dma_start(bdi2[f, ct * 128:(ct + 1) * 128, 96:192],
                                      h_loc[:, ct, f, 192:288])
            bdo2 = dram.tile([NCORES, NF, C, 4 * WF], f32r, tag="bdo", bufs=2,
                             addr_space="Shared", name="bdo2")
            nc.gpsimd.collective_compute(
                "AllGather", OP.bypass, replica_groups=RG,
                ins=[bdi2.opt()], outs=[bdo2.opt()])
            halo2 = work.tile([128, 2, NF, 2, 96], f32r, tag="halo", bufs=1, name="halo2")
            for ct in range(2):
                for rp in range(4):
                    ch = work.tile([128, 2, NF, 192], f32r, tag="hchk", bufs=1, name="hchk2")
                    for _rr in range(2):
                        nc.sync.dma_start(
                            ch[:, _rr, :, :],
                            bdo2[rp * 2 + _rr, :, ct * 128:(ct + 1) * 128, :]
                            .rearrange("f c x -> c f x"))
                    for rr in range(2):
                        r = rp * 2 + rr
                        for tb in range(2):
                            sel = seltop_t if tb == 0 else selbot_t
                            seg = ch[:, rr, :, 96:192] if tb == 0 else ch[:, rr, :, 0:96]
                            dst = halo2[:, ct, :, tb, :]
                            if r == 0:
                                nc.vector.tensor_scalar(dst, seg, sel[:, 0:1], None, OP.mult)
                            else:
                                nc.vector.scalar_tensor_tensor(
                                    dst, seg, sel[:, r:r + 1], dst, OP.mult, OP.add)

            mask_sb = pers.tile([1, NF, RW * WF], f32)
            for f in range(NF):
                ro_in = work.tile([128, 4, 512], f32r, tag="mh", bufs=3, name="ro_in")
                nc.gpsimd.memset(ro_in[...].bitcast(f32), 0.0)
                for ct in range(2):
                    hrows = ro_in[:, ct, 6:506].rearrange("p (r w) -> p r w", w=50)
                    nc.vector.tensor_copy(
                        hrows[:, 2:8, 1:49],
                        h_loc[:, ct, f, :].rearrange("p (r w) -> p r w", w=48))
                    nc.vector.tensor_copy(
                        hrows[:, 0:2, 1:49],
                        halo2[:, ct, f, 0, :].rearrange("p (r w) -> p r w", w=48))
                    nc.vector.tensor_copy(
                        hrows[:, 8:10, 1:49],
                        halo2[:, ct, f, 1, :].rearrange("p (r w) -> p r w", w=48))
                    vrows = ro_in[:, 2 + ct, 6:506].rearrange("p (r w) -> p r w", w=50)
                    nc.vector.tensor_copy(
                        vrows[:, 0:10, 1:49],
                        v_sb[:, ct, f, :].rearrange("p (r w) -> p r w", w=48))
                # y = relu(ro1 * cat) rows W1..W8 (N=400)
                yps = psum.tile([128, 2, 512], f32, tag="pe", bufs=2, name="yps")
                for tap in range(9):
                    dy, dx = tap // 3 - 1, tap % 3 - 1
                    wt = work.tile([128, 2, 4, 128], f32r, tag="wtap", bufs=2, name="wtap2")
                    nc.sync.dma_start(wt[...], ro1[tap])
                    o = 6 + (1 + dy) * 50 + dx
                    for kt in range(4):
                        for mt in range(2):
                            nc.tensor.matmul(
                                yps[:, mt, 0:400], wt[:, mt, kt, :],
                                r32(ro_in[:, kt, o:o + 400]),
                                start=(tap == 0 and kt == 0), stop=(tap == 8 and kt == 3))
                y_sb = work.tile([128, 2, 412], f32r, tag="y_sb", bufs=1, name="y_sb")
                nc.gpsimd.memset(y_sb[...].bitcast(f32), 0.0)
                for mt in range(2):
                    ypr = yps[:, mt, 0:400].rearrange("p (r w) -> p r w", w=50)
                    ydst = y_sb[:, mt, 6:406].rearrange("p (r w) -> p r w", w=50)
                    # y rows: 0 (global 6k-1, invalid on core 0), 1..7, 7 (invalid on core 7)
                    nc.scalar.activation(ydst[:, 0:1, 1:49], ypr[:, 0:1, 1:49], AF.Relu,
                                         bias=rb1top_t[:, mt:mt + 1], scale=sctop_t[:, :])
                    nc.scalar.activation(ydst[:, 1:7, 1:49], ypr[:, 1:7, 1:49], AF.Relu,
                                         bias=rb1_t[:, mt:mt + 1])
                    nc.scalar.activation(ydst[:, 7:8, 1:49], ypr[:, 7:8, 1:49], AF.Relu,
                                         bias=rb1bot_t[:, mt:mt + 1], scale=scbot_t[:, :])
                # mask = ro2 * y + b2, own rows (N=300 in 50-col layout; y pad
                # cols are zero so the windows are safe)
                mps = psum.tile([1, 300], f32, tag="aux", bufs=1, name="mps")
                for tap in range(9):
                    dy, dx = tap // 3 - 1, tap % 3 - 1
                    o = 6 + (1 + dy) * 50 + dx
                    for ct in range(2):
                        nc.tensor.matmul(
                            mps[0:1, 0:300],
                            r32(ro2_t[:, ct, tap:tap + 1]),
                            r32(y_sb[:, ct, o:o + 300]),
                            start=(tap == 0 and ct == 0), stop=(tap == 8 and ct == 1))
                nc.scalar.activation(
                    mask_sb[0:1, f, :].rearrange("p (r w) -> p r w", w=48),
                    mps[0:1, :].rearrange("p (r w) -> p r w", w=50)[:, :, 1:49],
                    AF.Identity, bias=rb2_t[0:1, :])
            nc.sync.dma_start(out_ext.ap(), mask_sb[0:1, :, :])

    nc.finalize()
    return nc


def _prep_inputs(inputs):
    """Host-side weight prep + per-core shards."""
    frames = np.ascontiguousarray(inputs['frames'], dtype=np.float32)  # (1,3,3,384,384)
    bb_w = np.asarray(inputs['backbone_w'], dtype=np.float32)
    bb_b = np.asarray(inputs['backbone_b'], dtype=np.float32).reshape(C, 1)
    W_intra = np.asarray(inputs['W_intra'], dtype=np.float32)
    W_inter = np.asarray(inputs['W_inter'], dtype=np.float32)
    gate_w = np.asarray(inputs['gate_w'], dtype=np.float32)[:, :, 0, 0]
    gate_b = np.asarray(inputs['gate_b'], dtype=np.float32).reshape(C, 1)

    def taps(w):
        return np.ascontiguousarray(
            np.asarray(w, dtype=np.float32).transpose(2, 3, 1, 0).reshape(9, 2 * C, C))

    def swz(w):
        """[c, d] (256x256) -> [128, 2ct, d]"""
        return np.ascontiguousarray(w.reshape(2, 128, C).transpose(1, 0, 2))

    def bias2(b):
        """(256,1) -> (128, 2)"""
        return np.ascontiguousarray(b.reshape(2, 128).T)

    def conv_taps(w):
        """(O,I,3,3) -> [9, kp(128), mt, kt, mp(128)]"""
        t = taps(w)                                    # (9, 512, 256)
        t = t.reshape(9, 4, 128, 2, 128)               # tap, kt, kp, mt, mp
        return np.ascontiguousarray(t.transpose(0, 2, 3, 1, 4))

    bbw192 = bb_w.transpose(1, 2, 3, 0).reshape(192, C)
    ro2_flat = np.asarray(inputs['ro_w2'], dtype=np.float32
                          ).transpose(2, 3, 1, 0).reshape(9, C)      # tap, c
    common = {
        'bbw0': np.ascontiguousarray(bbw192[0:128]),
        'bbw1': np.ascontiguousarray(bbw192[128:192]),
        'bbb': bias2(bb_b),
        'w_int': swz(W_inter),
        'w_inta': swz(W_intra),
        'gw_t': swz(np.ascontiguousarray(gate_w.T)),
        'gb_neg': bias2(-gate_b),
        'wz': conv_taps(inputs['Wz']), 'wr': conv_taps(inputs['Wr']),
        'wh': conv_taps(inputs['Wh']),
        'bz': bias2(np.asarray(inputs['bz'], dtype=np.float32)),
        'br': bias2(np.asarray(inputs['br'], dtype=np.float32)),
        'bh': bias2(np.asarray(inputs['bh'], dtype=np.float32)),
        'ro1': conv_taps(inputs['ro_w1']),
        'rb1': bias2(np.asarray(inputs['ro_b1'], dtype=np.float32)),
        'ro2': np.ascontiguousarray(
            ro2_flat.T.reshape(2, 128, 9).transpose(1, 0, 2)),
        'rb2': np.asarray(inputs['ro_b2'], dtype=np.float32).reshape(1, 1),
        'ident_in': np.eye(128, dtype=np.float32),
    }

    fp = np.zeros((NF, 3, 384 + 32, 384), np.float32)
    fp[:, :, 16:400] = frames[0]

    in_maps = []
    for k in range(NCORES):
        m = dict(common)
        # patches [192=(ch,dy,dx), f, 10 rows x 48]
        pc = fp[:, :, 48 * k:48 * k + 80, :].reshape(NF, 3, 10, 8, 48, 8)
        m['patches'] = np.ascontiguousarray(
            pc.transpose(1, 3, 5, 0, 2, 4).reshape(192, NF, 480))
        st = np.zeros((128, 8), np.float32)
        sb = np.zeros((128, 8), np.float32)
        if k > 0:
            st[:, k - 1] = 1.0
        if k < NCORES - 1:
            sb[:, k + 1] = 1.0
        m['sel_top'] = st
        m['sel_bot'] = sb
        sct = np.full((128, 1), 0.0 if k == 0 else 1.0, np.float32)
        scb = np.full((128, 1), 0.0 if k == NCORES - 1 else 1.0, np.float32)
        m['sc_top'] = sct
        m['sc_bot'] = scb
        m['bbb_top'] = common['bbb'] * sct[0, 0]
        m['bbb_bot'] = common['bbb'] * scb[0, 0]
        m['rb1_top'] = common['rb1'] * sct[0, 0]
        m['rb1_bot'] = common['rb1'] * scb[0, 0]
        in_maps.append(m)
    return in_maps


def run_cores(inputs, trace=False):
    """Returns (per_core_results, BassKernelResults)."""
    sys.path.insert(0, '/opt/trn_rl_repo')
    from concourse.bass_utils import run_bass_kernel_spmd
    if 'nc' not in _CACHE:
        _CACHE['nc'] = _build_graph()
    nc = _CACHE['nc']
    in_maps = _prep_inputs(inputs)
    res = run_bass_kernel_spmd(nc, in_maps, core_ids=list(range(NCORES)), trace=trace)
    return res


def kernel(**inputs):
    res = run_cores(inputs, trace=False)
    out = np.zeros((1, NF, 1, HF, WF), np.float32)
    for k in range(NCORES):
        out[0, :, 0, RW * k:RW * (k + 1), :] = res.results[k]['out']
    return out


if __name__ == '__main__':
    data = np.load('/tmp/ref_inputs.npz')
    inputs = {k: data[k] for k in data.files}
    out = kernel(**inputs)
    ref = np.load('/tmp/ref_out.npy')
    rel = np.linalg.norm(out - ref) / np.linalg.norm(ref)
    print('rel l2 err:', rel)


# revision 21
# speedup vs baseline: 38.5967x; 1.0008x over previous
"""Trainium2 Bass kernel for nn_AGNN (3-frame attentional GNN + ConvGRU).

Self-contained: builds an 8-core SPMD Bass graph (sequence-parallel over the
48x48 spatial tokens, 6 rows per core), runs it via run_bass_kernel_spmd,
and reassembles the full output.

Sharding: each core owns 6 rows (288 tokens) of every frame. Per iteration:
  AllGather h (bf16, ch-major + tok-major layouts) -> each core computes
  attention for its 288 query tokens against all 2304 keys of each frame
  (9 ordered frame pairs), gated aggregation, then a 4-row boundary
  AllGather (magg + h) feeds the halo rows of the 3x3 ConvGRU which each
  core evaluates for its own rows.  Readout convs are local (v computed
  with halo from the raw frames; h halo from a final boundary exchange).

Precision: attention matmuls in bf16 (fp32 PSUM accumulation), everything
else float32r (tf32 matmul mode).  Validated ~5.5e-4 rel error vs the
fp32 reference in simulation.
"""
import sys
import numpy as np

NF = 3          # frames
C = 256         # channels
HF = WF = 48    # feature map
P = HF * WF     # 2304 tokens/frame
NCORES = 8
RW = 6          # rows per core
PL = RW * WF    # 288 tokens per core
K_ITERS = 3

_CACHE = {}


def _build_graph():
    sys.path.insert(0, '/opt/trn_rl_repo')
    import concourse.bass as bass
    import concourse.mybir as mybir
    import concourse.tile as tile
    from concourse import bacc

    dt = mybir.dt
    f32 = dt.float32
    f32r = dt.float32r
    bf16 = dt.bfloat16
    AF = mybir.ActivationFunctionType
    OP = mybir.AluOpType
    RG = [list(range(NCORES))]

    nc = bacc.Bacc()

    # ---------------- external IO ----------------
    def ein(name, shape, dtype=None):
        return nc.dram_tensor(name, list(shape), dtype or f32, kind="ExternalInput")

    patches = ein("patches", (192, NF, 480), f32r)       # host patch-extract, rows 6k-2..6k+7
    bbw0 = ein("bbw0", (128, C), f32r)
    bbw1 = ein("bbw1", (64, C), f32r)
    bbb = ein("bbb", (128, 2))
    bbb_top = ein("bbb_top", (128, 2))
    bbb_bot = ein("bbb_bot", (128, 2))
    w_int = ein("w_int", (128, 2, C), f32r)              # W_inter [c, d] swizzled
    w_inta = ein("w_inta", (128, 2, C), f32r)
    gw_t = ein("gw_t", (128, 2, C), f32r)                # gate_w^T [i, o] swizzled
    gb_neg = ein("gb_neg", (128, 2))                       # -gate_b
    wz = ein("wz", (9, 128, 2, 4, 128), f32r)            # [tap, kp, mt, kt, mp]
    wr = ein("wr", (9, 128, 2, 4, 128), f32r)
    wh = ein("wh", (9, 128, 2, 4, 128), f32r)
    bz = ein("bz", (128, 2))
    br = ein("br", (128, 2))
    bh = ein("bh", (128, 2))
    ro1 = ein("ro1", (9, 128, 2, 4, 128), f32r)
    rb1 = ein("rb1", (128, 2))
    rb1_top = ein("rb1_top", (128, 2))
    rb1_bot = ein("rb1_bot", (128, 2))
    ro2 = ein("ro2", (128, 2, 9), f32r)
    rb2 = ein("rb2", (1, 1))
    sel_top = ein("sel_top", (128, 8))                   # one-hot rank k-1 (zeros at core 0)
    sel_bot = ein("sel_bot", (128, 8))                   # one-hot rank k+1 (zeros at core 7)
    sc_top = ein("sc_top", (128, 1))                     # 0.0 on core 0 else 1.0
    sc_bot = ein("sc_bot", (128, 1))                     # 0.0 on core 7 else 1.0
    ident_in = ein("ident_in", (128, 128), f32r)

    out_ext = nc.dram_tensor("out", [NF, RW, WF], f32, kind="ExternalOutput")

    SH = NF * C * PL            # 221184 elements per layout shard

    with tile.TileContext(nc) as tc:
        with (
            tc.tile_pool(name="pers", bufs=1) as pers,
            tc.tile_pool(name="dram", bufs=1, space="DRAM") as dram,
            tc.tile_pool(name="psum", bufs=1, space="PSUM") as psum,
            tc.tile_pool(name="work", bufs=1) as work,
        ):
            # ---------------- persistent SBUF ----------------
            ident = pers.tile([128, 128], f32r)
            nc.sync.dma_start(ident[...], ident_in.ap())

            def load_pers(name, ext, shape, view=None):
                t = pers.tile(list(shape), f32r, name=name)
                src = ext.ap() if view is None else view
                nc.sync.dma_start(t[...], src)
                return t

            # W_inter/W_intra/gate_w^T as [128, 2ct, 256]
            wint_t = load_pers("wint_t", w_int, (128, 2, C))
            winta_t = load_pers("winta_t", w_inta, (128, 2, C))
            gw_tt = load_pers("gw_tt", gw_t, (128, 2, C))
            bbw0_t = load_pers("bbw0_t", bbw0, (128, C))
            bbw1_t = load_pers("bbw1_t", bbw1, (64, C))
            ro2_t = load_pers("ro2_t", ro2, (128, 2, 9))

            def load_bias(name, ext):
                t = pers.tile([128, 2], f32, name=name)
                nc.sync.dma_start(t[...], ext.ap())
                return t

            bbb_t = load_bias("bbb_t", bbb)
            bbbtop_t = load_bias("bbbtop_t", bbb_top)
            bbbbot_t = load_bias("bbbbot_t", bbb_bot)
            gnb_t = load_bias("gnb_t", gb_neg)
            bz_t = load_bias("bz_t", bz)
            br_t = load_bias("br_t", br)
            bh_t = load_bias("bh_t", bh)
            rb1_t = load_bias("rb1_t", rb1)
            rb1top_t = load_bias("rb1top_t", rb1_top)
            rb1bot_t = load_bias("rb1bot_t", rb1_bot)
            rb2_t = pers.tile([1, 1], f32)
            nc.sync.dma_start(rb2_t[...], rb2.ap())
            seltop_t = pers.tile([128, 8], f32)
            nc.sync.dma_start(seltop_t[...], sel_top.ap())
            selbot_t = pers.tile([128, 8], f32)
            nc.sync.dma_start(selbot_t[...], sel_bot.ap())
            sctop_t = pers.tile([128, 1], f32)
            nc.sync.dma_start(sctop_t[...], sc_top.ap())
            scbot_t = pers.tile([128, 1], f32)
            nc.sync.dma_start(scbot_t[...], sc_bot.ap())

            # big persistent state
            v_sb = pers.tile([128, 2, NF, 10 * WF], f32)      # v rows 6k-2..6k+7
            h_loc = pers.tile([128, 2, NF, PL], f32r)          # own rows, ch-major
            magg = pers.tile([128, 2, NF, PL], f32r)
            t_sb = pers.tile([128, 2, 6, PL], bf16)           # t^T (3 inter + 3 intra)

            def r32(ap):
                return ap.bitcast(f32r)

            # ---------------- backbone ----------------
            with tc.tile_pool(name="bb", bufs=1) as bb:
                pk0 = bb.tile([128, NF, 480], f32r, name="pk0")
                pk1 = bb.tile([64, NF, 480], f32r, name="pk1")
                nc.sync.dma_start(pk0[...], patches[0:128])
                nc.sync.dma_start(pk1[...], patches[128:192])
                for f in range(NF):
                    vps = psum.tile([128, 2, 512], f32, tag="pe", bufs=2, name="vps")
                    for mt in range(2):
                        nc.tensor.matmul(vps[:, mt, 0:480],
                                         bbw0_t[:, mt * 128:(mt + 1) * 128],
                                         pk0[:, f, :], start=True, stop=False)
                        nc.tensor.matmul(vps[:, mt, 0:480],
                                         bbw1_t[0:64, mt * 128:(mt + 1) * 128],
                                         pk1[0:64, f, :], start=False, stop=True)
                    for mt in range(2):
                        # rows 0-1 / 2-7 / 8-9 with edge masking (v=0 outside image)
                        nc.scalar.activation(v_sb[:, mt, f, 0:96], vps[:, mt, 0:96],
                                             AF.Relu, bias=bbbtop_t[:, mt:mt + 1], scale=sctop_t[:, :])
                        nc.scalar.activation(v_sb[:, mt, f, 96:384], vps[:, mt, 96:384],
                                             AF.Relu, bias=bbb_t[:, mt:mt + 1])
                        nc.scalar.activation(v_sb[:, mt, f, 384:480], vps[:, mt, 384:480],
                                             AF.Relu, bias=bbbbot_t[:, mt:mt + 1], scale=scbot_t[:, :])
                        # h0 = v own rows (rows 2..8 of the 10-row window)
                        nc.vector.tensor_copy(h_loc[:, mt, f, :], v_sb[:, mt, f, 96:384])

            # ---------------- iterations ----------------
            for it in range(K_ITERS):
                # ---- write AG input: ch-major + tok-major (bf16) ----
                agi = dram.tile([2, NF, SH // NF], bf16, tag="agi", bufs=2, name="agi")
                hloc16 = work.tile([128, 2, NF, PL], bf16, tag="hloc16", bufs=1, name="hloc16")
                nc.vector.tensor_copy(hloc16[...], h_loc[...])
                for f in range(NF):
                    for ct in range(2):
                        nc.sync.dma_start(
                            agi[0, f].rearrange("(c t) -> c t", t=PL)[ct * 128:(ct + 1) * 128, :],
                            hloc16[:, ct, f, :])
                    # tok-major via TensorE transposes: [c,96tok] -> [96tok, 128c]
                    hlt = work.tile([96, 3, C], bf16, tag="hlt", bufs=2, name="hlt")
                    for ct in range(2):
                        for ps in range(3):
                            tp = psum.tile([96, 128], f32r, tag="aux", bufs=1, name="tp")
                            nc.tensor.transpose(
                                tp[0:96, 0:128],
                                h_loc[:, ct, f, ps * 96:(ps + 1) * 96],
                                ident[:, :])
                            nc.vector.tensor_copy(hlt[0:96, ps, ct * 128:(ct + 1) * 128],
                                                  tp[0:96, 0:128])
                    nc.sync.dma_start(
                        agi[1, f].rearrange("(t c) -> t c", c=C)
                        .rearrange("(ps p) c -> p ps c", p=96),
                        hlt[0:96, :, :])

                ago = dram.tile([NCORES, 2, NF, SH // NF], bf16, tag="ago", bufs=2,
                                addr_space="Shared", name="ago")
                nc.gpsimd.collective_compute(
                    "AllGather", OP.bypass, replica_groups=RG,
                    ins=[agi.opt()], outs=[ago.opt()])

                # ---- t = h_loc @ W (both kinds), bf16 out ----
                for i in range(NF):
                    for kind in range(2):       # 0 inter, 1 intra
                        wsel = wint_t if kind == 0 else winta_t
                        for dct in range(2):
                            tps = psum.tile([128, 2, 512], f32, tag="pe", bufs=2, name="tps")
                            for ct in range(2):
                                nc.tensor.matmul(
                                    tps[:, 0, 0:288],
                                    r32(wsel[:, ct, dct * 128:(dct + 1) * 128]),
                                    r32(h_loc[:, ct, i, :]),
                                    start=(ct == 0), stop=(ct == 1))
                            nc.vector.tensor_copy(t_sb[:, dct, kind * 3 + i, :],
                                                  tps[:, 0, 0:288])

                # ---- boundary AG input: magg written later; h part now ----
                bdi = dram.tile([2, NF, C, 4 * WF], f32r, tag="bdi", bufs=2, name="bdi")
                for f in range(NF):
                    for ct in range(2):
                        nc.sync.dma_start(
                            bdi[1, f, ct * 128:(ct + 1) * 128, 0:96], h_loc[:, ct, f, 0:96])
                        nc.sync.dma_start(
                            bdi[1, f, ct * 128:(ct + 1) * 128, 96:192], h_loc[:, ct, f, 192:288])

                # ---- attention over j (keys) and i (queries) ----
                for j in range(NF):
                    # stage frame j contiguously in DRAM (rank chunks are 288
                    # tokens; 128-token tiles cross rank boundaries otherwise)
                    stg = dram.tile([P, C], bf16, tag="stg", bufs=2, name="stg")
                    nc.sync.dma_start(stg[:, :], ago[:, 1, j])
                    stgc = dram.tile([C, P], bf16, tag="stgc", bufs=2, name="stgc")
                    nc.sync.dma_start(
                        stgc[:, :].rearrange("c (r t) -> r c t", r=NCORES),
                        ago[:, 0, j])
                    hch = []
                    for hh in range(2):
                        t_ = work.tile([128, 2, 9, 128], bf16, tag="hch", bufs=3, name="hch")
                        for ct in range(2):
                            nc.sync.dma_start(
                                t_[:, ct, :, :],
                                stgc[ct * 128:(ct + 1) * 128, :]
                                .rearrange("p (q x) -> p q x", x=128)
                                [:, hh * 9:(hh + 1) * 9, :])
                        hch.append(t_)
                    htok = []
                    for hh in range(2):
                        t_ = work.tile([128, 9, 257], bf16, tag="htok", bufs=3, name="htok")
                        nc.gpsimd.memset(t_[...], 1.0)
                        nc.sync.dma_start(
                            t_[:, :, 0:256],
                            stg[:, :].rearrange("(q p) c -> p q c", p=128)
                            [:, hh * 9:(hh + 1) * 9, :])
                        htok.append(t_)

                    for i in range(NF):
                        tix = (3 + i) if i == j else i
                        attn = work.tile([128, 18, 288], bf16, tag="attn", bufs=2, name="attn")
                        mu0 = psum.tile([128, 2, 512], f32, tag="mu0", bufs=1, name="mu0")
                        mu2 = psum.tile([32, 257], f32, tag="mu2", bufs=1, name="mu2")
                        # software-pipelined: e/exp group g, then m-matmuls of g-1
                        for g in range(10):
                            if g < 9:
                                e2 = psum.tile([128, 2, 512], f32, tag="pe", bufs=2, name="e2")
                                for u in range(2):
                                    q = g * 2 + u
                                    for ct in range(2):
                                        nc.tensor.matmul(
                                            e2[:, u, 0:288],
                                            hch[q // 9][:, ct, q % 9, :],
                                            t_sb[:, ct, tix, :],
                                            start=(ct == 0), stop=(ct == 1))
                                nc.scalar.activation(attn[:, g * 2:g * 2 + 2, :],
                                                     e2[:, :, 0:288], AF.Exp)
                            if g >= 1:
                                for u in range(2):
                                    q = (g - 1) * 2 + u
                                    st = (q == 0)
                                    sp = (q == 17)
                                    mv = htok[q // 9][:, q % 9, :]
                                    nc.tensor.matmul(mu0[:, 0, 0:257], attn[:, q, 0:128],
                                                     mv, start=st, stop=sp)
                                    nc.tensor.matmul(mu0[:, 1, 0:257], attn[:, q, 128:256],
                                                     mv, start=st, stop=sp)
                                    nc.tensor.matmul(mu2[0:32, 0:257], attn[:, q, 256:288],
                                                     mv, start=st, stop=sp)
                        # normalize m (softmax denominator = col 256)
                        mnorm = work.tile([128, 3, 256], f32r, tag="mnorm", bufs=2, name="mnorm")
                        rs = work.tile([128, 3, 1], f32, tag="rs", bufs=2, name="rs")
                        for s in range(3):
                            mus = mu0[:, s, :] if s < 2 else mu2[0:32, :]
                            pp = 128 if s < 2 else 32
                            nc.vector.reciprocal(rs[0:pp, s, :], mus[0:pp, 256:257])
                            nc.vector.tensor_scalar(mnorm[0:pp, s, :], mus[0:pp, 0:256],
                                                    rs[0:pp, s, :], None, OP.mult)
                        # transpose m -> ch-major
                        mT = work.tile([128, 2, 288], f32r, tag="mT", bufs=2, name="mT")
                        for ct in range(2):
                            tps = psum.tile([128, 288], f32r, tag="aux", bufs=1, name="mtp")
                            for s in range(3):
                                pp = 128 if s < 2 else 32
                                nc.tensor.transpose(
                                    tps[:, s * 128:s * 128 + pp],
                                    mnorm[0:pp, s, ct * 128:(ct + 1) * 128],
                                    ident[0:pp, 0:pp])
                            nc.vector.tensor_copy(mT[:, ct, :], tps[:, 0:288])
                        # gate: g = sigmoid(gate_w m + b) via exp (stay on exp table)
                        gps = psum.tile([128, 2, 512], f32, tag="mu0", bufs=1, name="gps")
                        for oct in range(2):
                            for ict in range(2):
                                nc.tensor.matmul(
                                    gps[:, oct, 0:288],
                                    r32(gw_tt[:, ict, oct * 128:(oct + 1) * 128]),
                                    r32(mT[:, ict, :]),
                                    start=(ict == 0), stop=(ict == 1))
                        gtmp = work.tile([128, 2, 288], f32, tag="gtmp", bufs=2, name="gtmp")
                        for oct in range(2):
                            nc.scalar.activation(gtmp[:, oct, :], gps[:, oct, 0:288],
                                                 AF.Exp, bias=gnb_t[:, oct:oct + 1], scale=-1.0)
                        nc.vector.tensor_scalar(gtmp[...], gtmp[...], 1.0, None, OP.add)
                        nc.vector.reciprocal(gtmp[...], gtmp[...])
                        nc.vector.tensor_tensor(gtmp[...], gtmp[...], mT[...], OP.mult)
                        if j == 0:
                            nc.vector.tensor_copy(magg[:, :, i, :], gtmp[...])
                        else:
                            nc.vector.tensor_tensor(magg[:, :, i, :], magg[:, :, i, :],
                                                    gtmp[...], OP.add)

                # ---- boundary AG (magg + h 2-row halos) ----
                for f in range(NF):
                    for ct in range(2):
                        nc.sync.dma_start(
                            bdi[0, f, ct * 128:(ct + 1) * 128, 0:96], magg[:, ct, f, 0:96])
                        nc.sync.dma_start(
                            bdi[0, f, ct * 128:(ct + 1) * 128, 96:192], magg[:, ct, f, 192:288])
                bdo = dram.tile([NCORES, 2, NF, C, 4 * WF], f32r, tag="bdo", bufs=2,
                                addr_space="Shared", name="bdo")
                nc.gpsimd.collective_compute(
                    "AllGather", OP.bypass, replica_groups=RG,
                    ins=[bdi.opt()], outs=[bdo.opt()])

                # ---- halo extraction via one-hot rank masks ----
                # halo[kind][ct]: top rows (from rank k-1 bottom seg) & bottom rows
                halo = work.tile([128, 2, 2, NF, 2, 96], f32r, tag="halo", bufs=1, name="halo")
                for kind in range(2):
                    for ct in range(2):
                        for rp in range(4):
                            ch = work.tile([128, 2, NF, 192], f32r, tag="hchk", bufs=1, name="hchk")
                            for _rr in range(2):
                                nc.sync.dma_start(
                                    ch[:, _rr, :, :],
                                    bdo[rp * 2 + _rr, kind, :, ct * 128:(ct + 1) * 128, :]
                                    .rearrange("f c x -> c f x"))
                            for rr in range(2):
                                r = rp * 2 + rr
                                for tb in range(2):
                                    sel = seltop_t if tb == 0 else selbot_t
                                    seg = ch[:, rr, :, 96:192] if tb == 0 else ch[:, rr, :, 0:96]
                                    dst = halo[:, kind, ct, :, tb, :]
                                    if r == 0:
                                        nc.vector.tensor_scalar(dst, seg, sel[:, 0:1],
                                                                None, OP.mult)
                                    else:
                                        nc.vector.scalar_tensor_tensor(
                                            dst, seg, sel[:, r:r + 1], dst,
                                            OP.mult, OP.add)

                # ---- ConvGRU ----
                mh = []
                for f in range(NF):
                    m_ = work.tile([128, 4, 512], f32r, tag="mh", bufs=3, name="mh")
                    nc.gpsimd.memset(m_[...].bitcast(f32), 0.0)
                    for ct in range(2):
                        rows = m_[:, ct, 6:506].rearrange("p (r w) -> p r w", w=50)
                        nc.vector.tensor_copy(
                            rows[:, 2:8, 1:49],
                            magg[:, ct, f, :].rearrange("p (r w) -> p r w", w=48))
                        nc.vector.tensor_copy(
                            rows[:, 0:2, 1:49],
                            halo[:, 0, ct, f, 0, :].rearrange("p (r w) -> p r w", w=48))
                        nc.vector.tensor_copy(
                            rows[:, 8:10, 1:49],
                            halo[:, 0, ct, f, 1, :].rearrange("p (r w) -> p r w", w=48))
                        hrows = m_[:, 2 + ct, 6:506].rearrange("p (r w) -> p r w", w=50)
                        nc.vector.tensor_copy(
                            hrows[:, 2:8, 1:49],
                            h_loc[:, ct, f, :].rearrange("p (r w) -> p r w", w=48))
                        nc.vector.tensor_copy(
                            hrows[:, 0:2, 1:49],
                            halo[:, 1, ct, f, 0, :].rearrange("p (r w) -> p r w", w=48))
                        nc.vector.tensor_copy(
                            hrows[:, 8:10, 1:49],
                            halo[:, 1, ct, f, 1, :].rearrange("p (r w) -> p r w", w=48))
                    mh.append(m_)

                def conv(wext, psums, NOUT, row0, src_of):
                    """9-tap conv: psums[f][:, mt, 0:NOUT] += taps."""
                    for tap in range(9):
                        dy, dx = tap // 3 - 1, tap % 3 - 1
                        wt = work.tile([128, 2, 4, 128], f32r, tag="wtap", bufs=2, name="wtap")
                        nc.sync.dma_start(wt[...], wext[tap])
                        for f in range(NF):
                            for kt in range(4):
                                mvs = src_of(f, kt, dy, dx)
                                if mvs is None:
                                    continue
                                for mt in range(2):
                                    nc.tensor.matmul(
                                        psums[f][:, mt, 0:NOUT],
                                        wt[:, mt, kt, :], r32(mvs),
                                        start=(tap == 0 and kt == 0),
                                        stop=(tap == 8 and kt == 3))

                def conv_psums():
                    ps = []
                    for f in range(NF):
                        tag = "pe" if f < 2 else "mu0"
                        ps.append(psum.tile([128, 2, 512], f32, tag=tag, bufs=2 if f < 2 else 1,
                                            name=f"cps{f}"))
                    return ps

                # z conv: out rows W2..W7 (own), N=300
                zps = conv_psums()
                conv(wz, zps, 300, 2,
                     lambda f, kt, dy, dx: mh[f][:, kt, 6 + (2 + dy) * 50 + dx:
                                                 6 + (2 + dy) * 50 + dx + 300])
                zgs = []
                for f in range(NF):
                    z_ = work.tile([128, 2, 300], f32, tag=f"zgs{f}", bufs=1, name="zgs")
                    for mt in range(2):
                        nc.scalar.activation(z_[:, mt, :], zps[f][:, mt, 0:300],
                                             AF.Sigmoid, bias=bz_t[:, mt:mt + 1])
                    zgs.append(z_)
                # r conv: out rows W1..W8, N=400
                rps = conv_psums()
                conv(wr, rps, 400, 1,
                     lambda f, kt, dy, dx: mh[f][:, kt, 6 + (1 + dy) * 50 + dx:
                                                 6 + (1 + dy) * 50 + dx + 400])
                rgh = []
                for f in range(NF):
                    # rg sigmoid written at 6-offset, then rg*h in place
                    rh_ = work.tile([128, 2, 416], f32r, tag=f"rgh{f}", bufs=1, name="rgh")
                    nc.gpsimd.memset(rh_[...].bitcast(f32), 0.0)
                    for mt in range(2):
                        nc.scalar.activation(rh_[:, mt, 6:406], rps[f][:, mt, 0:400],
                                             AF.Sigmoid, bias=br_t[:, mt:mt + 1])
                    nc.vector.tensor_tensor(
                        rh_[:, :, 6:406], rh_[:, :, 6:406],
                        mh[f][:, 2:4, 56:456], OP.mult)
                    rgh.append(rh_)
                # candidate conv: out rows W2..W7, N=300; inputs kt0-1 magg, kt2-3 rg*h
                hps = conv_psums()

                def hc_src(f, kt, dy, dx):
                    if kt < 2:
                        o = 6 + (2 + dy) * 50 + dx
                        return mh[f][:, kt, o:o + 300]
                    o = 6 + (1 + dy) * 50 + dx
                    return rgh[f][:, kt - 2, o:o + 300]

                conv(wh, hps, 300, 2, hc_src)
                for f in range(NF):
                    hc_ = work.tile([128, 2, 300], f32, tag="hcs", bufs=2, name="hcs")
                    for mt in range(2):
                        nc.scalar.activation(hc_[:, mt, :], hps[f][:, mt, 0:300],
                                             AF.Tanh, bias=bh_t[:, mt:mt + 1])
                    # h_new = h + z*(hc - h)
                    hold = mh[f][:, 2:4, 106:406]
                    nc.vector.tensor_tensor(hc_[...], hc_[...], hold, OP.subtract)
                    nc.vector.tensor_tensor(hc_[...], hc_[...], zgs[f][...], OP.mult)
                    for ct in range(2):
                        nc.vector.tensor_tensor(
                            h_loc[:, ct, f, :].rearrange("p (r w) -> p r w", w=48),
                            mh[f][:, 2 + ct, 106:406].rearrange("p (r w) -> p r w", w=50)[:, :, 1:49],
                            hc_[:, ct, :].rearrange("p (r w) -> p r w", w=50)[:, :, 1:49],
                            OP.add)

            # ---------------- readout ----------------
            bdi2 = dram.tile([NF, C, 4 * WF], f32r, tag="bdi", bufs=2, name="bdi2")
            for f in range(NF):
                for ct in range(2):
                    nc.sync.dma_start(bdi2[f, ct * 128:(ct + 1) * 128, 0:96],
                                      h_loc[:, ct, f, 0:96])
                    nc.sync.dma_start(bdi2[f, ct * 128:(ct + 1) * 128, 96:192],
                                      h_loc[:, ct, f, 192:288])
            bdo2 = dram.tile([NCORES, NF, C, 4 * WF], f32r, tag="bdo", bufs=2,
                             addr_space="Shared", name="bdo2")
            nc.gpsimd.collective_compute(
                "AllGather", OP.bypass, replica_groups=RG,
                ins=[bdi2.opt()], outs=[bdo2.opt()])
            halo2 = work.tile([128, 2, NF, 2, 96], f32r, tag="halo", bufs=1, name="halo2")
            for ct in range(2):
                for rp in range(4):
                    ch = work.tile([128, 2, NF, 192], f32r, tag="hchk", bufs=1, name="hchk2")
                    for _rr in range(2):
                        nc.sync.dma_start(
                            ch[:, _rr, :, :],
                            bdo2[rp * 2 + _rr, :, ct * 128:(ct + 1) * 128, :]
                            .rearrange("f c x -> c f x"))
                    for rr in range(2):
                        r = rp * 2 + rr
                        for tb in range(2):
                            sel = seltop_t if tb == 0 else selbot_t
                            seg = ch[:, rr, :, 96:192] if tb == 0 else ch[:, rr, :, 0:96]
                            dst = halo2[:, ct, :, tb, :]
                            if r == 0:
                                nc.vector.tensor_scalar(dst, seg, sel[:, 0:1], None, OP.mult)
                            else:
                                nc.vector.scalar_tensor_tensor(
                                    dst, seg, sel[:, r:r + 1], dst, OP.mult, OP.add)

            mask_sb = pers.tile([1, NF, RW * WF], f32)
            for f in range(NF):
                ro_in = work.tile([128, 4, 512], f32r, tag="mh", bufs=3, name="ro_in")
                nc.gpsimd.memset(ro_in[...].bitcast(f32), 0.0)
                for ct in range(2):
                    hrows = ro_in[:, ct, 6:506].rearrange("p (r w) -> p r w", w=50)
                    nc.vector.tensor_copy(
                        hrows[:, 2:8, 1:49],
                        h_loc[:, ct, f, :].rearrange("p (r w) -> p r w", w=48))
                    nc.vector.tensor_copy(
                        hrows[:, 0:2, 1:49],
                        halo2[:, ct, f, 0, :].rearrange("p (r w) -> p r w", w=48))
                    nc.vector.tensor_copy(
                        hrows[:, 8:10, 1:49],
                        halo2[:, ct, f, 1, :].rearrange("p (r w) -> p r w", w=48))
                    vrows = ro_in[:, 2 + ct, 6:506].rearrange("p (r w) -> p r w", w=50)
                    nc.vector.tensor_copy(
                        vrows[:, 0:10, 1:49],
                        v_sb[:, ct, f, :].rearrange("p (r w) -> p r w", w=48))
                # y = relu(ro1 * cat) rows W1..W8 (N=400)
                yps = psum.tile([128, 2, 512], f32, tag="pe", bufs=2, name="yps")
                for tap in range(9):
                    dy, dx = tap // 3 - 1, tap % 3 - 1
                    wt = work.tile([128, 2, 4, 128], f32r, tag="wtap", bufs=2, name="wtap2")
                    nc.sync.dma_start(wt[...], ro1[tap])
                    o = 6 + (1 + dy) * 50 + dx
                    for kt in range(4):
                        for mt in range(2):
                            nc.tensor.matmul(
                                yps[:, mt, 0:400], wt[:, mt, kt, :],
                                r32(ro_in[:, kt, o:o + 400]),
                                start=(tap == 0 and kt == 0), stop=(tap == 8 and kt == 3))
                y_sb = work.tile([128, 2, 412], f32r, tag="y_sb", bufs=1, name="y_sb")
                nc.gpsimd.memset(y_sb[...].bitcast(f32), 0.0)
                for mt in range(2):
                    ypr = yps[:, mt, 0:400].rearrange("p (r w) -> p r w", w=50)
                    ydst = y_sb[:, mt, 6:406].rearrange("p (r w) -> p r w", w=50)
                    # y rows: 0 (global 6k-1, invalid on core 0), 1..7, 7 (invalid on core 7)
                    nc.scalar.activation(ydst[:, 0:1, 1:49], ypr[:, 0:1, 1:49], AF.Relu,
                                         bias=rb1top_t[:, mt:mt + 1], scale=sctop_t[:, :])
                    nc.scalar.activation(ydst[:, 1:7, 1:49], ypr[:, 1:7, 1:49], AF.Relu,
                                         bias=rb1_t[:, mt:mt + 1])
                    nc.scalar.activation(ydst[:, 7:8, 1:49], ypr[:, 7:8, 1:49], AF.Relu,
                                         bias=rb1bot_t[:, mt:mt + 1], scale=scbot_t[:, :])
                # mask = ro2 * y + b2, own rows (N=300 in 50-col layout; y pad
                # cols are zero so the windows are safe)
                mps = psum.tile([1, 300], f32, tag="aux", bufs=1, name="mps")
                for tap in range(9):
                    dy, dx = tap // 3 - 1, tap % 3 - 1
                    o = 6 + (1 + dy) * 50 + dx
                    for ct in range(2):
                        nc.tensor.matmul(
                            mps[0:1, 0:300],
                            r32(ro2_t[:, ct, tap:tap + 1]),
                            r32(y_sb[:, ct, o:o + 300]),
                            start=(tap == 0 and ct == 0), stop=(tap == 8 and ct == 1))
                nc.scalar.activation(
                    mask_sb[0:1, f, :].rearrange("p (r w) -> p r w", w=48),
                    mps[0:1, :].rearrange("p (r w) -> p r w", w=50)[:, :, 1:49],
                    AF.Identity, bias=rb2_t[0:1, :])
            nc.sync.dma_start(out_ext.ap(), mask_sb[0:1, :, :])

    nc.finalize()
    return nc


def _prep_inputs(inputs):
    """Host-side weight prep + per-core shards."""
    frames = np.ascontiguousarray(inputs['frames'], dtype=np.float32)  # (1,3,3,384,384)
    bb_w = np.asarray(inputs['backbone_w'], dtype=np.float32)
    bb_b = np.asarray(inputs['backbone_b'], dtype=np.float32).reshape(C, 1)
    W_intra = np.asarray(inputs['W_intra'], dtype=np.float32)
    W_inter = np.asarray(inputs['W_inter'], dtype=np.float32)
    gate_w = np.asarray(inputs['gate_w'], dtype=np.float32)[:, :, 0, 0]
    gate_b = np.asarray(inputs['gate_b'], dtype=np.float32).reshape(C, 1)

    def taps(w):
        return np.ascontiguousarray(
            np.asarray(w, dtype=np.float32).transpose(2, 3, 1, 0).reshape(9, 2 * C, C))

    def swz(w):
        """[c, d] (256x256) -> [128, 2ct, d]"""
        return np.ascontiguousarray(w.reshape(2, 128, C).transpose(1, 0, 2))

    def bias2(b):
        """(256,1) -> (128, 2)"""
        return np.ascontiguousarray(b.reshape(2, 128).T)

    def conv_taps(w):
        """(O,I,3,3) -> [9, kp(128), mt, kt, mp(128)]"""
        t = taps(w)                                    # (9, 512, 256)
        t = t.reshape(9, 4, 128, 2, 128)               # tap, kt, kp, mt, mp
        return np.ascontiguousarray(t.transpose(0, 2, 3, 1, 4))

    bbw192 = bb_w.transpose(1, 2, 3, 0).reshape(192, C)
    ro2_flat = np.asarray(inputs['ro_w2'], dtype=np.float32
                          ).transpose(2, 3, 1, 0).reshape(9, C)      # tap, c
    common = {
        'bbw0': np.ascontiguousarray(bbw192[0:128]),
        'bbw1': np.ascontiguousarray(bbw192[128:192]),
        'bbb': bias2(bb_b),
        'w_int': swz(W_inter),
        'w_inta': swz(W_intra),
        'gw_t': swz(np.ascontiguousarray(gate_w.T)),
        'gb_neg': bias2(-gate_b),
        'wz': conv_taps(inputs['Wz']), 'wr': conv_taps(inputs['Wr']),
        'wh': conv_taps(inputs['Wh']),
        'bz': bias2(np.asarray(inputs['bz'], dtype=np.float32)),
        'br': bias2(np.asarray(inputs['br'], dtype=np.float32)),
        'bh': bias2(np.asarray(inputs['bh'], dtype=np.float32)),
        'ro1': conv_taps(inputs['ro_w1']),
        'rb1': bias2(np.asarray(inputs['ro_b1'], dtype=np.float32)),
        'ro2': np.ascontiguousarray(
            ro2_flat.T.reshape(2, 128, 9).transpose(1, 0, 2)),
        'rb2': np.asarray(inputs['ro_b2'], dtype=np.float32).reshape(1, 1),
        'ident_in': np.eye(128, dtype=np.float32),
    }

    fp = np.zeros((NF, 3, 384 + 32, 384), np.float32)
    fp[:, :, 16:400] = frames[0]

    in_maps = []
    for k in range(NCORES):
        m = dict(common)
        # patches [192=(ch,dy,dx), f, 10 rows x 48]
        pc = fp[:, :, 48 * k:48 * k + 80, :].reshape(NF, 3, 10, 8, 48, 8)
        m['patches'] = np.ascontiguousarray(
            pc.transpose(1, 3, 5, 0, 2, 4).reshape(192, NF, 480))
        st = np.zeros((128, 8), np.float32)
        sb = np.zeros((128, 8), np.float32)
        if k > 0:
            st[:, k - 1] = 1.0
        if k < NCORES - 1:
            sb[:, k + 1] = 1.0
        m['sel_top'] = st
        m['sel_bot'] = sb
        sct = np.full((128, 1), 0.0 if k == 0 else 1.0, np.float32)
        scb = np.full((128, 1), 0.0 if k == NCORES - 1 else 1.0, np.float32)
        m['sc_top'] = sct
        m['sc_bot'] = scb
        m['bbb_top'] = common['bbb'] * sct[0, 0]
        m['bbb_bot'] = common['bbb'] * scb[0, 0]
        m['rb1_top'] = common['rb1'] * sct[0, 0]
        m['rb1_bot'] = common['rb1'] * scb[0, 0]
        in_maps.append(m)
    return in_maps


def run_cores(inputs, trace=False):
    """Returns (per_core_results, BassKernelResults)."""
    sys.path.insert(0, '/opt/trn_rl_repo')
    from concourse.bass_utils import run_bass_kernel_spmd
    if 'nc' not in _CACHE:
        _CACHE['nc'] = _build_graph()
    nc = _CACHE['nc']
    in_maps = _prep_inputs(inputs)
    res = run_bass_kernel_spmd(nc, in_maps, core_ids=list(range(NCORES)), trace=trace)
    return res


def kernel(**inputs):
    res = run_cores(inputs, trace=False)
    out = np.zeros((1, NF, 1, HF, WF), np.float32)
    for k in range(NCORES):
        out[0, :, 0, RW * k:RW * (k + 1), :] = res.results[k]['out']
    return out


if __name__ == '__main__':
    data = np.load('/tmp/ref_inputs.npz')
    inputs = {k: data[k] for k in data.files}
    out = kernel(**inputs)
    ref = np.load('/tmp/ref_out.npy')
    rel = np.linalg.norm(out - ref) / np.linalg.norm(ref)
    print('rel l2 err:', rel)


# revision 23
# speedup vs baseline: 44.0039x; 1.1401x over previous
"""Trainium2 Bass kernel for nn_AGNN (3-frame attentional GNN + ConvGRU).

Self-contained: builds an 8-core SPMD Bass graph (sequence-parallel over the
48x48 spatial tokens, 6 rows per core), runs it via run_bass_kernel_spmd,
and reassembles the full output.

Sharding: each core owns 6 rows (288 tokens) of every frame. Per iteration:
  AllGather h (bf16, ch-major + tok-major layouts) -> each core computes
  attention for its 288 query tokens against all 2304 keys of each frame
  (9 ordered frame pairs), gated aggregation, then a 4-row boundary
  AllGather (magg + h) feeds the halo rows of the 3x3 ConvGRU which each
  core evaluates for its own rows.  Readout convs are local (v computed
  with halo from the raw frames; h halo from a final boundary exchange).

Precision: attention matmuls in bf16 (fp32 PSUM accumulation), everything
else float32r (tf32 matmul mode).  Validated ~5.5e-4 rel error vs the
fp32 reference in simulation.
"""
import sys
import numpy as np

NF = 3          # frames
C = 256         # channels
HF = WF = 48    # feature map
P = HF * WF     # 2304 tokens/frame
NCORES = 8
RW = 6          # rows per core
PL = RW * WF    # 288 tokens per core
K_ITERS = 3

_CACHE = {}


def _build_graph():
    sys.path.insert(0, '/opt/trn_rl_repo')
    import concourse.bass as bass
    import concourse.mybir as mybir
    import concourse.tile as tile
    from concourse import bacc

    dt = mybir.dt
    f32 = dt.float32
    f32r = dt.float32r
    bf16 = dt.bfloat16
    AF = mybir.ActivationFunctionType
    OP = mybir.AluOpType
    RG = [list(range(NCORES))]

    nc = bacc.Bacc()

    # ---------------- external IO ----------------
    def ein(name, shape, dtype=None):
        return nc.dram_tensor(name, list(shape), dtype or f32, kind="ExternalInput")

    patches = ein("patches", (192, NF, 480), f32r)       # host patch-extract, rows 6k-2..6k+7
    bbw0 = ein("bbw0", (128, C), f32r)
    bbw1 = ein("bbw1", (64, C), f32r)
    bbb = ein("bbb", (128, 2))
    bbb_top = ein("bbb_top", (128, 2))
    bbb_bot = ein("bbb_bot", (128, 2))
    w_int = ein("w_int", (128, 2, C), f32r)              # W_inter [c, d] swizzled
    w_inta = ein("w_inta", (128, 2, C), f32r)
    gw_t = ein("gw_t", (128, 2, C), f32r)                # gate_w^T [i, o] swizzled
    gb_neg = ein("gb_neg", (128, 2))                       # -gate_b
    wz = ein("wz", (9, 128, 2, 4, 128), f32r)            # [tap, kp, mt, kt, mp]
    wr = ein("wr", (9, 128, 2, 4, 128), f32r)
    wh = ein("wh", (9, 128, 2, 4, 128), f32r)
    bz = ein("bz", (128, 2))
    br = ein("br", (128, 2))
    bh = ein("bh", (128, 2))
    ro1 = ein("ro1", (9, 128, 2, 4, 128), f32r)
    rb1 = ein("rb1", (128, 2))
    rb1_top = ein("rb1_top", (128, 2))
    rb1_bot = ein("rb1_bot", (128, 2))
    ro2 = ein("ro2", (128, 2, 9), f32r)
    rb2 = ein("rb2", (1, 1))
    sel_top = ein("sel_top", (128, 8))                   # one-hot rank k-1 (zeros at core 0)
    sel_bot = ein("sel_bot", (128, 8))                   # one-hot rank k+1 (zeros at core 7)
    sc_top = ein("sc_top", (128, 1))                     # 0.0 on core 0 else 1.0
    sc_bot = ein("sc_bot", (128, 1))                     # 0.0 on core 7 else 1.0
    ident_in = ein("ident_in", (128, 128), f32r)

    out_ext = nc.dram_tensor("out", [NF, RW, WF], f32, kind="ExternalOutput")

    SH = NF * C * PL            # 221184 elements per layout shard

    with tile.TileContext(nc) as tc:
        with (
            tc.tile_pool(name="pers", bufs=1) as pers,
            tc.tile_pool(name="dram", bufs=1, space="DRAM") as dram,
            tc.tile_pool(name="psum", bufs=1, space="PSUM") as psum,
            tc.tile_pool(name="work", bufs=1) as work,
        ):
            # ---------------- persistent SBUF ----------------
            ident = pers.tile([128, 128], f32r)
            nc.sync.dma_start(ident[...], ident_in.ap())

            def load_pers(name, ext, shape, view=None):
                t = pers.tile(list(shape), f32r, name=name)
                src = ext.ap() if view is None else view
                nc.sync.dma_start(t[...], src)
                return t

            # W_inter/W_intra/gate_w^T as [128, 2ct, 256]
            wint_t = load_pers("wint_t", w_int, (128, 2, C))
            winta_t = load_pers("winta_t", w_inta, (128, 2, C))
            gw_tt = load_pers("gw_tt", gw_t, (128, 2, C))
            bbw0_t = load_pers("bbw0_t", bbw0, (128, C))
            bbw1_t = load_pers("bbw1_t", bbw1, (64, C))
            ro2_t = load_pers("ro2_t", ro2, (128, 2, 9))

            def load_bias(name, ext):
                t = pers.tile([128, 2], f32, name=name)
                nc.sync.dma_start(t[...], ext.ap())
                return t

            bbb_t = load_bias("bbb_t", bbb)
            bbbtop_t = load_bias("bbbtop_t", bbb_top)
            bbbbot_t = load_bias("bbbbot_t", bbb_bot)
            gnb_t = load_bias("gnb_t", gb_neg)
            bz_t = load_bias("bz_t", bz)
            br_t = load_bias("br_t", br)
            bh_t = load_bias("bh_t", bh)
            rb1_t = load_bias("rb1_t", rb1)
            rb1top_t = load_bias("rb1top_t", rb1_top)
            rb1bot_t = load_bias("rb1bot_t", rb1_bot)
            rb2_t = pers.tile([1, 1], f32)
            nc.sync.dma_start(rb2_t[...], rb2.ap())
            seltop_t = pers.tile([128, 8], f32)
            nc.sync.dma_start(seltop_t[...], sel_top.ap())
            selbot_t = pers.tile([128, 8], f32)
            nc.sync.dma_start(selbot_t[...], sel_bot.ap())
            sctop_t = pers.tile([128, 1], f32)
            nc.sync.dma_start(sctop_t[...], sc_top.ap())
            scbot_t = pers.tile([128, 1], f32)
            nc.sync.dma_start(scbot_t[...], sc_bot.ap())

            # big persistent state
            v_sb = pers.tile([128, 2, NF, 10 * WF], f32)      # v rows 6k-2..6k+7
            h_loc = pers.tile([128, 2, NF, PL], f32r)          # own rows, ch-major
            magg = pers.tile([128, 2, NF, PL], f32r)
            t_sb = pers.tile([128, 2, 6, PL], bf16)           # t^T (3 inter + 3 intra)

            def r32(ap):
                return ap.bitcast(f32r)

            # ---------------- backbone ----------------
            with tc.tile_pool(name="bb", bufs=1) as bb:
                pk0 = bb.tile([128, NF, 480], f32r, name="pk0")
                pk1 = bb.tile([64, NF, 480], f32r, name="pk1")
                nc.sync.dma_start(pk0[...], patches[0:128])
                nc.sync.dma_start(pk1[...], patches[128:192])
                for f in range(NF):
                    vps = psum.tile([128, 2, 512], f32, tag="pe", bufs=2, name="vps")
                    for mt in range(2):
                        nc.tensor.matmul(vps[:, mt, 0:480],
                                         bbw0_t[:, mt * 128:(mt + 1) * 128],
                                         pk0[:, f, :], start=True, stop=False)
                        nc.tensor.matmul(vps[:, mt, 0:480],
                                         bbw1_t[0:64, mt * 128:(mt + 1) * 128],
                                         pk1[0:64, f, :], start=False, stop=True)
                    for mt in range(2):
                        # rows 0-1 / 2-7 / 8-9 with edge masking (v=0 outside image)
                        nc.scalar.activation(v_sb[:, mt, f, 0:96], vps[:, mt, 0:96],
                                             AF.Relu, bias=bbbtop_t[:, mt:mt + 1], scale=sctop_t[:, :])
                        nc.scalar.activation(v_sb[:, mt, f, 96:384], vps[:, mt, 96:384],
                                             AF.Relu, bias=bbb_t[:, mt:mt + 1])
                        nc.scalar.activation(v_sb[:, mt, f, 384:480], vps[:, mt, 384:480],
                                             AF.Relu, bias=bbbbot_t[:, mt:mt + 1], scale=scbot_t[:, :])
                        # h0 = v own rows (rows 2..8 of the 10-row window)
                        nc.vector.tensor_copy(h_loc[:, mt, f, :], v_sb[:, mt, f, 96:384])

            # ---------------- iterations ----------------
            for it in range(K_ITERS):
                # ---- write AG input: ch-major + tok-major (bf16) ----
                agi = dram.tile([2, NF, SH // NF], bf16, tag="agi", bufs=2, name="agi")
                hloc16 = work.tile([128, 2, NF, PL], bf16, tag="hloc16", bufs=1, name="hloc16")
                nc.vector.tensor_copy(hloc16[...], h_loc[...])
                for f in range(NF):
                    for ct in range(2):
                        nc.sync.dma_start(
                            agi[0, f].rearrange("(c t) -> c t", t=PL)[ct * 128:(ct + 1) * 128, :],
                            hloc16[:, ct, f, :])
                    # tok-major via TensorE transposes: [c,96tok] -> [96tok, 128c]
                    hlt = work.tile([96, 3, C], bf16, tag="hlt", bufs=2, name="hlt")
                    for ct in range(2):
                        for ps in range(3):
                            tp = psum.tile([96, 128], f32r, tag="aux", bufs=1, name="tp")
                            nc.tensor.transpose(
                                tp[0:96, 0:128],
                                h_loc[:, ct, f, ps * 96:(ps + 1) * 96],
                                ident[:, :])
                            nc.vector.tensor_copy(hlt[0:96, ps, ct * 128:(ct + 1) * 128],
                                                  tp[0:96, 0:128])
                    nc.sync.dma_start(
                        agi[1, f].rearrange("(t c) -> t c", c=C)
                        .rearrange("(ps p) c -> p ps c", p=96),
                        hlt[0:96, :, :])

                ago = dram.tile([NCORES, 2, NF, SH // NF], bf16, tag="ago", bufs=2,
                                addr_space="Shared", name="ago")
                nc.gpsimd.collective_compute(
                    "AllGather", OP.bypass, replica_groups=RG,
                    ins=[agi.opt()], outs=[ago.opt()])

                # ---- t = h_loc @ W (both kinds), bf16 out ----
                for i in range(NF):
                    for kind in range(2):       # 0 inter, 1 intra
                        wsel = wint_t if kind == 0 else winta_t
                        for dct in range(2):
                            tps = psum.tile([128, 2, 512], f32, tag="pe", bufs=2, name="tps")
                            for ct in range(2):
                                nc.tensor.matmul(
                                    tps[:, 0, 0:288],
                                    r32(wsel[:, ct, dct * 128:(dct + 1) * 128]),
                                    r32(h_loc[:, ct, i, :]),
                                    start=(ct == 0), stop=(ct == 1))
                            nc.vector.tensor_copy(t_sb[:, dct, kind * 3 + i, :],
                                                  tps[:, 0, 0:288])

                # ---- boundary AG input: magg written later; h part now ----
                bdi = dram.tile([2, NF, C, 4 * WF], bf16, tag="bdi", bufs=2, name="bdi")
                for f in range(NF):
                    for ct in range(2):
                        nc.sync.dma_start(
                            bdi[1, f, ct * 128:(ct + 1) * 128, 0:96], hloc16[:, ct, f, 0:96])
                        nc.sync.dma_start(
                            bdi[1, f, ct * 128:(ct + 1) * 128, 96:192], hloc16[:, ct, f, 192:288])

                # ---- attention over j (keys) and i (queries) ----
                # stage all frames contiguously in DRAM up front (rank chunks
                # are 288 tokens; 128-token tiles cross rank boundaries
                # otherwise)
                stgs, stgcs = [], []
                for j in range(NF):
                    stg = dram.tile([P, C], bf16, tag="stg", bufs=3, name="stg")
                    nc.sync.dma_start(stg[:, :], ago[:, 1, j])
                    stgs.append(stg)
                    stgc = dram.tile([C, P], bf16, tag="stgc", bufs=3, name="stgc")
                    nc.sync.dma_start(
                        stgc[:, :].rearrange("c (r t) -> r c t", r=NCORES),
                        ago[:, 0, j])
                    stgcs.append(stgc)
                for j in range(NF):
                    stg, stgc = stgs[j], stgcs[j]
                    hch = []
                    for hh in range(2):
                        t_ = work.tile([128, 2, 9, 128], bf16, tag="hch", bufs=3, name="hch")
                        for ct in range(2):
                            nc.sync.dma_start(
                                t_[:, ct, :, :],
                                stgc[ct * 128:(ct + 1) * 128, :]
                                .rearrange("p (q x) -> p q x", x=128)
                                [:, hh * 9:(hh + 1) * 9, :])
                        hch.append(t_)
                    htok = []
                    for hh in range(2):
                        t_ = work.tile([128, 9, 257], bf16, tag="htok", bufs=3, name="htok")
                        nc.gpsimd.memset(t_[...], 1.0)
                        nc.sync.dma_start(
                            t_[:, :, 0:256],
                            stg[:, :].rearrange("(q p) c -> p q c", p=128)
                            [:, hh * 9:(hh + 1) * 9, :])
                        htok.append(t_)

                    for i in range(NF):
                        tix = (3 + i) if i == j else i
                        attn = work.tile([128, 18, 288], bf16, tag="attn", bufs=2, name="attn")
                        mu0 = psum.tile([128, 2, 512], f32, tag="mu0", bufs=1, name="mu0")
                        mu2 = psum.tile([32, 257], f32, tag="mu2", bufs=1, name="mu2")
                        # software-pipelined: e/exp group g, then m-matmuls of g-1
                        for g in range(10):
                            if g < 9:
                                e2 = psum.tile([128, 2, 512], f32, tag="pe", bufs=2, name="e2")
                                for u in range(2):
                                    q = g * 2 + u
                                    for ct in range(2):
                                        nc.tensor.matmul(
                                            e2[:, u, 0:288],
                                            hch[q // 9][:, ct, q % 9, :],
                                            t_sb[:, ct, tix, :],
                                            start=(ct == 0), stop=(ct == 1))
                                nc.scalar.activation(attn[:, g * 2:g * 2 + 2, :],
                                                     e2[:, :, 0:288], AF.Exp)
                            if g >= 1:
                                for u in range(2):
                                    q = (g - 1) * 2 + u
                                    st = (q == 0)
                                    sp = (q == 17)
                                    mv = htok[q // 9][:, q % 9, :]
                                    nc.tensor.matmul(mu0[:, 0, 0:257], attn[:, q, 0:128],
                                                     mv, start=st, stop=sp)
                                    nc.tensor.matmul(mu0[:, 1, 0:257], attn[:, q, 128:256],
                                                     mv, start=st, stop=sp)
                                    nc.tensor.matmul(mu2[0:32, 0:257], attn[:, q, 256:288],
                                                     mv, start=st, stop=sp)
                        # normalize m (softmax denominator = col 256)
                        mnorm = work.tile([128, 3, 256], f32r, tag="mnorm", bufs=2, name="mnorm")
                        rs = work.tile([128, 3, 1], f32, tag="rs", bufs=2, name="rs")
                        for s in range(3):
                            mus = mu0[:, s, :] if s < 2 else mu2[0:32, :]
                            pp = 128 if s < 2 else 32
                            nc.vector.reciprocal(rs[0:pp, s, :], mus[0:pp, 256:257])
                            nc.vector.tensor_scalar(mnorm[0:pp, s, :], mus[0:pp, 0:256],
                                                    rs[0:pp, s, :], None, OP.mult)
                        # transpose m -> ch-major; alternate psum banks (aux
                        # and the just-freed mu2 slot) so TensorE doesn't wait
                        # on the DVE copy between the two c-tile groups
                        mT = work.tile([128, 2, 288], f32r, tag="mT", bufs=2, name="mT")
                        for ct in range(2):
                            tps = psum.tile([128, 288], f32r,
                                            tag="aux" if ct == 0 else "mu2",
                                            bufs=1, name="mtp")
                            for s in range(3):
                                pp = 128 if s < 2 else 32
                                nc.tensor.transpose(
                                    tps[:, s * 128:s * 128 + pp],
                                    mnorm[0:pp, s, ct * 128:(ct + 1) * 128],
                                    ident[0:pp, 0:pp])
                            nc.vector.tensor_copy(mT[:, ct, :], tps[:, 0:288])
                        # gate: g = sigmoid(gate_w m + b) via exp (stay on exp table)
                        gps = psum.tile([128, 2, 512], f32, tag="mu0", bufs=1, name="gps")
                        for oct in range(2):
                            for ict in range(2):
                                nc.tensor.matmul(
                                    gps[:, oct, 0:288],
                                    r32(gw_tt[:, ict, oct * 128:(oct + 1) * 128]),
                                    r32(mT[:, ict, :]),
                                    start=(ict == 0), stop=(ict == 1))
                        gtmp = work.tile([128, 2, 288], f32, tag="gtmp", bufs=2, name="gtmp")
                        for oct in range(2):
                            nc.scalar.activation(gtmp[:, oct, :], gps[:, oct, 0:288],
                                                 AF.Exp, bias=gnb_t[:, oct:oct + 1], scale=-1.0)
                        nc.vector.tensor_scalar(gtmp[...], gtmp[...], 1.0, None, OP.add)
                        nc.vector.reciprocal(gtmp[...], gtmp[...])
                        nc.vector.tensor_tensor(gtmp[...], gtmp[...], mT[...], OP.mult)
                        if j == 0:
                            nc.vector.tensor_copy(magg[:, :, i, :], gtmp[...])
                        else:
                            nc.vector.tensor_tensor(magg[:, :, i, :], magg[:, :, i, :],
                                                    gtmp[...], OP.add)

                # ---- boundary AG (magg + h 2-row halos, bf16) ----
                magg16 = work.tile([128, 2, NF, PL], bf16, tag="magg16", bufs=1,
                                   name="magg16")
                nc.vector.tensor_copy(magg16[...], magg[...])
                for f in range(NF):
                    for ct in range(2):
                        nc.sync.dma_start(
                            bdi[0, f, ct * 128:(ct + 1) * 128, 0:96], magg16[:, ct, f, 0:96])
                        nc.sync.dma_start(
                            bdi[0, f, ct * 128:(ct + 1) * 128, 96:192], magg16[:, ct, f, 192:288])
                bdo = dram.tile([NCORES, 2, NF, C, 4 * WF], bf16, tag="bdo", bufs=2,
                                addr_space="Shared", name="bdo")
                nc.gpsimd.collective_compute(
                    "AllGather", OP.bypass, replica_groups=RG,
                    ins=[bdi.opt()], outs=[bdo.opt()])

                # ---- halo extraction via one-hot rank masks ----
                # halo[kind][ct]: top rows (from rank k-1 bottom seg) & bottom rows
                halo = work.tile([128, 2, 2, NF, 2, 96], f32, tag="halo", bufs=1, name="halo")
                for kind in range(2):
                    for ct in range(2):
                        for rp in range(4):
                            ch = work.tile([128, 2, NF, 192], bf16, tag="hchk", bufs=1, name="hchk")
                            for _rr in range(2):
                                nc.sync.dma_start(
                                    ch[:, _rr, :, :],
                                    bdo[rp * 2 + _rr, kind, :, ct * 128:(ct + 1) * 128, :]
                                    .rearrange("f c x -> c f x"))
                            for rr in range(2):
                                r = rp * 2 + rr
                                for tb in range(2):
                                    sel = seltop_t if tb == 0 else selbot_t
                                    seg = ch[:, rr, :, 96:192] if tb == 0 else ch[:, rr, :, 0:96]
                                    dst = halo[:, kind, ct, :, tb, :]
                                    if r == 0:
                                        nc.vector.tensor_scalar(dst, seg, sel[:, 0:1],
                                                                None, OP.mult)
                                    else:
                                        nc.vector.scalar_tensor_tensor(
                                            dst, seg, sel[:, r:r + 1], dst,
                                            OP.mult, OP.add)

                # ---- ConvGRU ----
                mh = []
                for f in range(NF):
                    m_ = work.tile([128, 4, 512], f32r, tag="mh", bufs=3, name="mh")
                    nc.gpsimd.memset(m_[...].bitcast(f32), 0.0)
                    for ct in range(2):
                        rows = m_[:, ct, 6:506].rearrange("p (r w) -> p r w", w=50)
                        nc.vector.tensor_copy(
                            rows[:, 2:8, 1:49],
                            magg[:, ct, f, :].rearrange("p (r w) -> p r w", w=48))
                        nc.vector.tensor_copy(
                            rows[:, 0:2, 1:49],
                            halo[:, 0, ct, f, 0, :].rearrange("p (r w) -> p r w", w=48))
                        nc.vector.tensor_copy(
                            rows[:, 8:10, 1:49],
                            halo[:, 0, ct, f, 1, :].rearrange("p (r w) -> p r w", w=48))
                        hrows = m_[:, 2 + ct, 6:506].rearrange("p (r w) -> p r w", w=50)
                        nc.vector.tensor_copy(
                            hrows[:, 2:8, 1:49],
                            h_loc[:, ct, f, :].rearrange("p (r w) -> p r w", w=48))
                        nc.vector.tensor_copy(
                            hrows[:, 0:2, 1:49],
                            halo[:, 1, ct, f, 0, :].rearrange("p (r w) -> p r w", w=48))
                        nc.vector.tensor_copy(
                            hrows[:, 8:10, 1:49],
                            halo[:, 1, ct, f, 1, :].rearrange("p (r w) -> p r w", w=48))
                    mh.append(m_)

                def conv(wext, psums, NOUT, row0, src_of):
                    """9-tap conv: psums[f][:, mt, 0:NOUT] += taps."""
                    for tap in range(9):
                        dy, dx = tap // 3 - 1, tap % 3 - 1
                        wt = work.tile([128, 2, 4, 128], f32r, tag="wtap", bufs=2, name="wtap")
                        nc.sync.dma_start(wt[...], wext[tap])
                        for f in range(NF):
                            for kt in range(4):
                                mvs = src_of(f, kt, dy, dx)
                                if mvs is None:
                                    continue
                                for mt in range(2):
                                    nc.tensor.matmul(
                                        psums[f][:, mt, 0:NOUT],
                                        wt[:, mt, kt, :], r32(mvs),
                                        start=(tap == 0 and kt == 0),
                                        stop=(tap == 8 and kt == 3))

                def conv_psums():
                    ps = []
                    for f in range(NF):
                        tag = "pe" if f < 2 else "mu0"
                        ps.append(psum.tile([128, 2, 512], f32, tag=tag, bufs=2 if f < 2 else 1,
                                            name=f"cps{f}"))
                    return ps

                # z conv: out rows W2..W7 (own), N=300
                zps = conv_psums()
                conv(wz, zps, 300, 2,
                     lambda f, kt, dy, dx: mh[f][:, kt, 6 + (2 + dy) * 50 + dx:
                                                 6 + (2 + dy) * 50 + dx + 300])
                zgs = []
                for f in range(NF):
                    z_ = work.tile([128, 2, 300], f32, tag=f"zgs{f}", bufs=1, name="zgs")
                    for mt in range(2):
                        nc.scalar.activation(z_[:, mt, :], zps[f][:, mt, 0:300],
                                             AF.Sigmoid, bias=bz_t[:, mt:mt + 1])
                    zgs.append(z_)
                # r conv: out rows W1..W8, N=400
                rps = conv_psums()
                conv(wr, rps, 400, 1,
                     lambda f, kt, dy, dx: mh[f][:, kt, 6 + (1 + dy) * 50 + dx:
                                                 6 + (1 + dy) * 50 + dx + 400])
                rgh = []
                for f in range(NF):
                    # rg sigmoid written at 6-offset, then rg*h in place
                    rh_ = work.tile([128, 2, 416], f32r, tag=f"rgh{f}", bufs=1, name="rgh")
                    nc.gpsimd.memset(rh_[...].bitcast(f32), 0.0)
                    for mt in range(2):
                        nc.scalar.activation(rh_[:, mt, 6:406], rps[f][:, mt, 0:400],
                                             AF.Sigmoid, bias=br_t[:, mt:mt + 1])
                    nc.vector.tensor_tensor(
                        rh_[:, :, 6:406], rh_[:, :, 6:406],
                        mh[f][:, 2:4, 56:456], OP.mult)
                    rgh.append(rh_)
                # candidate conv: out rows W2..W7, N=300; inputs kt0-1 magg, kt2-3 rg*h
                hps = conv_psums()

                def hc_src(f, kt, dy, dx):
                    if kt < 2:
                        o = 6 + (2 + dy) * 50 + dx
                        return mh[f][:, kt, o:o + 300]
                    o = 6 + (1 + dy) * 50 + dx
                    return rgh[f][:, kt - 2, o:o + 300]

                conv(wh, hps, 300, 2, hc_src)
                for f in range(NF):
                    hc_ = work.tile([128, 2, 300], f32, tag="hcs", bufs=2, name="hcs")
                    for mt in range(2):
                        nc.scalar.activation(hc_[:, mt, :], hps[f][:, mt, 0:300],
                                             AF.Tanh, bias=bh_t[:, mt:mt + 1])
                    # h_new = h + z*(hc - h)
                    hold = mh[f][:, 2:4, 106:406]
                    nc.vector.tensor_tensor(hc_[...], hc_[...], hold, OP.subtract)
                    nc.vector.tensor_tensor(hc_[...], hc_[...], zgs[f][...], OP.mult)
                    for ct in range(2):
                        nc.vector.tensor_tensor(
                            h_loc[:, ct, f, :].rearrange("p (r w) -> p r w", w=48),
                            mh[f][:, 2 + ct, 106:406].rearrange("p (r w) -> p r w", w=50)[:, :, 1:49],
                            hc_[:, ct, :].rearrange("p (r w) -> p r w", w=50)[:, :, 1:49],
                            OP.add)

            # ---------------- readout ----------------
            hloc16f = work.tile([128, 2, NF, PL], bf16, tag="hloc16", bufs=1,
                                name="hloc16f")
            nc.vector.tensor_copy(hloc16f[...], h_loc[...])
            bdi2 = dram.tile([NF, C, 4 * WF], bf16, tag="bdi", bufs=2, name="bdi2")
            for f in range(NF):
                for ct in range(2):
                    nc.sync.dma_start(bdi2[f, ct * 128:(ct + 1) * 128, 0:96],
                                      hloc16f[:, ct, f, 0:96])
                    nc.sync.dma_start(bdi2[f, ct * 128:(ct + 1) * 128, 96:192],
                                      hloc16f[:, ct, f, 192:288])
            bdo2 = dram.tile([NCORES, NF, C, 4 * WF], bf16, tag="bdo", bufs=2,
                             addr_space="Shared", name="bdo2")
            nc.gpsimd.collective_compute(
                "AllGather", OP.bypass, replica_groups=RG,
                ins=[bdi2.opt()], outs=[bdo2.opt()])
            halo2 = work.tile([128, 2, NF, 2, 96], f32, tag="halo", bufs=1, name="halo2")
            for ct in range(2):
                for rp in range(4):
                    ch = work.tile([128, 2, NF, 192], bf16, tag="hchk", bufs=1, name="hchk2")
                    for _rr in range(2):
                        nc.sync.dma_start(
                            ch[:, _rr, :, :],
                            bdo2[rp * 2 + _rr, :, ct * 128:(ct + 1) * 128, :]
                            .rearrange("f c x -> c f x"))
                    for rr in range(2):
                        r = rp * 2 + rr
                        for tb in range(2):
                            sel = seltop_t if tb == 0 else selbot_t
                            seg = ch[:, rr, :, 96:192] if tb == 0 else ch[:, rr, :, 0:96]
                            dst = halo2[:, ct, :, tb, :]
                            if r == 0:
                                nc.vector.tensor_scalar(dst, seg, sel[:, 0:1], None, OP.mult)
                            else:
                                nc.vector.scalar_tensor_tensor(
                                    dst, seg, sel[:, r:r + 1], dst, OP.mult, OP.add)

            mask_sb = pers.tile([1, NF, RW * WF], f32)
            for f in range(NF):
                ro_in = work.tile([128, 4, 512], f32r, tag="mh", bufs=3, name="ro_in")
                nc.gpsimd.memset(ro_in[...].bitcast(f32), 0.0)
                for ct in range(2):
                    hrows = ro_in[:, ct, 6:506].rearrange("p (r w) -> p r w", w=50)
                    nc.vector.tensor_copy(
                        hrows[:, 2:8, 1:49],
                        h_loc[:, ct, f, :].rearrange("p (r w) -> p r w", w=48))
                    nc.vector.tensor_copy(
                        hrows[:, 0:2, 1:49],
                        halo2[:, ct, f, 0, :].rearrange("p (r w) -> p r w", w=48))
                    nc.vector.tensor_copy(
                        hrows[:, 8:10, 1:49],
                        halo2[:, ct, f, 1, :].rearrange("p (r w) -> p r w", w=48))
                    vrows = ro_in[:, 2 + ct, 6:506].rearrange("p (r w) -> p r w", w=50)
                    nc.vector.tensor_copy(
                        vrows[:, 0:10, 1:49],
                        v_sb[:, ct, f, :].rearrange("p (r w) -> p r w", w=48))
                # y = relu(ro1 * cat) rows W1..W8 (N=400)
                yps = psum.tile([128, 2, 512], f32, tag="pe", bufs=2, name="yps")
                for tap in range(9):
                    dy, dx = tap // 3 - 1, tap % 3 - 1
                    wt = work.tile([128, 2, 4, 128], f32r, tag="wtap", bufs=2, name="wtap2")
                    nc.sync.dma_start(wt[...], ro1[tap])
                    o = 6 + (1 + dy) * 50 + dx
                    for kt in range(4):
                        for mt in range(2):
                            nc.tensor.matmul(
                                yps[:, mt, 0:400], wt[:, mt, kt, :],
                                r32(ro_in[:, kt, o:o + 400]),
                                start=(tap == 0 and kt == 0), stop=(tap == 8 and kt == 3))
                y_sb = work.tile([128, 2, 412], f32r, tag="y_sb", bufs=1, name="y_sb")
                nc.gpsimd.memset(y_sb[...].bitcast(f32), 0.0)
                for mt in range(2):
                    ypr = yps[:, mt, 0:400].rearrange("p (r w) -> p r w", w=50)
                    ydst = y_sb[:, mt, 6:406].rearrange("p (r w) -> p r w", w=50)
                    # y rows: 0 (global 6k-1, invalid on core 0), 1..7, 7 (invalid on core 7)
                    nc.scalar.activation(ydst[:, 0:1, 1:49], ypr[:, 0:1, 1:49], AF.Relu,
                                         bias=rb1top_t[:, mt:mt + 1], scale=sctop_t[:, :])
                    nc.scalar.activation(ydst[:, 1:7, 1:49], ypr[:, 1:7, 1:49], AF.Relu,
                                         bias=rb1_t[:, mt:mt + 1])
                    nc.scalar.activation(ydst[:, 7:8, 1:49], ypr[:, 7:8, 1:49], AF.Relu,
                                         bias=rb1bot_t[:, mt:mt + 1], scale=scbot_t[:, :])
                # mask = ro2 * y + b2, own rows (N=300 in 50-col layout; y pad
                # cols are zero so the windows are safe)
                mps = psum.tile([1, 300], f32, tag="aux", bufs=1, name="mps")
                for tap in range(9):
                    dy, dx = tap // 3 - 1, tap % 3 - 1
                    o = 6 + (1 + dy) * 50 + dx
                    for ct in range(2):
                        nc.tensor.matmul(
                            mps[0:1, 0:300],
                            r32(ro2_t[:, ct, tap:tap + 1]),
                            r32(y_sb[:, ct, o:o + 300]),
                            start=(tap == 0 and ct == 0), stop=(tap == 8 and ct == 1))
                nc.scalar.activation(
                    mask_sb[0:1, f, :].rearrange("p (r w) -> p r w", w=48),
                    mps[0:1, :].rearrange("p (r w) -> p r w", w=50)[:, :, 1:49],
                    AF.Identity, bias=rb2_t[0:1, :])
            nc.sync.dma_start(out_ext.ap(), mask_sb[0:1, :, :])

    nc.finalize()
    return nc


def _prep_inputs(inputs):
    """Host-side weight prep + per-core shards."""
    frames = np.ascontiguousarray(inputs['frames'], dtype=np.float32)  # (1,3,3,384,384)
    bb_w = np.asarray(inputs['backbone_w'], dtype=np.float32)
    bb_b = np.asarray(inputs['backbone_b'], dtype=np.float32).reshape(C, 1)
    W_intra = np.asarray(inputs['W_intra'], dtype=np.float32)
    W_inter = np.asarray(inputs['W_inter'], dtype=np.float32)
    gate_w = np.asarray(inputs['gate_w'], dtype=np.float32)[:, :, 0, 0]
    gate_b = np.asarray(inputs['gate_b'], dtype=np.float32).reshape(C, 1)

    def taps(w):
        return np.ascontiguousarray(
            np.asarray(w, dtype=np.float32).transpose(2, 3, 1, 0).reshape(9, 2 * C, C))

    def swz(w):
        """[c, d] (256x256) -> [128, 2ct, d]"""
        return np.ascontiguousarray(w.reshape(2, 128, C).transpose(1, 0, 2))

    def bias2(b):
        """(256,1) -> (128, 2)"""
        return np.ascontiguousarray(b.reshape(2, 128).T)

    def conv_taps(w):
        """(O,I,3,3) -> [9, kp(128), mt, kt, mp(128)]"""
        t = taps(w)                                    # (9, 512, 256)
        t = t.reshape(9, 4, 128, 2, 128)               # tap, kt, kp, mt, mp
        return np.ascontiguousarray(t.transpose(0, 2, 3, 1, 4))

    bbw192 = bb_w.transpose(1, 2, 3, 0).reshape(192, C)
    ro2_flat = np.asarray(inputs['ro_w2'], dtype=np.float32
                          ).transpose(2, 3, 1, 0).reshape(9, C)      # tap, c
    common = {
        'bbw0': np.ascontiguousarray(bbw192[0:128]),
        'bbw1': np.ascontiguousarray(bbw192[128:192]),
        'bbb': bias2(bb_b),
        'w_int': swz(W_inter),
        'w_inta': swz(W_intra),
        'gw_t': swz(np.ascontiguousarray(gate_w.T)),
        'gb_neg': bias2(-gate_b),
        'wz': conv_taps(inputs['Wz']), 'wr': conv_taps(inputs['Wr']),
        'wh': conv_taps(inputs['Wh']),
        'bz': bias2(np.asarray(inputs['bz'], dtype=np.float32)),
        'br': bias2(np.asarray(inputs['br'], dtype=np.float32)),
        'bh': bias2(np.asarray(inputs['bh'], dtype=np.float32)),
        'ro1': conv_taps(inputs['ro_w1']),
        'rb1': bias2(np.asarray(inputs['ro_b1'], dtype=np.float32)),
        'ro2': np.ascontiguousarray(
            ro2_flat.T.reshape(2, 128, 9).transpose(1, 0, 2)),
        'rb2': np.asarray(inputs['ro_b2'], dtype=np.float32).reshape(1, 1),
        'ident_in': np.eye(128, dtype=np.float32),
    }

    fp = np.zeros((NF, 3, 384 + 32, 384), np.float32)
    fp[:, :, 16:400] = frames[0]

    in_maps = []
    for k in range(NCORES):
        m = dict(common)
        # patches [192=(ch,dy,dx), f, 10 rows x 48]
        pc = fp[:, :, 48 * k:48 * k + 80, :].reshape(NF, 3, 10, 8, 48, 8)
        m['patches'] = np.ascontiguousarray(
            pc.transpose(1, 3, 5, 0, 2, 4).reshape(192, NF, 480))
        st = np.zeros((128, 8), np.float32)
        sb = np.zeros((128, 8), np.float32)
        if k > 0:
            st[:, k - 1] = 1.0
        if k < NCORES - 1:
            sb[:, k + 1] = 1.0
        m['sel_top'] = st
        m['sel_bot'] = sb
        sct = np.full((128, 1), 0.0 if k == 0 else 1.0, np.float32)
        scb = np.full((128, 1), 0.0 if k == NCORES - 1 else 1.0, np.float32)
        m['sc_top'] = sct
        m['sc_bot'] = scb
        m['bbb_top'] = common['bbb'] * sct[0, 0]
        m['bbb_bot'] = common['bbb'] * scb[0, 0]
        m['rb1_top'] = common['rb1'] * sct[0, 0]
        m['rb1_bot'] = common['rb1'] * scb[0, 0]
        in_maps.append(m)
    return in_maps


def run_cores(inputs, trace=False):
    """Returns (per_core_results, BassKernelResults)."""
    sys.path.insert(0, '/opt/trn_rl_repo')
    from concourse.bass_utils import run_bass_kernel_spmd
    if 'nc' not in _CACHE:
        _CACHE['nc'] = _build_graph()
    nc = _CACHE['nc']
    in_maps = _prep_inputs(inputs)
    res = run_bass_kernel_spmd(nc, in_maps, core_ids=list(range(NCORES)), trace=trace)
    return res


def kernel(**inputs):
    res = run_cores(inputs, trace=False)
    out = np.zeros((1, NF, 1, HF, WF), np.float32)
    for k in range(NCORES):
        out[0, :, 0, RW * k:RW * (k + 1), :] = res.results[k]['out']
    return out


if __name__ == '__main__':
    data = np.load('/tmp/ref_inputs.npz')
    inputs = {k: data[k] for k in data.files}
    out = kernel(**inputs)
    ref = np.load('/tmp/ref_out.npy')
    rel = np.linalg.norm(out - ref) / np.linalg.norm(ref)
    print('rel l2 err:', rel)


# revision 24
# speedup vs baseline: 44.2050x; 1.0046x over previous
"""Trainium2 Bass kernel for nn_AGNN (3-frame attentional GNN + ConvGRU).

Self-contained: builds an 8-core SPMD Bass graph (sequence-parallel over the
48x48 spatial tokens, 6 rows per core), runs it via run_bass_kernel_spmd,
and reassembles the full output.

Sharding: each core owns 6 rows (288 tokens) of every frame. Per iteration:
  AllGather h (bf16, ch-major + tok-major layouts) -> each core computes
  attention for its 288 query tokens against all 2304 keys of each frame
  (9 ordered frame pairs), gated aggregation, then a 4-row boundary
  AllGather (magg + h) feeds the halo rows of the 3x3 ConvGRU which each
  core evaluates for its own rows.  Readout convs are local (v computed
  with halo from the raw frames; h halo from a final boundary exchange).

Precision: attention matmuls in bf16 (fp32 PSUM accumulation), everything
else float32r (tf32 matmul mode).  Validated ~5.5e-4 rel error vs the
fp32 reference in simulation.
"""
import sys
import numpy as np

NF = 3          # frames
C = 256         # channels
HF = WF = 48    # feature map
P = HF * WF     # 2304 tokens/frame
NCORES = 8
RW = 6          # rows per core
PL = RW * WF    # 288 tokens per core
K_ITERS = 3

_CACHE = {}


def _build_graph():
    sys.path.insert(0, '/opt/trn_rl_repo')
    import concourse.bass as bass
    import concourse.mybir as mybir
    import concourse.tile as tile
    from concourse import bacc

    dt = mybir.dt
    f32 = dt.float32
    f32r = dt.float32r
    bf16 = dt.bfloat16
    AF = mybir.ActivationFunctionType
    OP = mybir.AluOpType
    RG = [list(range(NCORES))]

    nc = bacc.Bacc()

    # ---------------- external IO ----------------
    def ein(name, shape, dtype=None):
        return nc.dram_tensor(name, list(shape), dtype or f32, kind="ExternalInput")

    patches = ein("patches", (192, NF, 480), f32r)       # host patch-extract, rows 6k-2..6k+7
    bbw0 = ein("bbw0", (128, C), f32r)
    bbw1 = ein("bbw1", (64, C), f32r)
    bbb = ein("bbb", (128, 2))
    bbb_top = ein("bbb_top", (128, 2))
    bbb_bot = ein("bbb_bot", (128, 2))
    w_int = ein("w_int", (128, 2, C), f32r)              # W_inter [c, d] swizzled
    w_inta = ein("w_inta", (128, 2, C), f32r)
    gw_t = ein("gw_t", (128, 2, C), f32r)                # gate_w^T [i, o] swizzled
    gb_neg = ein("gb_neg", (128, 2))                       # -gate_b
    wz = ein("wz", (9, 128, 2, 4, 128), f32r)            # [tap, kp, mt, kt, mp]
    wr = ein("wr", (9, 128, 2, 4, 128), f32r)
    wh = ein("wh", (9, 128, 2, 4, 128), f32r)
    bz = ein("bz", (128, 2))
    br = ein("br", (128, 2))
    bh = ein("bh", (128, 2))
    ro1 = ein("ro1", (9, 128, 2, 4, 128), f32r)
    rb1 = ein("rb1", (128, 2))
    rb1_top = ein("rb1_top", (128, 2))
    rb1_bot = ein("rb1_bot", (128, 2))
    ro2 = ein("ro2", (128, 2, 9), f32r)
    rb2 = ein("rb2", (1, 1))
    sel_top = ein("sel_top", (128, 8))                   # one-hot rank k-1 (zeros at core 0)
    sel_bot = ein("sel_bot", (128, 8))                   # one-hot rank k+1 (zeros at core 7)
    sc_top = ein("sc_top", (128, 1))                     # 0.0 on core 0 else 1.0
    sc_bot = ein("sc_bot", (128, 1))                     # 0.0 on core 7 else 1.0
    ident_in = ein("ident_in", (128, 128), f32r)

    out_ext = nc.dram_tensor("out", [NF, RW, WF], f32, kind="ExternalOutput")

    SH = NF * C * PL            # 221184 elements per layout shard

    with tile.TileContext(nc) as tc:
        with (
            tc.tile_pool(name="pers", bufs=1) as pers,
            tc.tile_pool(name="dram", bufs=1, space="DRAM") as dram,
            tc.tile_pool(name="psum", bufs=1, space="PSUM") as psum,
            tc.tile_pool(name="work", bufs=1) as work,
        ):
            # ---------------- persistent SBUF ----------------
            ident = pers.tile([128, 128], f32r)
            nc.sync.dma_start(ident[...], ident_in.ap())

            def load_pers(name, ext, shape, view=None):
                t = pers.tile(list(shape), f32r, name=name)
                src = ext.ap() if view is None else view
                nc.sync.dma_start(t[...], src)
                return t

            # W_inter/W_intra/gate_w^T as [128, 2ct, 256]
            wint_t = load_pers("wint_t", w_int, (128, 2, C))
            winta_t = load_pers("winta_t", w_inta, (128, 2, C))
            gw_tt = load_pers("gw_tt", gw_t, (128, 2, C))
            bbw0_t = load_pers("bbw0_t", bbw0, (128, C))
            bbw1_t = load_pers("bbw1_t", bbw1, (64, C))
            ro2_t = load_pers("ro2_t", ro2, (128, 2, 9))

            def load_bias(name, ext):
                t = pers.tile([128, 2], f32, name=name)
                nc.sync.dma_start(t[...], ext.ap())
                return t

            bbb_t = load_bias("bbb_t", bbb)
            bbbtop_t = load_bias("bbbtop_t", bbb_top)
            bbbbot_t = load_bias("bbbbot_t", bbb_bot)
            gnb_t = load_bias("gnb_t", gb_neg)
            bz_t = load_bias("bz_t", bz)
            br_t = load_bias("br_t", br)
            bh_t = load_bias("bh_t", bh)
            rb1_t = load_bias("rb1_t", rb1)
            rb1top_t = load_bias("rb1top_t", rb1_top)
            rb1bot_t = load_bias("rb1bot_t", rb1_bot)
            rb2_t = pers.tile([1, 1], f32)
            nc.sync.dma_start(rb2_t[...], rb2.ap())
            seltop_t = pers.tile([128, 8], f32)
            nc.sync.dma_start(seltop_t[...], sel_top.ap())
            selbot_t = pers.tile([128, 8], f32)
            nc.sync.dma_start(selbot_t[...], sel_bot.ap())
            sctop_t = pers.tile([128, 1], f32)
            nc.sync.dma_start(sctop_t[...], sc_top.ap())
            scbot_t = pers.tile([128, 1], f32)
            nc.sync.dma_start(scbot_t[...], sc_bot.ap())

            # big persistent state
            v_sb = pers.tile([128, 2, NF, 10 * WF], f32)      # v rows 6k-2..6k+7
            h_loc = pers.tile([128, 2, NF, PL], f32r)          # own rows, ch-major
            magg = pers.tile([128, 2, NF, PL], f32r)
            t_sb = pers.tile([128, 2, 6, PL], bf16)           # t^T (3 inter + 3 intra)

            def r32(ap):
                return ap.bitcast(f32r)

            # ---------------- backbone ----------------
            with tc.tile_pool(name="bb", bufs=1) as bb:
                pk0 = bb.tile([128, NF, 480], f32r, name="pk0")
                pk1 = bb.tile([64, NF, 480], f32r, name="pk1")
                nc.sync.dma_start(pk0[...], patches[0:128])
                nc.sync.dma_start(pk1[...], patches[128:192])
                for f in range(NF):
                    vps = psum.tile([128, 2, 512], f32, tag="pe", bufs=2, name="vps")
                    for mt in range(2):
                        nc.tensor.matmul(vps[:, mt, 0:480],
                                         bbw0_t[:, mt * 128:(mt + 1) * 128],
                                         pk0[:, f, :], start=True, stop=False)
                        nc.tensor.matmul(vps[:, mt, 0:480],
                                         bbw1_t[0:64, mt * 128:(mt + 1) * 128],
                                         pk1[0:64, f, :], start=False, stop=True)
                    for mt in range(2):
                        # rows 0-1 / 2-7 / 8-9 with edge masking (v=0 outside image)
                        nc.scalar.activation(v_sb[:, mt, f, 0:96], vps[:, mt, 0:96],
                                             AF.Relu, bias=bbbtop_t[:, mt:mt + 1], scale=sctop_t[:, :])
                        nc.scalar.activation(v_sb[:, mt, f, 96:384], vps[:, mt, 96:384],
                                             AF.Relu, bias=bbb_t[:, mt:mt + 1])
                        nc.scalar.activation(v_sb[:, mt, f, 384:480], vps[:, mt, 384:480],
                                             AF.Relu, bias=bbbbot_t[:, mt:mt + 1], scale=scbot_t[:, :])
                        # h0 = v own rows (rows 2..8 of the 10-row window)
                        nc.vector.tensor_copy(h_loc[:, mt, f, :], v_sb[:, mt, f, 96:384])

            # ---------------- iterations ----------------
            for it in range(K_ITERS):
                # ---- write AG input: ch-major + tok-major (bf16) ----
                agi_parts = [
                    dram.tile([2, 1, SH // NF], bf16, tag="agi0", bufs=2, name="agi0"),
                    dram.tile([2, 2, SH // NF], bf16, tag="agi12", bufs=2, name="agi12"),
                ]

                def agi_v(layout, f):
                    part = agi_parts[0] if f == 0 else agi_parts[1]
                    return part[layout, 0 if f == 0 else f - 1]
                hloc16 = work.tile([128, 2, NF, PL], bf16, tag="hloc16", bufs=1, name="hloc16")
                nc.vector.tensor_copy(hloc16[...], h_loc[...])
                for f in range(NF):
                    for ct in range(2):
                        nc.sync.dma_start(
                            agi_v(0, f).rearrange("(c t) -> c t", t=PL)[ct * 128:(ct + 1) * 128, :],
                            hloc16[:, ct, f, :])
                    # tok-major via TensorE transposes: [c,96tok] -> [96tok, 128c]
                    hlt = work.tile([96, 3, C], bf16, tag="hlt", bufs=2, name="hlt")
                    for ct in range(2):
                        for ps in range(3):
                            tp = psum.tile([96, 128], f32r, tag="aux", bufs=1, name="tp")
                            nc.tensor.transpose(
                                tp[0:96, 0:128],
                                h_loc[:, ct, f, ps * 96:(ps + 1) * 96],
                                ident[:, :])
                            nc.vector.tensor_copy(hlt[0:96, ps, ct * 128:(ct + 1) * 128],
                                                  tp[0:96, 0:128])
                    nc.sync.dma_start(
                        agi_v(1, f).rearrange("(t c) -> t c", c=C)
                        .rearrange("(ps p) c -> p ps c", p=96),
                        hlt[0:96, :, :])

                ago_parts = [
                    dram.tile([NCORES, 2, 1, SH // NF], bf16, tag="ago0", bufs=2,
                              addr_space="Shared", name="ago0"),
                    dram.tile([NCORES, 2, 2, SH // NF], bf16, tag="ago12", bufs=2,
                              addr_space="Shared", name="ago12"),
                ]
                for _p in range(2):
                    nc.gpsimd.collective_compute(
                        "AllGather", OP.bypass, replica_groups=RG,
                        ins=[agi_parts[_p].opt()], outs=[ago_parts[_p].opt()])

                def ago_v(layout, f):
                    part = ago_parts[0] if f == 0 else ago_parts[1]
                    return part[:, layout, 0 if f == 0 else f - 1]

                # ---- t = h_loc @ W (both kinds), bf16 out ----
                for i in range(NF):
                    for kind in range(2):       # 0 inter, 1 intra
                        wsel = wint_t if kind == 0 else winta_t
                        for dct in range(2):
                            tps = psum.tile([128, 2, 512], f32, tag="pe", bufs=2, name="tps")
                            for ct in range(2):
                                nc.tensor.matmul(
                                    tps[:, 0, 0:288],
                                    r32(wsel[:, ct, dct * 128:(dct + 1) * 128]),
                                    r32(h_loc[:, ct, i, :]),
                                    start=(ct == 0), stop=(ct == 1))
                            nc.vector.tensor_copy(t_sb[:, dct, kind * 3 + i, :],
                                                  tps[:, 0, 0:288])

                # ---- boundary AG input: magg written later; h part now ----
                bdi = dram.tile([2, NF, C, 4 * WF], bf16, tag="bdi", bufs=2, name="bdi")
                for f in range(NF):
                    for ct in range(2):
                        nc.sync.dma_start(
                            bdi[1, f, ct * 128:(ct + 1) * 128, 0:96], hloc16[:, ct, f, 0:96])
                        nc.sync.dma_start(
                            bdi[1, f, ct * 128:(ct + 1) * 128, 96:192], hloc16[:, ct, f, 192:288])

                # ---- attention over j (keys) and i (queries) ----
                # stage all frames contiguously in DRAM up front (rank chunks
                # are 288 tokens; 128-token tiles cross rank boundaries
                # otherwise)
                stgs, stgcs = [], []
                for j in range(NF):
                    stg = dram.tile([P, C], bf16, tag="stg", bufs=3, name="stg")
                    nc.sync.dma_start(stg[:, :], ago_v(1, j))
                    stgs.append(stg)
                    stgc = dram.tile([C, P], bf16, tag="stgc", bufs=3, name="stgc")
                    nc.sync.dma_start(
                        stgc[:, :].rearrange("c (r t) -> r c t", r=NCORES),
                        ago_v(0, j))
                    stgcs.append(stgc)
                for j in range(NF):
                    stg, stgc = stgs[j], stgcs[j]
                    hch = []
                    for hh in range(2):
                        t_ = work.tile([128, 2, 9, 128], bf16, tag="hch", bufs=3, name="hch")
                        for ct in range(2):
                            nc.sync.dma_start(
                                t_[:, ct, :, :],
                                stgc[ct * 128:(ct + 1) * 128, :]
                                .rearrange("p (q x) -> p q x", x=128)
                                [:, hh * 9:(hh + 1) * 9, :])
                        hch.append(t_)
                    htok = []
                    for hh in range(2):
                        t_ = work.tile([128, 9, 257], bf16, tag="htok", bufs=3, name="htok")
                        nc.gpsimd.memset(t_[...], 1.0)
                        nc.sync.dma_start(
                            t_[:, :, 0:256],
                            stg[:, :].rearrange("(q p) c -> p q c", p=128)
                            [:, hh * 9:(hh + 1) * 9, :])
                        htok.append(t_)

                    for i in range(NF):
                        tix = (3 + i) if i == j else i
                        attn = work.tile([128, 18, 288], bf16, tag="attn", bufs=2, name="attn")
                        mu0 = psum.tile([128, 2, 512], f32, tag="mu0", bufs=1, name="mu0")
                        mu2 = psum.tile([32, 257], f32, tag="mu2", bufs=1, name="mu2")
                        # software-pipelined: e/exp group g, then m-matmuls of g-1
                        for g in range(10):
                            if g < 9:
                                e2 = psum.tile([128, 2, 512], f32, tag="pe", bufs=2, name="e2")
                                for u in range(2):
                                    q = g * 2 + u
                                    for ct in range(2):
                                        nc.tensor.matmul(
                                            e2[:, u, 0:288],
                                            hch[q // 9][:, ct, q % 9, :],
                                            t_sb[:, ct, tix, :],
                                            start=(ct == 0), stop=(ct == 1))
                                nc.scalar.activation(attn[:, g * 2:g * 2 + 2, :],
                                                     e2[:, :, 0:288], AF.Exp)
                            if g >= 1:
                                for u in range(2):
                                    q = (g - 1) * 2 + u
                                    st = (q == 0)
                                    sp = (q == 17)
                                    mv = htok[q // 9][:, q % 9, :]
                                    nc.tensor.matmul(mu0[:, 0, 0:257], attn[:, q, 0:128],
                                                     mv, start=st, stop=sp)
                                    nc.tensor.matmul(mu0[:, 1, 0:257], attn[:, q, 128:256],
                                                     mv, start=st, stop=sp)
                                    nc.tensor.matmul(mu2[0:32, 0:257], attn[:, q, 256:288],
                                                     mv, start=st, stop=sp)
                        # normalize m (softmax denominator = col 256)
                        mnorm = work.tile([128, 3, 256], f32r, tag="mnorm", bufs=2, name="mnorm")
                        rs = work.tile([128, 3, 1], f32, tag="rs", bufs=2, name="rs")
                        for s in range(3):
                            mus = mu0[:, s, :] if s < 2 else mu2[0:32, :]
                            pp = 128 if s < 2 else 32
                            nc.vector.reciprocal(rs[0:pp, s, :], mus[0:pp, 256:257])
                            nc.vector.tensor_scalar(mnorm[0:pp, s, :], mus[0:pp, 0:256],
                                                    rs[0:pp, s, :], None, OP.mult)
                        # transpose m -> ch-major; alternate psum banks (aux
                        # and the just-freed mu2 slot) so TensorE doesn't wait
                        # on the DVE copy between the two c-tile groups
                        mT = work.tile([128, 2, 288], f32r, tag="mT", bufs=2, name="mT")
                        for ct in range(2):
                            tps = psum.tile([128, 288], f32r,
                                            tag="aux" if ct == 0 else "mu2",
                                            bufs=1, name="mtp")
                            for s in range(3):
                                pp = 128 if s < 2 else 32
                                nc.tensor.transpose(
                                    tps[:, s * 128:s * 128 + pp],
                                    mnorm[0:pp, s, ct * 128:(ct + 1) * 128],
                                    ident[0:pp, 0:pp])
                            nc.vector.tensor_copy(mT[:, ct, :], tps[:, 0:288])
                        # gate: g = sigmoid(gate_w m + b) via exp (stay on exp table)
                        gps = psum.tile([128, 2, 512], f32, tag="mu0", bufs=1, name="gps")
                        for oct in range(2):
                            for ict in range(2):
                                nc.tensor.matmul(
                                    gps[:, oct, 0:288],
                                    r32(gw_tt[:, ict, oct * 128:(oct + 1) * 128]),
                                    r32(mT[:, ict, :]),
                                    start=(ict == 0), stop=(ict == 1))
                        gtmp = work.tile([128, 2, 288], f32, tag="gtmp", bufs=2, name="gtmp")
                        for oct in range(2):
                            nc.scalar.activation(gtmp[:, oct, :], gps[:, oct, 0:288],
                                                 AF.Exp, bias=gnb_t[:, oct:oct + 1], scale=-1.0)
                        nc.vector.tensor_scalar(gtmp[...], gtmp[...], 1.0, None, OP.add)
                        nc.vector.reciprocal(gtmp[...], gtmp[...])
                        nc.vector.tensor_tensor(gtmp[...], gtmp[...], mT[...], OP.mult)
                        if j == 0:
                            nc.vector.tensor_copy(magg[:, :, i, :], gtmp[...])
                        else:
                            nc.vector.tensor_tensor(magg[:, :, i, :], magg[:, :, i, :],
                                                    gtmp[...], OP.add)

                # ---- boundary AG (magg + h 2-row halos, bf16) ----
                magg16 = work.tile([128, 2, NF, PL], bf16, tag="magg16", bufs=1,
                                   name="magg16")
                nc.vector.tensor_copy(magg16[...], magg[...])
                for f in range(NF):
                    for ct in range(2):
                        nc.sync.dma_start(
                            bdi[0, f, ct * 128:(ct + 1) * 128, 0:96], magg16[:, ct, f, 0:96])
                        nc.sync.dma_start(
                            bdi[0, f, ct * 128:(ct + 1) * 128, 96:192], magg16[:, ct, f, 192:288])
                bdo = dram.tile([NCORES, 2, NF, C, 4 * WF], bf16, tag="bdo", bufs=2,
                                addr_space="Shared", name="bdo")
                nc.gpsimd.collective_compute(
                    "AllGather", OP.bypass, replica_groups=RG,
                    ins=[bdi.opt()], outs=[bdo.opt()])

                # ---- halo extraction via one-hot rank masks ----
                # halo[kind][ct]: top rows (from rank k-1 bottom seg) & bottom rows
                halo = work.tile([128, 2, 2, NF, 2, 96], f32, tag="halo", bufs=1, name="halo")
                for kind in range(2):
                    for ct in range(2):
                        for rp in range(4):
                            ch = work.tile([128, 2, NF, 192], bf16, tag="hchk", bufs=1, name="hchk")
                            for _rr in range(2):
                                nc.sync.dma_start(
                                    ch[:, _rr, :, :],
                                    bdo[rp * 2 + _rr, kind, :, ct * 128:(ct + 1) * 128, :]
                                    .rearrange("f c x -> c f x"))
                            for rr in range(2):
                                r = rp * 2 + rr
                                for tb in range(2):
                                    sel = seltop_t if tb == 0 else selbot_t
                                    seg = ch[:, rr, :, 96:192] if tb == 0 else ch[:, rr, :, 0:96]
                                    dst = halo[:, kind, ct, :, tb, :]
                                    if r == 0:
                                        nc.vector.tensor_scalar(dst, seg, sel[:, 0:1],
                                                                None, OP.mult)
                                    else:
                                        nc.vector.scalar_tensor_tensor(
                                            dst, seg, sel[:, r:r + 1], dst,
                                            OP.mult, OP.add)

                # ---- ConvGRU ----
                mh = []
                for f in range(NF):
                    m_ = work.tile([128, 4, 512], f32r, tag="mh", bufs=3, name="mh")
                    nc.gpsimd.memset(m_[...].bitcast(f32), 0.0)
                    for ct in range(2):
                        rows = m_[:, ct, 6:506].rearrange("p (r w) -> p r w", w=50)
                        nc.vector.tensor_copy(
                            rows[:, 2:8, 1:49],
                            magg[:, ct, f, :].rearrange("p (r w) -> p r w", w=48))
                        nc.vector.tensor_copy(
                            rows[:, 0:2, 1:49],
                            halo[:, 0, ct, f, 0, :].rearrange("p (r w) -> p r w", w=48))
                        nc.vector.tensor_copy(
                            rows[:, 8:10, 1:49],
                            halo[:, 0, ct, f, 1, :].rearrange("p (r w) -> p r w", w=48))
                        hrows = m_[:, 2 + ct, 6:506].rearrange("p (r w) -> p r w", w=50)
                        nc.vector.tensor_copy(
                            hrows[:, 2:8, 1:49],
                            h_loc[:, ct, f, :].rearrange("p (r w) -> p r w", w=48))
                        nc.vector.tensor_copy(
                            hrows[:, 0:2, 1:49],
                            halo[:, 1, ct, f, 0, :].rearrange("p (r w) -> p r w", w=48))
                        nc.vector.tensor_copy(
                            hrows[:, 8:10, 1:49],
                            halo[:, 1, ct, f, 1, :].rearrange("p (r w) -> p r w", w=48))
                    mh.append(m_)

                def conv(wext, psums, NOUT, row0, src_of):
                    """9-tap conv: psums[f][:, mt, 0:NOUT] += taps."""
                    for tap in range(9):
                        dy, dx = tap // 3 - 1, tap % 3 - 1
                        wt = work.tile([128, 2, 4, 128], f32r, tag="wtap", bufs=2, name="wtap")
                        nc.sync.dma_start(wt[...], wext[tap])
                        for f in range(NF):
                            for kt in range(4):
                                mvs = src_of(f, kt, dy, dx)
                                if mvs is None:
                                    continue
                                for mt in range(2):
                                    nc.tensor.matmul(
                                        psums[f][:, mt, 0:NOUT],
                                        wt[:, mt, kt, :], r32(mvs),
                                        start=(tap == 0 and kt == 0),
                                        stop=(tap == 8 and kt == 3))

                def conv_psums():
                    ps = []
                    for f in range(NF):
                        tag = "pe" if f < 2 else "mu0"
                        ps.append(psum.tile([128, 2, 512], f32, tag=tag, bufs=2 if f < 2 else 1,
                                            name=f"cps{f}"))
                    return ps

                # z conv: out rows W2..W7 (own), N=300
                zps = conv_psums()
                conv(wz, zps, 300, 2,
                     lambda f, kt, dy, dx: mh[f][:, kt, 6 + (2 + dy) * 50 + dx:
                                                 6 + (2 + dy) * 50 + dx + 300])
                zgs = []
                for f in range(NF):
                    z_ = work.tile([128, 2, 300], f32, tag=f"zgs{f}", bufs=1, name="zgs")
                    for mt in range(2):
                        nc.scalar.activation(z_[:, mt, :], zps[f][:, mt, 0:300],
                                             AF.Sigmoid, bias=bz_t[:, mt:mt + 1])
                    zgs.append(z_)
                # r conv: out rows W1..W8, N=400
                rps = conv_psums()
                conv(wr, rps, 400, 1,
                     lambda f, kt, dy, dx: mh[f][:, kt, 6 + (1 + dy) * 50 + dx:
                                                 6 + (1 + dy) * 50 + dx + 400])
                rgh = []
                for f in range(NF):
                    # rg sigmoid written at 6-offset, then rg*h in place
                    rh_ = work.tile([128, 2, 416], f32r, tag=f"rgh{f}", bufs=1, name="rgh")
                    nc.gpsimd.memset(rh_[...].bitcast(f32), 0.0)
                    for mt in range(2):
                        nc.scalar.activation(rh_[:, mt, 6:406], rps[f][:, mt, 0:400],
                                             AF.Sigmoid, bias=br_t[:, mt:mt + 1])
                    nc.vector.tensor_tensor(
                        rh_[:, :, 6:406], rh_[:, :, 6:406],
                        mh[f][:, 2:4, 56:456], OP.mult)
                    rgh.append(rh_)
                # candidate conv: out rows W2..W7, N=300; inputs kt0-1 magg, kt2-3 rg*h
                hps = conv_psums()

                def hc_src(f, kt, dy, dx):
                    if kt < 2:
                        o = 6 + (2 + dy) * 50 + dx
                        return mh[f][:, kt, o:o + 300]
                    o = 6 + (1 + dy) * 50 + dx
                    return rgh[f][:, kt - 2, o:o + 300]

                conv(wh, hps, 300, 2, hc_src)
                for f in range(NF):
                    hc_ = work.tile([128, 2, 300], f32, tag="hcs", bufs=2, name="hcs")
                    for mt in range(2):
                        nc.scalar.activation(hc_[:, mt, :], hps[f][:, mt, 0:300],
                                             AF.Tanh, bias=bh_t[:, mt:mt + 1])
                    # h_new = h + z*(hc - h)
                    hold = mh[f][:, 2:4, 106:406]
                    nc.vector.tensor_tensor(hc_[...], hc_[...], hold, OP.subtract)
                    nc.vector.tensor_tensor(hc_[...], hc_[...], zgs[f][...], OP.mult)
                    for ct in range(2):
                        nc.vector.tensor_tensor(
                            h_loc[:, ct, f, :].rearrange("p (r w) -> p r w", w=48),
                            mh[f][:, 2 + ct, 106:406].rearrange("p (r w) -> p r w", w=50)[:, :, 1:49],
                            hc_[:, ct, :].rearrange("p (r w) -> p r w", w=50)[:, :, 1:49],
                            OP.add)

            # ---------------- readout ----------------
            hloc16f = work.tile([128, 2, NF, PL], bf16, tag="hloc16", bufs=1,
                                name="hloc16f")
            nc.vector.tensor_copy(hloc16f[...], h_loc[...])
            bdi2 = dram.tile([NF, C, 4 * WF], bf16, tag="bdi", bufs=2, name="bdi2")
            for f in range(NF):
                for ct in range(2):
                    nc.sync.dma_start(bdi2[f, ct * 128:(ct + 1) * 128, 0:96],
                                      hloc16f[:, ct, f, 0:96])
                    nc.sync.dma_start(bdi2[f, ct * 128:(ct + 1) * 128, 96:192],
                                      hloc16f[:, ct, f, 192:288])
            bdo2 = dram.tile([NCORES, NF, C, 4 * WF], bf16, tag="bdo", bufs=2,
                             addr_space="Shared", name="bdo2")
            nc.gpsimd.collective_compute(
                "AllGather", OP.bypass, replica_groups=RG,
                ins=[bdi2.opt()], outs=[bdo2.opt()])
            halo2 = work.tile([128, 2, NF, 2, 96], f32, tag="halo", bufs=1, name="halo2")
            for ct in range(2):
                for rp in range(4):
                    ch = work.tile([128, 2, NF, 192], bf16, tag="hchk", bufs=1, name="hchk2")
                    for _rr in range(2):
                        nc.sync.dma_start(
                            ch[:, _rr, :, :],
                            bdo2[rp * 2 + _rr, :, ct * 128:(ct + 1) * 128, :]
                            .rearrange("f c x -> c f x"))
                    for rr in range(2):
                        r = rp * 2 + rr
                        for tb in range(2):
                            sel = seltop_t if tb == 0 else selbot_t
                            seg = ch[:, rr, :, 96:192] if tb == 0 else ch[:, rr, :, 0:96]
                            dst = halo2[:, ct, :, tb, :]
                            if r == 0:
                                nc.vector.tensor_scalar(dst, seg, sel[:, 0:1], None, OP.mult)
                            else:
                                nc.vector.scalar_tensor_tensor(
                                    dst, seg, sel[:, r:r + 1], dst, OP.mult, OP.add)

            mask_sb = pers.tile([1, NF, RW * WF], f32)
            for f in range(NF):
                ro_in = work.tile([128, 4, 512], f32r, tag="mh", bufs=3, name="ro_in")
                nc.gpsimd.memset(ro_in[...].bitcast(f32), 0.0)
                for ct in range(2):
                    hrows = ro_in[:, ct, 6:506].rearrange("p (r w) -> p r w", w=50)
                    nc.vector.tensor_copy(
                        hrows[:, 2:8, 1:49],
                        h_loc[:, ct, f, :].rearrange("p (r w) -> p r w", w=48))
                    nc.vector.tensor_copy(
                        hrows[:, 0:2, 1:49],
                        halo2[:, ct, f, 0, :].rearrange("p (r w) -> p r w", w=48))
                    nc.vector.tensor_copy(
                        hrows[:, 8:10, 1:49],
                        halo2[:, ct, f, 1, :].rearrange("p (r w) -> p r w", w=48))
                    vrows = ro_in[:, 2 + ct, 6:506].rearrange("p (r w) -> p r w", w=50)
                    nc.vector.tensor_copy(
                        vrows[:, 0:10, 1:49],
                        v_sb[:, ct, f, :].rearrange("p (r w) -> p r w", w=48))
                # y = relu(ro1 * cat) rows W1..W8 (N=400)
                yps = psum.tile([128, 2, 512], f32, tag="pe", bufs=2, name="yps")
                for tap in range(9):
                    dy, dx = tap // 3 - 1, tap % 3 - 1
                    wt = work.tile([128, 2, 4, 128], f32r, tag="wtap", bufs=2, name="wtap2")
                    nc.sync.dma_start(wt[...], ro1[tap])
                    o = 6 + (1 + dy) * 50 + dx
                    for kt in range(4):
                        for mt in range(2):
                            nc.tensor.matmul(
                                yps[:, mt, 0:400], wt[:, mt, kt, :],
                                r32(ro_in[:, kt, o:o + 400]),
                                start=(tap == 0 and kt == 0), stop=(tap == 8 and kt == 3))
                y_sb = work.tile([128, 2, 412], f32r, tag="y_sb", bufs=1, name="y_sb")
                nc.gpsimd.memset(y_sb[...].bitcast(f32), 0.0)
                for mt in range(2):
                    ypr = yps[:, mt, 0:400].rearrange("p (r w) -> p r w", w=50)
                    ydst = y_sb[:, mt, 6:406].rearrange("p (r w) -> p r w", w=50)
                    # y rows: 0 (global 6k-1, invalid on core 0), 1..7, 7 (invalid on core 7)
                    nc.scalar.activation(ydst[:, 0:1, 1:49], ypr[:, 0:1, 1:49], AF.Relu,
                                         bias=rb1top_t[:, mt:mt + 1], scale=sctop_t[:, :])
                    nc.scalar.activation(ydst[:, 1:7, 1:49], ypr[:, 1:7, 1:49], AF.Relu,
                                         bias=rb1_t[:, mt:mt + 1])
                    nc.scalar.activation(ydst[:, 7:8, 1:49], ypr[:, 7:8, 1:49], AF.Relu,
                                         bias=rb1bot_t[:, mt:mt + 1], scale=scbot_t[:, :])
                # mask = ro2 * y + b2, own rows (N=300 in 50-col layout; y pad
                # cols are zero so the windows are safe)
                mps = psum.tile([1, 300], f32, tag="aux", bufs=1, name="mps")
                for tap in range(9):
                    dy, dx = tap // 3 - 1, tap % 3 - 1
                    o = 6 + (1 + dy) * 50 + dx
                    for ct in range(2):
                        nc.tensor.matmul(
                            mps[0:1, 0:300],
                            r32(ro2_t[:, ct, tap:tap + 1]),
                            r32(y_sb[:, ct, o:o + 300]),
                            start=(tap == 0 and ct == 0), stop=(tap == 8 and ct == 1))
                nc.scalar.activation(
                    mask_sb[0:1, f, :].rearrange("p (r w) -> p r w", w=48),
                    mps[0:1, :].rearrange("p (r w) -> p r w", w=50)[:, :, 1:49],
                    AF.Identity, bias=rb2_t[0:1, :])
            nc.sync.dma_start(out_ext.ap(), mask_sb[0:1, :, :])

    nc.finalize()
    return nc


def _prep_inputs(inputs):
    """Host-side weight prep + per-core shards."""
    frames = np.ascontiguousarray(inputs['frames'], dtype=np.float32)  # (1,3,3,384,384)
    bb_w = np.asarray(inputs['backbone_w'], dtype=np.float32)
    bb_b = np.asarray(inputs['backbone_b'], dtype=np.float32).reshape(C, 1)
    W_intra = np.asarray(inputs['W_intra'], dtype=np.float32)
    W_inter = np.asarray(inputs['W_inter'], dtype=np.float32)
    gate_w = np.asarray(inputs['gate_w'], dtype=np.float32)[:, :, 0, 0]
    gate_b = np.asarray(inputs['gate_b'], dtype=np.float32).reshape(C, 1)

    def taps(w):
        return np.ascontiguousarray(
            np.asarray(w, dtype=np.float32).transpose(2, 3, 1, 0).reshape(9, 2 * C, C))

    def swz(w):
        """[c, d] (256x256) -> [128, 2ct, d]"""
        return np.ascontiguousarray(w.reshape(2, 128, C).transpose(1, 0, 2))

    def bias2(b):
        """(256,1) -> (128, 2)"""
        return np.ascontiguousarray(b.reshape(2, 128).T)

    def conv_taps(w):
        """(O,I,3,3) -> [9, kp(128), mt, kt, mp(128)]"""
        t = taps(w)                                    # (9, 512, 256)
        t = t.reshape(9, 4, 128, 2, 128)               # tap, kt, kp, mt, mp
        return np.ascontiguousarray(t.transpose(0, 2, 3, 1, 4))

    bbw192 = bb_w.transpose(1, 2, 3, 0).reshape(192, C)
    ro2_flat = np.asarray(inputs['ro_w2'], dtype=np.float32
                          ).transpose(2, 3, 1, 0).reshape(9, C)      # tap, c
    common = {
        'bbw0': np.ascontiguousarray(bbw192[0:128]),
        'bbw1': np.ascontiguousarray(bbw192[128:192]),
        'bbb': bias2(bb_b),
        'w_int': swz(W_inter),
        'w_inta': swz(W_intra),
        'gw_t': swz(np.ascontiguousarray(gate_w.T)),
        'gb_neg': bias2(-gate_b),
        'wz': conv_taps(inputs['Wz']), 'wr': conv_taps(inputs['Wr']),
        'wh': conv_taps(inputs['Wh']),
        'bz': bias2(np.asarray(inputs['bz'], dtype=np.float32)),
        'br': bias2(np.asarray(inputs['br'], dtype=np.float32)),
        'bh': bias2(np.asarray(inputs['bh'], dtype=np.float32)),
        'ro1': conv_taps(inputs['ro_w1']),
        'rb1': bias2(np.asarray(inputs['ro_b1'], dtype=np.float32)),
        'ro2': np.ascontiguousarray(
            ro2_flat.T.reshape(2, 128, 9).transpose(1, 0, 2)),
        'rb2': np.asarray(inputs['ro_b2'], dtype=np.float32).reshape(1, 1),
        'ident_in': np.eye(128, dtype=np.float32),
    }

    fp = np.zeros((NF, 3, 384 + 32, 384), np.float32)
    fp[:, :, 16:400] = frames[0]

    in_maps = []
    for k in range(NCORES):
        m = dict(common)
        # patches [192=(ch,dy,dx), f, 10 rows x 48]
        pc = fp[:, :, 48 * k:48 * k + 80, :].reshape(NF, 3, 10, 8, 48, 8)
        m['patches'] = np.ascontiguousarray(
            pc.transpose(1, 3, 5, 0, 2, 4).reshape(192, NF, 480))
        st = np.zeros((128, 8), np.float32)
        sb = np.zeros((128, 8), np.float32)
        if k > 0:
            st[:, k - 1] = 1.0
        if k < NCORES - 1:
            sb[:, k + 1] = 1.0
        m['sel_top'] = st
        m['sel_bot'] = sb
        sct = np.full((128, 1), 0.0 if k == 0 else 1.0, np.float32)
        scb = np.full((128, 1), 0.0 if k == NCORES - 1 else 1.0, np.float32)
        m['sc_top'] = sct
        m['sc_bot'] = scb
        m['bbb_top'] = common['bbb'] * sct[0, 0]
        m['bbb_bot'] = common['bbb'] * scb[0, 0]
        m['rb1_top'] = common['rb1'] * sct[0, 0]
        m['rb1_bot'] = common['rb1'] * scb[0, 0]
        in_maps.append(m)
    return in_maps


def run_cores(inputs, trace=False):
    """Returns (per_core_results, BassKernelResults)."""
    sys.path.insert(0, '/opt/trn_rl_repo')
    from concourse.bass_utils import run_bass_kernel_spmd
    if 'nc' not in _CACHE:
        _CACHE['nc'] = _build_graph()
    nc = _CACHE['nc']
    in_maps = _prep_inputs(inputs)
    res = run_bass_kernel_spmd(nc, in_maps, core_ids=list(range(NCORES)), trace=trace)
    return res


def kernel(**inputs):
    res = run_cores(inputs, trace=False)
    out = np.zeros((1, NF, 1, HF, WF), np.float32)
    for k in range(NCORES):
        out[0, :, 0, RW * k:RW * (k + 1), :] = res.results[k]['out']
    return out


if __name__ == '__main__':
    data = np.load('/tmp/ref_inputs.npz')
    inputs = {k: data[k] for k in data.files}
    out = kernel(**inputs)
    ref = np.load('/tmp/ref_out.npy')
    rel = np.linalg.norm(out - ref) / np.linalg.norm(ref)
    print('rel l2 err:', rel)


# revision 25
# speedup vs baseline: 44.7492x; 1.0123x over previous
"""Trainium2 Bass kernel for nn_AGNN (3-frame attentional GNN + ConvGRU).

Self-contained: builds an 8-core SPMD Bass graph (sequence-parallel over the
48x48 spatial tokens, 6 rows per core), runs it via run_bass_kernel_spmd,
and reassembles the full output.

Sharding: each core owns 6 rows (288 tokens) of every frame. Per iteration:
  AllGather h (bf16, ch-major + tok-major layouts) -> each core computes
  attention for its 288 query tokens against all 2304 keys of each frame
  (9 ordered frame pairs), gated aggregation, then a 4-row boundary
  AllGather (magg + h) feeds the halo rows of the 3x3 ConvGRU which each
  core evaluates for its own rows.  Readout convs are local (v computed
  with halo from the raw frames; h halo from a final boundary exchange).

Precision: attention matmuls in bf16 (fp32 PSUM accumulation), everything
else float32r (tf32 matmul mode).  Validated ~5.5e-4 rel error vs the
fp32 reference in simulation.
"""
import sys
import numpy as np

NF = 3          # frames
C = 256         # channels
HF = WF = 48    # feature map
P = HF * WF     # 2304 tokens/frame
NCORES = 8
RW = 6          # rows per core
PL = RW * WF    # 288 tokens per core
K_ITERS = 3

_CACHE = {}


def _build_graph():
    sys.path.insert(0, '/opt/trn_rl_repo')
    import concourse.bass as bass
    import concourse.mybir as mybir
    import concourse.tile as tile
    from concourse import bacc

    dt = mybir.dt
    f32 = dt.float32
    f32r = dt.float32r
    bf16 = dt.bfloat16
    AF = mybir.ActivationFunctionType
    OP = mybir.AluOpType
    RG = [list(range(NCORES))]

    nc = bacc.Bacc()

    # ---------------- external IO ----------------
    def ein(name, shape, dtype=None):
        return nc.dram_tensor(name, list(shape), dtype or f32, kind="ExternalInput")

    patches = ein("patches", (192, NF, 480), f32r)       # host patch-extract, rows 6k-2..6k+7
    bbw0 = ein("bbw0", (128, C), f32r)
    bbw1 = ein("bbw1", (64, C), f32r)
    bbb = ein("bbb", (128, 2))
    bbb_top = ein("bbb_top", (128, 2))
    bbb_bot = ein("bbb_bot", (128, 2))
    w_int = ein("w_int", (128, 2, C), f32r)              # W_inter [c, d] swizzled
    w_inta = ein("w_inta", (128, 2, C), f32r)
    gw_t = ein("gw_t", (128, 2, C), f32r)                # gate_w^T [i, o] swizzled
    gb_neg = ein("gb_neg", (128, 2))                       # -gate_b
    wz = ein("wz", (9, 128, 2, 4, 128), f32r)            # [tap, kp, mt, kt, mp]
    wr = ein("wr", (9, 128, 2, 4, 128), f32r)
    wh = ein("wh", (9, 128, 2, 4, 128), f32r)
    bz = ein("bz", (128, 2))
    br = ein("br", (128, 2))
    bh = ein("bh", (128, 2))
    ro1 = ein("ro1", (9, 128, 2, 4, 128), f32r)
    rb1 = ein("rb1", (128, 2))
    rb1_top = ein("rb1_top", (128, 2))
    rb1_bot = ein("rb1_bot", (128, 2))
    ro2 = ein("ro2", (128, 2, 9), f32r)
    rb2 = ein("rb2", (1, 1))
    sel_top = ein("sel_top", (128, 8))                   # one-hot rank k-1 (zeros at core 0)
    sel_bot = ein("sel_bot", (128, 8))                   # one-hot rank k+1 (zeros at core 7)
    sc_top = ein("sc_top", (128, 1))                     # 0.0 on core 0 else 1.0
    sc_bot = ein("sc_bot", (128, 1))                     # 0.0 on core 7 else 1.0
    ident_in = ein("ident_in", (128, 128), f32r)

    out_ext = nc.dram_tensor("out", [NF, RW, WF], f32, kind="ExternalOutput")

    SH = NF * C * PL            # 221184 elements per layout shard

    with tile.TileContext(nc) as tc:
        with (
            tc.tile_pool(name="pers", bufs=1) as pers,
            tc.tile_pool(name="dram", bufs=1, space="DRAM") as dram,
            tc.tile_pool(name="psum", bufs=1, space="PSUM") as psum,
            tc.tile_pool(name="work", bufs=1) as work,
        ):
            # ---------------- persistent SBUF ----------------
            ident = pers.tile([128, 128], f32r)
            nc.sync.dma_start(ident[...], ident_in.ap())

            def load_pers(name, ext, shape, view=None):
                t = pers.tile(list(shape), f32r, name=name)
                src = ext.ap() if view is None else view
                nc.sync.dma_start(t[...], src)
                return t

            # W_inter/W_intra/gate_w^T as [128, 2ct, 256]
            wint_t = load_pers("wint_t", w_int, (128, 2, C))
            winta_t = load_pers("winta_t", w_inta, (128, 2, C))
            gw_tt = load_pers("gw_tt", gw_t, (128, 2, C))
            bbw0_t = load_pers("bbw0_t", bbw0, (128, C))
            bbw1_t = load_pers("bbw1_t", bbw1, (64, C))
            ro2_t = load_pers("ro2_t", ro2, (128, 2, 9))

            def load_bias(name, ext):
                t = pers.tile([128, 2], f32, name=name)
                nc.sync.dma_start(t[...], ext.ap())
                return t

            bbb_t = load_bias("bbb_t", bbb)
            bbbtop_t = load_bias("bbbtop_t", bbb_top)
            bbbbot_t = load_bias("bbbbot_t", bbb_bot)
            gnb_t = load_bias("gnb_t", gb_neg)
            bz_t = load_bias("bz_t", bz)
            br_t = load_bias("br_t", br)
            bh_t = load_bias("bh_t", bh)
            rb1_t = load_bias("rb1_t", rb1)
            rb1top_t = load_bias("rb1top_t", rb1_top)
            rb1bot_t = load_bias("rb1bot_t", rb1_bot)
            rb2_t = pers.tile([1, 1], f32)
            nc.sync.dma_start(rb2_t[...], rb2.ap())
            seltop_t = pers.tile([128, 8], f32)
            nc.sync.dma_start(seltop_t[...], sel_top.ap())
            selbot_t = pers.tile([128, 8], f32)
            nc.sync.dma_start(selbot_t[...], sel_bot.ap())
            sctop_t = pers.tile([128, 1], f32)
            nc.sync.dma_start(sctop_t[...], sc_top.ap())
            scbot_t = pers.tile([128, 1], f32)
            nc.sync.dma_start(scbot_t[...], sc_bot.ap())

            # big persistent state
            v_sb = pers.tile([128, 2, NF, 10 * WF], f32)      # v rows 6k-2..6k+7
            h_loc = pers.tile([128, 2, NF, PL], f32r)          # own rows, ch-major
            magg = pers.tile([128, 2, NF, PL], f32r)
            t_sb = pers.tile([128, 2, 6, PL], bf16)           # t^T (3 inter + 3 intra)

            def r32(ap):
                return ap.bitcast(f32r)

            # ---------------- backbone ----------------
            with tc.tile_pool(name="bb", bufs=1) as bb:
                pk0 = bb.tile([128, NF, 480], f32r, name="pk0")
                pk1 = bb.tile([64, NF, 480], f32r, name="pk1")
                nc.sync.dma_start(pk0[...], patches[0:128])
                nc.sync.dma_start(pk1[...], patches[128:192])
                for f in range(NF):
                    vps = psum.tile([128, 2, 512], f32, tag="pe", bufs=2, name="vps")
                    for mt in range(2):
                        nc.tensor.matmul(vps[:, mt, 0:480],
                                         bbw0_t[:, mt * 128:(mt + 1) * 128],
                                         pk0[:, f, :], start=True, stop=False)
                        nc.tensor.matmul(vps[:, mt, 0:480],
                                         bbw1_t[0:64, mt * 128:(mt + 1) * 128],
                                         pk1[0:64, f, :], start=False, stop=True)
                    for mt in range(2):
                        # rows 0-1 / 2-7 / 8-9 with edge masking (v=0 outside image)
                        nc.scalar.activation(v_sb[:, mt, f, 0:96], vps[:, mt, 0:96],
                                             AF.Relu, bias=bbbtop_t[:, mt:mt + 1], scale=sctop_t[:, :])
                        nc.scalar.activation(v_sb[:, mt, f, 96:384], vps[:, mt, 96:384],
                                             AF.Relu, bias=bbb_t[:, mt:mt + 1])
                        nc.scalar.activation(v_sb[:, mt, f, 384:480], vps[:, mt, 384:480],
                                             AF.Relu, bias=bbbbot_t[:, mt:mt + 1], scale=scbot_t[:, :])
                        # h0 = v own rows (rows 2..8 of the 10-row window)
                        nc.vector.tensor_copy(h_loc[:, mt, f, :], v_sb[:, mt, f, 96:384])

            # ---------------- iterations ----------------
            for it in range(K_ITERS):
                # ---- write AG input: ch-major + tok-major (bf16) ----
                agi_parts = [
                    dram.tile([2, 1, SH // NF], bf16, tag="agi0", bufs=2, name="agi0"),
                    dram.tile([2, 2, SH // NF], bf16, tag="agi12", bufs=2, name="agi12"),
                ]

                def agi_v(layout, f):
                    part = agi_parts[0] if f == 0 else agi_parts[1]
                    return part[layout, 0 if f == 0 else f - 1]
                hloc16 = work.tile([128, 2, NF, PL], bf16, tag="hloc16", bufs=1, name="hloc16")
                nc.vector.tensor_copy(hloc16[...], h_loc[...])
                for f in range(NF):
                    for ct in range(2):
                        nc.sync.dma_start(
                            agi_v(0, f).rearrange("(c t) -> c t", t=PL)[ct * 128:(ct + 1) * 128, :],
                            hloc16[:, ct, f, :])
                    # tok-major via TensorE transposes: [c,96tok] -> [96tok, 128c]
                    # (alternate psum banks so TensorE doesn't stall on the
                    # DVE copy of the previous block)
                    hlt = work.tile([96, 3, C], bf16, tag="hlt", bufs=2, name="hlt")
                    for ct in range(2):
                        for ps in range(3):
                            tp = psum.tile([96, 128], f32r,
                                           tag="aux" if (ct * 3 + ps) % 2 == 0 else "mu2",
                                           bufs=1, name="tp")
                            nc.tensor.transpose(
                                tp[0:96, 0:128],
                                h_loc[:, ct, f, ps * 96:(ps + 1) * 96],
                                ident[:, :])
                            nc.vector.tensor_copy(hlt[0:96, ps, ct * 128:(ct + 1) * 128],
                                                  tp[0:96, 0:128])
                    nc.sync.dma_start(
                        agi_v(1, f).rearrange("(t c) -> t c", c=C)
                        .rearrange("(ps p) c -> p ps c", p=96),
                        hlt[0:96, :, :])

                ago_parts = [
                    dram.tile([NCORES, 2, 1, SH // NF], bf16, tag="ago0", bufs=2,
                              addr_space="Shared", name="ago0"),
                    dram.tile([NCORES, 2, 2, SH // NF], bf16, tag="ago12", bufs=2,
                              addr_space="Shared", name="ago12"),
                ]
                for _p in range(2):
                    nc.gpsimd.collective_compute(
                        "AllGather", OP.bypass, replica_groups=RG,
                        ins=[agi_parts[_p].opt()], outs=[ago_parts[_p].opt()])

                def ago_v(layout, f):
                    part = ago_parts[0] if f == 0 else ago_parts[1]
                    return part[:, layout, 0 if f == 0 else f - 1]

                # ---- t = h_loc @ W (both kinds), bf16 out ----
                for i in range(NF):
                    for kind in range(2):       # 0 inter, 1 intra
                        wsel = wint_t if kind == 0 else winta_t
                        for dct in range(2):
                            tps = psum.tile([128, 2, 512], f32, tag="pe", bufs=2, name="tps")
                            for ct in range(2):
                                nc.tensor.matmul(
                                    tps[:, 0, 0:288],
                                    r32(wsel[:, ct, dct * 128:(dct + 1) * 128]),
                                    r32(h_loc[:, ct, i, :]),
                                    start=(ct == 0), stop=(ct == 1))
                            nc.vector.tensor_copy(t_sb[:, dct, kind * 3 + i, :],
                                                  tps[:, 0, 0:288])

                # ---- boundary AG input: magg written later; h part now ----
                bdi = dram.tile([2, NF, C, 4 * WF], bf16, tag="bdi", bufs=2, name="bdi")
                for f in range(NF):
                    for ct in range(2):
                        nc.sync.dma_start(
                            bdi[1, f, ct * 128:(ct + 1) * 128, 0:96], hloc16[:, ct, f, 0:96])
                        nc.sync.dma_start(
                            bdi[1, f, ct * 128:(ct + 1) * 128, 96:192], hloc16[:, ct, f, 192:288])

                # ---- attention over j (keys) and i (queries) ----
                # stage all frames contiguously in DRAM up front (rank chunks
                # are 288 tokens; 128-token tiles cross rank boundaries
                # otherwise)
                stgs, stgcs = [], []
                for j in range(NF):
                    stg = dram.tile([P, C], bf16, tag="stg", bufs=3, name="stg")
                    nc.sync.dma_start(stg[:, :], ago_v(1, j))
                    stgs.append(stg)
                    stgc = dram.tile([C, P], bf16, tag="stgc", bufs=3, name="stgc")
                    nc.sync.dma_start(
                        stgc[:, :].rearrange("c (r t) -> r c t", r=NCORES),
                        ago_v(0, j))
                    stgcs.append(stgc)
                for j in range(NF):
                    stg, stgc = stgs[j], stgcs[j]
                    hch = []
                    for hh in range(2):
                        t_ = work.tile([128, 2, 9, 128], bf16, tag="hch", bufs=3, name="hch")
                        for ct in range(2):
                            nc.sync.dma_start(
                                t_[:, ct, :, :],
                                stgc[ct * 128:(ct + 1) * 128, :]
                                .rearrange("p (q x) -> p q x", x=128)
                                [:, hh * 9:(hh + 1) * 9, :])
                        hch.append(t_)
                    htok = []
                    for hh in range(2):
                        t_ = work.tile([128, 9, 257], bf16, tag="htok", bufs=3, name="htok")
                        nc.gpsimd.memset(t_[...], 1.0)
                        nc.sync.dma_start(
                            t_[:, :, 0:256],
                            stg[:, :].rearrange("(q p) c -> p q c", p=128)
                            [:, hh * 9:(hh + 1) * 9, :])
                        htok.append(t_)

                    for i in range(NF):
                        tix = (3 + i) if i == j else i
                        attn = work.tile([128, 18, 288], bf16, tag="attn", bufs=2, name="attn")
                        mu0 = psum.tile([128, 2, 512], f32, tag="mu0", bufs=1, name="mu0")
                        mu2 = psum.tile([32, 257], f32, tag="mu2", bufs=1, name="mu2")
                        # software-pipelined: e/exp group g, then m-matmuls of g-1
                        for g in range(10):
                            if g < 9:
                                e2 = psum.tile([128, 2, 512], f32, tag="pe", bufs=2, name="e2")
                                for u in range(2):
                                    q = g * 2 + u
                                    for ct in range(2):
                                        nc.tensor.matmul(
                                            e2[:, u, 0:288],
                                            hch[q // 9][:, ct, q % 9, :],
                                            t_sb[:, ct, tix, :],
                                            start=(ct == 0), stop=(ct == 1))
                                nc.scalar.activation(attn[:, g * 2:g * 2 + 2, :],
                                                     e2[:, :, 0:288], AF.Exp)
                            if g >= 1:
                                for u in range(2):
                                    q = (g - 1) * 2 + u
                                    st = (q == 0)
                                    sp = (q == 17)
                                    mv = htok[q // 9][:, q % 9, :]
                                    nc.tensor.matmul(mu0[:, 0, 0:257], attn[:, q, 0:128],
                                                     mv, start=st, stop=sp)
                                    nc.tensor.matmul(mu0[:, 1, 0:257], attn[:, q, 128:256],
                                                     mv, start=st, stop=sp)
                                    nc.tensor.matmul(mu2[0:32, 0:257], attn[:, q, 256:288],
                                                     mv, start=st, stop=sp)
                        # normalize m (softmax denominator = col 256)
                        mnorm = work.tile([128, 3, 256], f32r, tag="mnorm", bufs=2, name="mnorm")
                        rs = work.tile([128, 3, 1], f32, tag="rs", bufs=2, name="rs")
                        for s in range(3):
                            mus = mu0[:, s, :] if s < 2 else mu2[0:32, :]
                            pp = 128 if s < 2 else 32
                            nc.vector.reciprocal(rs[0:pp, s, :], mus[0:pp, 256:257])
                            nc.vector.tensor_scalar(mnorm[0:pp, s, :], mus[0:pp, 0:256],
                                                    rs[0:pp, s, :], None, OP.mult)
                        # transpose m -> ch-major; alternate psum banks (aux
                        # and the just-freed mu2 slot) so TensorE doesn't wait
                        # on the DVE copy between the two c-tile groups
                        mT = work.tile([128, 2, 288], f32r, tag="mT", bufs=2, name="mT")
                        for ct in range(2):
                            tps = psum.tile([128, 288], f32r,
                                            tag="aux" if ct == 0 else "mu2",
                                            bufs=1, name="mtp")
                            for s in range(3):
                                pp = 128 if s < 2 else 32
                                nc.tensor.transpose(
                                    tps[:, s * 128:s * 128 + pp],
                                    mnorm[0:pp, s, ct * 128:(ct + 1) * 128],
                                    ident[0:pp, 0:pp])
                            nc.vector.tensor_copy(mT[:, ct, :], tps[:, 0:288])
                        # gate: g = sigmoid(gate_w m + b) via exp (stay on exp table)
                        gps = psum.tile([128, 2, 512], f32, tag="mu0", bufs=1, name="gps")
                        for oct in range(2):
                            for ict in range(2):
                                nc.tensor.matmul(
                                    gps[:, oct, 0:288],
                                    r32(gw_tt[:, ict, oct * 128:(oct + 1) * 128]),
                                    r32(mT[:, ict, :]),
                                    start=(ict == 0), stop=(ict == 1))
                        gtmp = work.tile([128, 2, 288], f32, tag="gtmp", bufs=2, name="gtmp")
                        for oct in range(2):
                            nc.scalar.activation(gtmp[:, oct, :], gps[:, oct, 0:288],
                                                 AF.Exp, bias=gnb_t[:, oct:oct + 1], scale=-1.0)
                        nc.vector.tensor_scalar(gtmp[...], gtmp[...], 1.0, None, OP.add)
                        nc.vector.reciprocal(gtmp[...], gtmp[...])
                        nc.vector.tensor_tensor(gtmp[...], gtmp[...], mT[...], OP.mult)
                        if j == 0:
                            nc.vector.tensor_copy(magg[:, :, i, :], gtmp[...])
                        else:
                            nc.vector.tensor_tensor(magg[:, :, i, :], magg[:, :, i, :],
                                                    gtmp[...], OP.add)

                # ---- boundary AG (magg + h 2-row halos, bf16) ----
                magg16 = work.tile([128, 2, NF, PL], bf16, tag="magg16", bufs=1,
                                   name="magg16")
                nc.vector.tensor_copy(magg16[...], magg[...])
                for f in range(NF):
                    for ct in range(2):
                        nc.sync.dma_start(
                            bdi[0, f, ct * 128:(ct + 1) * 128, 0:96], magg16[:, ct, f, 0:96])
                        nc.sync.dma_start(
                            bdi[0, f, ct * 128:(ct + 1) * 128, 96:192], magg16[:, ct, f, 192:288])
                bdo = dram.tile([NCORES, 2, NF, C, 4 * WF], bf16, tag="bdo", bufs=2,
                                addr_space="Shared", name="bdo")
                nc.gpsimd.collective_compute(
                    "AllGather", OP.bypass, replica_groups=RG,
                    ins=[bdi.opt()], outs=[bdo.opt()])

                # ---- halo extraction via one-hot rank masks ----
                # halo[kind][ct]: top rows (from rank k-1 bottom seg) & bottom rows
                halo = work.tile([128, 2, 2, NF, 2, 96], f32, tag="halo", bufs=1, name="halo")
                for kind in range(2):
                    for ct in range(2):
                        for rp in range(4):
                            ch = work.tile([128, 2, NF, 192], bf16, tag="hchk", bufs=1, name="hchk")
                            for _rr in range(2):
                                nc.sync.dma_start(
                                    ch[:, _rr, :, :],
                                    bdo[rp * 2 + _rr, kind, :, ct * 128:(ct + 1) * 128, :]
                                    .rearrange("f c x -> c f x"))
                            for rr in range(2):
                                r = rp * 2 + rr
                                for tb in range(2):
                                    sel = seltop_t if tb == 0 else selbot_t
                                    seg = ch[:, rr, :, 96:192] if tb == 0 else ch[:, rr, :, 0:96]
                                    dst = halo[:, kind, ct, :, tb, :]
                                    if r == 0:
                                        nc.vector.tensor_scalar(dst, seg, sel[:, 0:1],
                                                                None, OP.mult)
                                    else:
                                        nc.vector.scalar_tensor_tensor(
                                            dst, seg, sel[:, r:r + 1], dst,
                                            OP.mult, OP.add)

                # ---- ConvGRU ----
                mh = []
                for f in range(NF):
                    m_ = work.tile([128, 4, 512], f32r, tag="mh", bufs=3, name="mh")
                    nc.gpsimd.memset(m_[...].bitcast(f32), 0.0)
                    for ct in range(2):
                        rows = m_[:, ct, 6:506].rearrange("p (r w) -> p r w", w=50)
                        nc.vector.tensor_copy(
                            rows[:, 2:8, 1:49],
                            magg[:, ct, f, :].rearrange("p (r w) -> p r w", w=48))
                        nc.vector.tensor_copy(
                            rows[:, 0:2, 1:49],
                            halo[:, 0, ct, f, 0, :].rearrange("p (r w) -> p r w", w=48))
                        nc.vector.tensor_copy(
                            rows[:, 8:10, 1:49],
                            halo[:, 0, ct, f, 1, :].rearrange("p (r w) -> p r w", w=48))
                        hrows = m_[:, 2 + ct, 6:506].rearrange("p (r w) -> p r w", w=50)
                        nc.vector.tensor_copy(
                            hrows[:, 2:8, 1:49],
                            h_loc[:, ct, f, :].rearrange("p (r w) -> p r w", w=48))
                        nc.vector.tensor_copy(
                            hrows[:, 0:2, 1:49],
                            halo[:, 1, ct, f, 0, :].rearrange("p (r w) -> p r w", w=48))
                        nc.vector.tensor_copy(
                            hrows[:, 8:10, 1:49],
                            halo[:, 1, ct, f, 1, :].rearrange("p (r w) -> p r w", w=48))
                    mh.append(m_)

                def conv(wext, psums, NOUT, row0, src_of):
                    """9-tap conv: psums[f][:, mt, 0:NOUT] += taps."""
                    for tap in range(9):
                        dy, dx = tap // 3 - 1, tap % 3 - 1
                        wt = work.tile([128, 2, 4, 128], f32r, tag="wtap", bufs=2, name="wtap")
                        nc.sync.dma_start(wt[...], wext[tap])
                        for f in range(NF):
                            for kt in range(4):
                                mvs = src_of(f, kt, dy, dx)
                                if mvs is None:
                                    continue
                                for mt in range(2):
                                    nc.tensor.matmul(
                                        psums[f][:, mt, 0:NOUT],
                                        wt[:, mt, kt, :], r32(mvs),
                                        start=(tap == 0 and kt == 0),
                                        stop=(tap == 8 and kt == 3))

                def conv_psums():
                    ps = []
                    for f in range(NF):
                        tag = "pe" if f < 2 else "mu0"
                        ps.append(psum.tile([128, 2, 512], f32, tag=tag, bufs=2 if f < 2 else 1,
                                            name=f"cps{f}"))
                    return ps

                # z conv: out rows W2..W7 (own), N=300
                zps = conv_psums()
                conv(wz, zps, 300, 2,
                     lambda f, kt, dy, dx: mh[f][:, kt, 6 + (2 + dy) * 50 + dx:
                                                 6 + (2 + dy) * 50 + dx + 300])
                zgs = []
                for f in range(NF):
                    z_ = work.tile([128, 2, 300], f32, tag=f"zgs{f}", bufs=1, name="zgs")
                    for mt in range(2):
                        nc.scalar.activation(z_[:, mt, :], zps[f][:, mt, 0:300],
                                             AF.Sigmoid, bias=bz_t[:, mt:mt + 1])
                    zgs.append(z_)
                # r conv: out rows W1..W8, N=400
                rps = conv_psums()
                conv(wr, rps, 400, 1,
                     lambda f, kt, dy, dx: mh[f][:, kt, 6 + (1 + dy) * 50 + dx:
                                                 6 + (1 + dy) * 50 + dx + 400])
                rgh = []
                for f in range(NF):
                    # rg sigmoid written at 6-offset, then rg*h in place
                    rh_ = work.tile([128, 2, 416], f32r, tag=f"rgh{f}", bufs=1, name="rgh")
                    nc.gpsimd.memset(rh_[...].bitcast(f32), 0.0)
                    for mt in range(2):
                        nc.scalar.activation(rh_[:, mt, 6:406], rps[f][:, mt, 0:400],
                                             AF.Sigmoid, bias=br_t[:, mt:mt + 1])
                    nc.vector.tensor_tensor(
                        rh_[:, :, 6:406], rh_[:, :, 6:406],
                        mh[f][:, 2:4, 56:456], OP.mult)
                    rgh.append(rh_)
                # candidate conv: out rows W2..W7, N=300; inputs kt0-1 magg, kt2-3 rg*h
                hps = conv_psums()

                def hc_src(f, kt, dy, dx):
                    if kt < 2:
                        o = 6 + (2 + dy) * 50 + dx
                        return mh[f][:, kt, o:o + 300]
                    o = 6 + (1 + dy) * 50 + dx
                    return rgh[f][:, kt - 2, o:o + 300]

                conv(wh, hps, 300, 2, hc_src)
                for f in range(NF):
                    hc_ = work.tile([128, 2, 300], f32, tag="hcs", bufs=2, name="hcs")
                    for mt in range(2):
                        nc.scalar.activation(hc_[:, mt, :], hps[f][:, mt, 0:300],
                                             AF.Tanh, bias=bh_t[:, mt:mt + 1])
                    # h_new = h + z*(hc - h)
                    hold = mh[f][:, 2:4, 106:406]
                    nc.vector.tensor_tensor(hc_[...], hc_[...], hold, OP.subtract)
                    nc.vector.tensor_tensor(hc_[...], hc_[...], zgs[f][...], OP.mult)
                    for ct in range(2):
                        nc.vector.tensor_tensor(
                            h_loc[:, ct, f, :].rearrange("p (r w) -> p r w", w=48),
                            mh[f][:, 2 + ct, 106:406].rearrange("p (r w) -> p r w", w=50)[:, :, 1:49],
                            hc_[:, ct, :].rearrange("p (r w) -> p r w", w=50)[:, :, 1:49],
                            OP.add)

            # ---------------- readout ----------------
            hloc16f = work.tile([128, 2, NF, PL], bf16, tag="hloc16", bufs=1,
                                name="hloc16f")
            nc.vector.tensor_copy(hloc16f[...], h_loc[...])
            bdi2 = dram.tile([NF, C, 4 * WF], bf16, tag="bdi", bufs=2, name="bdi2")
            for f in range(NF):
                for ct in range(2):
                    nc.sync.dma_start(bdi2[f, ct * 128:(ct + 1) * 128, 0:96],
                                      hloc16f[:, ct, f, 0:96])
                    nc.sync.dma_start(bdi2[f, ct * 128:(ct + 1) * 128, 96:192],
                                      hloc16f[:, ct, f, 192:288])
            bdo2 = dram.tile([NCORES, NF, C, 4 * WF], bf16, tag="bdo", bufs=2,
                             addr_space="Shared", name="bdo2")
            nc.gpsimd.collective_compute(
                "AllGather", OP.bypass, replica_groups=RG,
                ins=[bdi2.opt()], outs=[bdo2.opt()])
            halo2 = work.tile([128, 2, NF, 2, 96], f32, tag="halo", bufs=1, name="halo2")
            for ct in range(2):
                for rp in range(4):
                    ch = work.tile([128, 2, NF, 192], bf16, tag="hchk", bufs=1, name="hchk2")
                    for _rr in range(2):
                        nc.sync.dma_start(
                            ch[:, _rr, :, :],
                            bdo2[rp * 2 + _rr, :, ct * 128:(ct + 1) * 128, :]
                            .rearrange("f c x -> c f x"))
                    for rr in range(2):
                        r = rp * 2 + rr
                        for tb in range(2):
                            sel = seltop_t if tb == 0 else selbot_t
                            seg = ch[:, rr, :, 96:192] if tb == 0 else ch[:, rr, :, 0:96]
                            dst = halo2[:, ct, :, tb, :]
                            if r == 0:
                                nc.vector.tensor_scalar(dst, seg, sel[:, 0:1], None, OP.mult)
                            else:
                                nc.vector.scalar_tensor_tensor(
                                    dst, seg, sel[:, r:r + 1], dst, OP.mult, OP.add)

            mask_sb = pers.tile([1, NF, RW * WF], f32)
            for f in range(NF):
                ro_in = work.tile([128, 4, 512], f32r, tag="mh", bufs=3, name="ro_in")
                nc.gpsimd.memset(ro_in[...].bitcast(f32), 0.0)
                for ct in range(2):
                    hrows = ro_in[:, ct, 6:506].rearrange("p (r w) -> p r w", w=50)
                    nc.vector.tensor_copy(
                        hrows[:, 2:8, 1:49],
                        h_loc[:, ct, f, :].rearrange("p (r w) -> p r w", w=48))
                    nc.vector.tensor_copy(
                        hrows[:, 0:2, 1:49],
                        halo2[:, ct, f, 0, :].rearrange("p (r w) -> p r w", w=48))
                    nc.vector.tensor_copy(
                        hrows[:, 8:10, 1:49],
                        halo2[:, ct, f, 1, :].rearrange("p (r w) -> p r w", w=48))
                    vrows = ro_in[:, 2 + ct, 6:506].rearrange("p (r w) -> p r w", w=50)
                    nc.vector.tensor_copy(
                        vrows[:, 0:10, 1:49],
                        v_sb[:, ct, f, :].rearrange("p (r w) -> p r w", w=48))
                # y = relu(ro1 * cat) rows W1..W8 (N=400)
                yps = psum.tile([128, 2, 512], f32, tag="pe", bufs=2, name="yps")
                for tap in range(9):
                    dy, dx = tap // 3 - 1, tap % 3 - 1
                    wt = work.tile([128, 2, 4, 128], f32r, tag="wtap", bufs=2, name="wtap2")
                    nc.sync.dma_start(wt[...], ro1[tap])
                    o = 6 + (1 + dy) * 50 + dx
                    for kt in range(4):
                        for mt in range(2):
                            nc.tensor.matmul(
                                yps[:, mt, 0:400], wt[:, mt, kt, :],
                                r32(ro_in[:, kt, o:o + 400]),
                                start=(tap == 0 and kt == 0), stop=(tap == 8 and kt == 3))
                y_sb = work.tile([128, 2, 412], f32r, tag="y_sb", bufs=1, name="y_sb")
                nc.gpsimd.memset(y_sb[...].bitcast(f32), 0.0)
                for mt in range(2):
                    ypr = yps[:, mt, 0:400].rearrange("p (r w) -> p r w", w=50)
                    ydst = y_sb[:, mt, 6:406].rearrange("p (r w) -> p r w", w=50)
                    # y rows: 0 (global 6k-1, invalid on core 0), 1..7, 7 (invalid on core 7)
                    nc.scalar.activation(ydst[:, 0:1, 1:49], ypr[:, 0:1, 1:49], AF.Relu,
                                         bias=rb1top_t[:, mt:mt + 1], scale=sctop_t[:, :])
                    nc.scalar.activation(ydst[:, 1:7, 1:49], ypr[:, 1:7, 1:49], AF.Relu,
                                         bias=rb1_t[:, mt:mt + 1])
                    nc.scalar.activation(ydst[:, 7:8, 1:49], ypr[:, 7:8, 1:49], AF.Relu,
                                         bias=rb1bot_t[:, mt:mt + 1], scale=scbot_t[:, :])
                # mask = ro2 * y + b2, own rows (N=300 in 50-col layout; y pad
                # cols are zero so the windows are safe)
                mps = psum.tile([1, 300], f32, tag="aux", bufs=1, name="mps")
                for tap in range(9):
                    dy, dx = tap // 3 - 1, tap % 3 - 1
                    o = 6 + (1 + dy) * 50 + dx
                    for ct in range(2):
                        nc.tensor.matmul(
                            mps[0:1, 0:300],
                            r32(ro2_t[:, ct, tap:tap + 1]),
                            r32(y_sb[:, ct, o:o + 300]),
                            start=(tap == 0 and ct == 0), stop=(tap == 8 and ct == 1))
                nc.scalar.activation(
                    mask_sb[0:1, f, :].rearrange("p (r w) -> p r w", w=48),
                    mps[0:1, :].rearrange("p (r w) -> p r w", w=50)[:, :, 1:49],
                    AF.Identity, bias=rb2_t[0:1, :])
            nc.sync.dma_start(out_ext.ap(), mask_sb[0:1, :, :])

    nc.finalize()
    return nc


def _prep_inputs(inputs):
    """Host-side weight prep + per-core shards."""
    frames = np.ascontiguousarray(inputs['frames'], dtype=np.float32)  # (1,3,3,384,384)
    bb_w = np.asarray(inputs['backbone_w'], dtype=np.float32)
    bb_b = np.asarray(inputs['backbone_b'], dtype=np.float32).reshape(C, 1)
    W_intra = np.asarray(inputs['W_intra'], dtype=np.float32)
    W_inter = np.asarray(inputs['W_inter'], dtype=np.float32)
    gate_w = np.asarray(inputs['gate_w'], dtype=np.float32)[:, :, 0, 0]
    gate_b = np.asarray(inputs['gate_b'], dtype=np.float32).reshape(C, 1)

    def taps(w):
        return np.ascontiguousarray(
            np.asarray(w, dtype=np.float32).transpose(2, 3, 1, 0).reshape(9, 2 * C, C))

    def swz(w):
        """[c, d] (256x256) -> [128, 2ct, d]"""
        return np.ascontiguousarray(w.reshape(2, 128, C).transpose(1, 0, 2))

    def bias2(b):
        """(256,1) -> (128, 2)"""
        return np.ascontiguousarray(b.reshape(2, 128).T)

    def conv_taps(w):
        """(O,I,3,3) -> [9, kp(128), mt, kt, mp(128)]"""
        t = taps(w)                                    # (9, 512, 256)
        t = t.reshape(9, 4, 128, 2, 128)               # tap, kt, kp, mt, mp
        return np.ascontiguousarray(t.transpose(0, 2, 3, 1, 4))

    bbw192 = bb_w.transpose(1, 2, 3, 0).reshape(192, C)
    ro2_flat = np.asarray(inputs['ro_w2'], dtype=np.float32
                          ).transpose(2, 3, 1, 0).reshape(9, C)      # tap, c
    common = {
        'bbw0': np.ascontiguousarray(bbw192[0:128]),
        'bbw1': np.ascontiguousarray(bbw192[128:192]),
        'bbb': bias2(bb_b),
        'w_int': swz(W_inter),
        'w_inta': swz(W_intra),
        'gw_t': swz(np.ascontiguousarray(gate_w.T)),
        'gb_neg': bias2(-gate_b),
        'wz': conv_taps(inputs['Wz']), 'wr': conv_taps(inputs['Wr']),
        'wh': conv_taps(inputs['Wh']),
        'bz': bias2(np.asarray(inputs['bz'], dtype=np.float32)),
        'br': bias2(np.asarray(inputs['br'], dtype=np.float32)),
        'bh': bias2(np.asarray(inputs['bh'], dtype=np.float32)),
        'ro1': conv_taps(inputs['ro_w1']),
        'rb1': bias2(np.asarray(inputs['ro_b1'], dtype=np.float32)),
        'ro2': np.ascontiguousarray(
            ro2_flat.T.reshape(2, 128, 9).transpose(1, 0, 2)),
        'rb2': np.asarray(inputs['ro_b2'], dtype=np.float32).reshape(1, 1),
        'ident_in': np.eye(128, dtype=np.float32),
    }

    fp = np.zeros((NF, 3, 384 + 32, 384), np.float32)
    fp[:, :, 16:400] = frames[0]

    in_maps = []
    for k in range(NCORES):
        m = dict(common)
        # patches [192=(ch,dy,dx), f, 10 rows x 48]
        pc = fp[:, :, 48 * k:48 * k + 80, :].reshape(NF, 3, 10, 8, 48, 8)
        m['patches'] = np.ascontiguousarray(
            pc.transpose(1, 3, 5, 0, 2, 4).reshape(192, NF, 480))
        st = np.zeros((128, 8), np.float32)
        sb = np.zeros((128, 8), np.float32)
        if k > 0:
            st[:, k - 1] = 1.0
        if k < NCORES - 1:
            sb[:, k + 1] = 1.0
        m['sel_top'] = st
        m['sel_bot'] = sb
        sct = np.full((128, 1), 0.0 if k == 0 else 1.0, np.float32)
        scb = np.full((128, 1), 0.0 if k == NCORES - 1 else 1.0, np.float32)
        m['sc_top'] = sct
        m['sc_bot'] = scb
        m['bbb_top'] = common['bbb'] * sct[0, 0]
        m['bbb_bot'] = common['bbb'] * scb[0, 0]
        m['rb1_top'] = common['rb1'] * sct[0, 0]
        m['rb1_bot'] = common['rb1'] * scb[0, 0]
        in_maps.append(m)
    return in_maps


def run_cores(inputs, trace=False):
    """Returns (per_core_results, BassKernelResults)."""
    sys.path.insert(0, '/opt/trn_rl_repo')
    from concourse.bass_utils import run_bass_kernel_spmd
    if 'nc' not in _CACHE:
        _CACHE['nc'] = _build_graph()
    nc = _CACHE['nc']
    in_maps = _prep_inputs(inputs)
    res = run_bass_kernel_spmd(nc, in_maps, core_ids=list(range(NCORES)), trace=trace)
    return res


def kernel(**inputs):
    res = run_cores(inputs, trace=False)
    out = np.zeros((1, NF, 1, HF, WF), np.float32)
    for k in range(NCORES):
        out[0, :, 0, RW * k:RW * (k + 1), :] = res.results[k]['out']
    return out


if __name__ == '__main__':
    data = np.load('/tmp/ref_inputs.npz')
    inputs = {k: data[k] for k in data.files}
    out = kernel(**inputs)
    ref = np.load('/tmp/ref_out.npy')
    rel = np.linalg.norm(out - ref) / np.linalg.norm(ref)
    print('rel l2 err:', rel)


# revision 26
# speedup vs baseline: 45.0724x; 1.0072x over previous
"""Trainium2 Bass kernel for nn_AGNN (3-frame attentional GNN + ConvGRU).

Self-contained: builds an 8-core SPMD Bass graph (sequence-parallel over the
48x48 spatial tokens, 6 rows per core), runs it via run_bass_kernel_spmd,
and reassembles the full output.

Sharding: each core owns 6 rows (288 tokens) of every frame. Per iteration:
  AllGather h (bf16, ch-major + tok-major layouts) -> each core computes
  attention for its 288 query tokens against all 2304 keys of each frame
  (9 ordered frame pairs), gated aggregation, then a 4-row boundary
  AllGather (magg + h) feeds the halo rows of the 3x3 ConvGRU which each
  core evaluates for its own rows.  Readout convs are local (v computed
  with halo from the raw frames; h halo from a final boundary exchange).

Precision: attention and ConvGRU/readout matmuls in bf16 (fp32 PSUM
accumulation), backbone/t/gate matmuls in float32r (tf32 mode).
Measured ~3.8e-3 rel error vs the fp32 reference on hardware.
"""
import sys
import numpy as np

NF = 3          # frames
C = 256         # channels
HF = WF = 48    # feature map
P = HF * WF     # 2304 tokens/frame
NCORES = 8
RW = 6          # rows per core
PL = RW * WF    # 288 tokens per core
K_ITERS = 3

_CACHE = {}


def _build_graph():
    sys.path.insert(0, '/opt/trn_rl_repo')
    import concourse.bass as bass
    import concourse.mybir as mybir
    import concourse.tile as tile
    from concourse import bacc

    dt = mybir.dt
    f32 = dt.float32
    f32r = dt.float32r
    bf16 = dt.bfloat16
    AF = mybir.ActivationFunctionType
    OP = mybir.AluOpType
    RG = [list(range(NCORES))]

    nc = bacc.Bacc()

    # ---------------- external IO ----------------
    def ein(name, shape, dtype=None):
        return nc.dram_tensor(name, list(shape), dtype or f32, kind="ExternalInput")

    patches = ein("patches", (192, NF, 480), f32r)       # host patch-extract, rows 6k-2..6k+7
    bbw0 = ein("bbw0", (128, C), f32r)
    bbw1 = ein("bbw1", (64, C), f32r)
    bbb = ein("bbb", (128, 2))
    bbb_top = ein("bbb_top", (128, 2))
    bbb_bot = ein("bbb_bot", (128, 2))
    w_int = ein("w_int", (128, 2, C), f32r)              # W_inter [c, d] swizzled
    w_inta = ein("w_inta", (128, 2, C), f32r)
    gw_t = ein("gw_t", (128, 2, C), f32r)                # gate_w^T [i, o] swizzled
    gb_neg = ein("gb_neg", (128, 2))                       # -gate_b
    wz = ein("wz", (9, 128, 2, 4, 128), f32r)            # [tap, kp, mt, kt, mp]
    wr = ein("wr", (9, 128, 2, 4, 128), f32r)
    wh = ein("wh", (9, 128, 2, 4, 128), f32r)
    bz = ein("bz", (128, 2))
    br = ein("br", (128, 2))
    bh = ein("bh", (128, 2))
    ro1 = ein("ro1", (9, 128, 2, 4, 128), f32r)
    rb1 = ein("rb1", (128, 2))
    rb1_top = ein("rb1_top", (128, 2))
    rb1_bot = ein("rb1_bot", (128, 2))
    ro2 = ein("ro2", (128, 2, 9), f32r)
    rb2 = ein("rb2", (1, 1))
    sel_top = ein("sel_top", (128, 8))                   # one-hot rank k-1 (zeros at core 0)
    sel_bot = ein("sel_bot", (128, 8))                   # one-hot rank k+1 (zeros at core 7)
    sc_top = ein("sc_top", (128, 1))                     # 0.0 on core 0 else 1.0
    sc_bot = ein("sc_bot", (128, 1))                     # 0.0 on core 7 else 1.0
    ident_in = ein("ident_in", (128, 128), f32r)

    out_ext = nc.dram_tensor("out", [NF, RW, WF], f32, kind="ExternalOutput")

    SH = NF * C * PL            # 221184 elements per layout shard

    with tile.TileContext(nc) as tc:
        with (
            tc.tile_pool(name="pers", bufs=1) as pers,
            tc.tile_pool(name="dram", bufs=1, space="DRAM") as dram,
            tc.tile_pool(name="psum", bufs=1, space="PSUM") as psum,
            tc.tile_pool(name="work", bufs=1) as work,
        ):
            # ---------------- persistent SBUF ----------------
            ident = pers.tile([128, 128], f32r)
            nc.sync.dma_start(ident[...], ident_in.ap())

            def load_pers(name, ext, shape, view=None):
                t = pers.tile(list(shape), f32r, name=name)
                src = ext.ap() if view is None else view
                nc.sync.dma_start(t[...], src)
                return t

            # W_inter/W_intra/gate_w^T as [128, 2ct, 256]
            wint_t = load_pers("wint_t", w_int, (128, 2, C))
            winta_t = load_pers("winta_t", w_inta, (128, 2, C))
            gw_tt = load_pers("gw_tt", gw_t, (128, 2, C))
            bbw0_t = load_pers("bbw0_t", bbw0, (128, C))
            bbw1_t = load_pers("bbw1_t", bbw1, (64, C))
            ro2_t = load_pers("ro2_t", ro2, (128, 2, 9))

            def load_bias(name, ext):
                t = pers.tile([128, 2], f32, name=name)
                nc.sync.dma_start(t[...], ext.ap())
                return t

            bbb_t = load_bias("bbb_t", bbb)
            bbbtop_t = load_bias("bbbtop_t", bbb_top)
            bbbbot_t = load_bias("bbbbot_t", bbb_bot)
            gnb_t = load_bias("gnb_t", gb_neg)
            bz_t = load_bias("bz_t", bz)
            br_t = load_bias("br_t", br)
            bh_t = load_bias("bh_t", bh)
            rb1_t = load_bias("rb1_t", rb1)
            rb1top_t = load_bias("rb1top_t", rb1_top)
            rb1bot_t = load_bias("rb1bot_t", rb1_bot)
            rb2_t = pers.tile([1, 1], f32)
            nc.sync.dma_start(rb2_t[...], rb2.ap())
            seltop_t = pers.tile([128, 8], f32)
            nc.sync.dma_start(seltop_t[...], sel_top.ap())
            selbot_t = pers.tile([128, 8], f32)
            nc.sync.dma_start(selbot_t[...], sel_bot.ap())
            sctop_t = pers.tile([128, 1], f32)
            nc.sync.dma_start(sctop_t[...], sc_top.ap())
            scbot_t = pers.tile([128, 1], f32)
            nc.sync.dma_start(scbot_t[...], sc_bot.ap())

            # big persistent state
            v_sb = pers.tile([128, 2, NF, 10 * WF], f32)      # v rows 6k-2..6k+7
            h_loc = pers.tile([128, 2, NF, PL], f32r)          # own rows, ch-major
            magg = pers.tile([128, 2, NF, PL], f32r)
            t_sb = pers.tile([128, 2, 6, PL], bf16)           # t^T (3 inter + 3 intra)

            def r32(ap):
                return ap.bitcast(f32r)

            # ---------------- backbone ----------------
            with tc.tile_pool(name="bb", bufs=1) as bb:
                pk0 = bb.tile([128, NF, 480], f32r, name="pk0")
                pk1 = bb.tile([64, NF, 480], f32r, name="pk1")
                nc.sync.dma_start(pk0[...], patches[0:128])
                nc.sync.dma_start(pk1[...], patches[128:192])
                for f in range(NF):
                    vps = psum.tile([128, 2, 512], f32, tag="pe", bufs=2, name="vps")
                    for mt in range(2):
                        nc.tensor.matmul(vps[:, mt, 0:480],
                                         bbw0_t[:, mt * 128:(mt + 1) * 128],
                                         pk0[:, f, :], start=True, stop=False)
                        nc.tensor.matmul(vps[:, mt, 0:480],
                                         bbw1_t[0:64, mt * 128:(mt + 1) * 128],
                                         pk1[0:64, f, :], start=False, stop=True)
                    for mt in range(2):
                        # rows 0-1 / 2-7 / 8-9 with edge masking (v=0 outside image)
                        nc.scalar.activation(v_sb[:, mt, f, 0:96], vps[:, mt, 0:96],
                                             AF.Relu, bias=bbbtop_t[:, mt:mt + 1], scale=sctop_t[:, :])
                        nc.scalar.activation(v_sb[:, mt, f, 96:384], vps[:, mt, 96:384],
                                             AF.Relu, bias=bbb_t[:, mt:mt + 1])
                        nc.scalar.activation(v_sb[:, mt, f, 384:480], vps[:, mt, 384:480],
                                             AF.Relu, bias=bbbbot_t[:, mt:mt + 1], scale=scbot_t[:, :])
                        # h0 = v own rows (rows 2..8 of the 10-row window)
                        nc.vector.tensor_copy(h_loc[:, mt, f, :], v_sb[:, mt, f, 96:384])

            # ---------------- iterations ----------------
            for it in range(K_ITERS):
                # ---- write AG input: ch-major + tok-major (bf16) ----
                agi_parts = [
                    dram.tile([2, 1, SH // NF], bf16, tag="agi0", bufs=2, name="agi0"),
                    dram.tile([2, 2, SH // NF], bf16, tag="agi12", bufs=2, name="agi12"),
                ]

                def agi_v(layout, f):
                    part = agi_parts[0] if f == 0 else agi_parts[1]
                    return part[layout, 0 if f == 0 else f - 1]
                hloc16 = work.tile([128, 2, NF, PL], bf16, tag="hloc16", bufs=1, name="hloc16")
                nc.vector.tensor_copy(hloc16[...], h_loc[...])
                for f in range(NF):
                    for ct in range(2):
                        nc.sync.dma_start(
                            agi_v(0, f).rearrange("(c t) -> c t", t=PL)[ct * 128:(ct + 1) * 128, :],
                            hloc16[:, ct, f, :])
                    # tok-major via TensorE transposes: [c,96tok] -> [96tok, 128c]
                    # (alternate psum banks so TensorE doesn't stall on the
                    # DVE copy of the previous block)
                    hlt = work.tile([96, 3, C], bf16, tag="hlt", bufs=2, name="hlt")
                    for ct in range(2):
                        for ps in range(3):
                            tp = psum.tile([96, 128], f32r,
                                           tag="aux" if (ct * 3 + ps) % 2 == 0 else "mu2",
                                           bufs=1, name="tp")
                            nc.tensor.transpose(
                                tp[0:96, 0:128],
                                h_loc[:, ct, f, ps * 96:(ps + 1) * 96],
                                ident[:, :])
                            nc.vector.tensor_copy(hlt[0:96, ps, ct * 128:(ct + 1) * 128],
                                                  tp[0:96, 0:128])
                    nc.sync.dma_start(
                        agi_v(1, f).rearrange("(t c) -> t c", c=C)
                        .rearrange("(ps p) c -> p ps c", p=96),
                        hlt[0:96, :, :])

                ago_parts = [
                    dram.tile([NCORES, 2, 1, SH // NF], bf16, tag="ago0", bufs=2,
                              addr_space="Shared", name="ago0"),
                    dram.tile([NCORES, 2, 2, SH // NF], bf16, tag="ago12", bufs=2,
                              addr_space="Shared", name="ago12"),
                ]
                for _p in range(2):
                    nc.gpsimd.collective_compute(
                        "AllGather", OP.bypass, replica_groups=RG,
                        ins=[agi_parts[_p].opt()], outs=[ago_parts[_p].opt()])

                def ago_v(layout, f):
                    part = ago_parts[0] if f == 0 else ago_parts[1]
                    return part[:, layout, 0 if f == 0 else f - 1]

                # ---- t = h_loc @ W (both kinds), bf16 out ----
                for i in range(NF):
                    for kind in range(2):       # 0 inter, 1 intra
                        wsel = wint_t if kind == 0 else winta_t
                        for dct in range(2):
                            tps = psum.tile([128, 2, 512], f32, tag="pe", bufs=2, name="tps")
                            for ct in range(2):
                                nc.tensor.matmul(
                                    tps[:, 0, 0:288],
                                    r32(wsel[:, ct, dct * 128:(dct + 1) * 128]),
                                    r32(h_loc[:, ct, i, :]),
                                    start=(ct == 0), stop=(ct == 1))
                            nc.vector.tensor_copy(t_sb[:, dct, kind * 3 + i, :],
                                                  tps[:, 0, 0:288])

                # ---- boundary AG input: magg written later; h part now ----
                bdi = dram.tile([2, NF, C, 4 * WF], bf16, tag="bdi", bufs=2, name="bdi")
                for f in range(NF):
                    for ct in range(2):
                        nc.sync.dma_start(
                            bdi[1, f, ct * 128:(ct + 1) * 128, 0:96], hloc16[:, ct, f, 0:96])
                        nc.sync.dma_start(
                            bdi[1, f, ct * 128:(ct + 1) * 128, 96:192], hloc16[:, ct, f, 192:288])

                # ---- attention over j (keys) and i (queries) ----
                # stage all frames contiguously in DRAM up front (rank chunks
                # are 288 tokens; 128-token tiles cross rank boundaries
                # otherwise)
                stgs, stgcs = [], []
                for j in range(NF):
                    stg = dram.tile([P, C], bf16, tag="stg", bufs=3, name="stg")
                    nc.sync.dma_start(stg[:, :], ago_v(1, j))
                    stgs.append(stg)
                    stgc = dram.tile([C, P], bf16, tag="stgc", bufs=3, name="stgc")
                    nc.sync.dma_start(
                        stgc[:, :].rearrange("c (r t) -> r c t", r=NCORES),
                        ago_v(0, j))
                    stgcs.append(stgc)
                for j in range(NF):
                    stg, stgc = stgs[j], stgcs[j]
                    hch = []
                    for hh in range(2):
                        t_ = work.tile([128, 2, 9, 128], bf16, tag="hch", bufs=3, name="hch")
                        for ct in range(2):
                            nc.sync.dma_start(
                                t_[:, ct, :, :],
                                stgc[ct * 128:(ct + 1) * 128, :]
                                .rearrange("p (q x) -> p q x", x=128)
                                [:, hh * 9:(hh + 1) * 9, :])
                        hch.append(t_)
                    htok = []
                    for hh in range(2):
                        t_ = work.tile([128, 9, 257], bf16, tag="htok", bufs=3, name="htok")
                        nc.gpsimd.memset(t_[...], 1.0)
                        nc.sync.dma_start(
                            t_[:, :, 0:256],
                            stg[:, :].rearrange("(q p) c -> p q c", p=128)
                            [:, hh * 9:(hh + 1) * 9, :])
                        htok.append(t_)

                    for i in range(NF):
                        tix = (3 + i) if i == j else i
                        attn = work.tile([128, 18, 288], bf16, tag="attn", bufs=2, name="attn")
                        mu0 = psum.tile([128, 2, 512], f32, tag="mu0", bufs=1, name="mu0")
                        mu2 = psum.tile([32, 257], f32, tag="mu2", bufs=1, name="mu2")
                        # software-pipelined: e/exp group g, then m-matmuls of g-1
                        for g in range(10):
                            if g < 9:
                                e2 = psum.tile([128, 2, 512], f32, tag="pe", bufs=2, name="e2")
                                for u in range(2):
                                    q = g * 2 + u
                                    for ct in range(2):
                                        nc.tensor.matmul(
                                            e2[:, u, 0:288],
                                            hch[q // 9][:, ct, q % 9, :],
                                            t_sb[:, ct, tix, :],
                                            start=(ct == 0), stop=(ct == 1))
                                nc.scalar.activation(attn[:, g * 2:g * 2 + 2, :],
                                                     e2[:, :, 0:288], AF.Exp)
                            if g >= 1:
                                for u in range(2):
                                    q = (g - 1) * 2 + u
                                    st = (q == 0)
                                    sp = (q == 17)
                                    mv = htok[q // 9][:, q % 9, :]
                                    nc.tensor.matmul(mu0[:, 0, 0:257], attn[:, q, 0:128],
                                                     mv, start=st, stop=sp)
                                    nc.tensor.matmul(mu0[:, 1, 0:257], attn[:, q, 128:256],
                                                     mv, start=st, stop=sp)
                                    nc.tensor.matmul(mu2[0:32, 0:257], attn[:, q, 256:288],
                                                     mv, start=st, stop=sp)
                        # normalize m (softmax denominator = col 256)
                        mnorm = work.tile([128, 3, 256], f32r, tag="mnorm", bufs=2, name="mnorm")
                        rs = work.tile([128, 3, 1], f32, tag="rs", bufs=2, name="rs")
                        for s in range(3):
                            mus = mu0[:, s, :] if s < 2 else mu2[0:32, :]
                            pp = 128 if s < 2 else 32
                            nc.vector.reciprocal(rs[0:pp, s, :], mus[0:pp, 256:257])
                            nc.vector.tensor_scalar(mnorm[0:pp, s, :], mus[0:pp, 0:256],
                                                    rs[0:pp, s, :], None, OP.mult)
                        # transpose m -> ch-major; alternate psum banks (aux
                        # and the just-freed mu2 slot) so TensorE doesn't wait
                        # on the DVE copy between the two c-tile groups
                        mT = work.tile([128, 2, 288], f32r, tag="mT", bufs=2, name="mT")
                        for ct in range(2):
                            tps = psum.tile([128, 288], f32r,
                                            tag="aux" if ct == 0 else "mu2",
                                            bufs=1, name="mtp")
                            for s in range(3):
                                pp = 128 if s < 2 else 32
                                nc.tensor.transpose(
                                    tps[:, s * 128:s * 128 + pp],
                                    mnorm[0:pp, s, ct * 128:(ct + 1) * 128],
                                    ident[0:pp, 0:pp])
                            nc.vector.tensor_copy(mT[:, ct, :], tps[:, 0:288])
                        # gate: g = sigmoid(gate_w m + b) via exp (stay on exp table)
                        gps = psum.tile([128, 2, 512], f32, tag="mu0", bufs=1, name="gps")
                        for oct in range(2):
                            for ict in range(2):
                                nc.tensor.matmul(
                                    gps[:, oct, 0:288],
                                    r32(gw_tt[:, ict, oct * 128:(oct + 1) * 128]),
                                    r32(mT[:, ict, :]),
                                    start=(ict == 0), stop=(ict == 1))
                        gtmp = work.tile([128, 2, 288], f32, tag="gtmp", bufs=2, name="gtmp")
                        for oct in range(2):
                            nc.scalar.activation(gtmp[:, oct, :], gps[:, oct, 0:288],
                                                 AF.Exp, bias=gnb_t[:, oct:oct + 1], scale=-1.0)
                        nc.vector.tensor_scalar(gtmp[...], gtmp[...], 1.0, None, OP.add)
                        nc.vector.reciprocal(gtmp[...], gtmp[...])
                        nc.vector.tensor_tensor(gtmp[...], gtmp[...], mT[...], OP.mult)
                        if j == 0:
                            nc.vector.tensor_copy(magg[:, :, i, :], gtmp[...])
                        else:
                            nc.vector.tensor_tensor(magg[:, :, i, :], magg[:, :, i, :],
                                                    gtmp[...], OP.add)

                # ---- boundary AG (magg + h 2-row halos, bf16) ----
                magg16 = work.tile([128, 2, NF, PL], bf16, tag="magg16", bufs=1,
                                   name="magg16")
                nc.vector.tensor_copy(magg16[...], magg[...])
                for f in range(NF):
                    for ct in range(2):
                        nc.sync.dma_start(
                            bdi[0, f, ct * 128:(ct + 1) * 128, 0:96], magg16[:, ct, f, 0:96])
                        nc.sync.dma_start(
                            bdi[0, f, ct * 128:(ct + 1) * 128, 96:192], magg16[:, ct, f, 192:288])
                bdo = dram.tile([NCORES, 2, NF, C, 4 * WF], bf16, tag="bdo", bufs=2,
                                addr_space="Shared", name="bdo")
                nc.gpsimd.collective_compute(
                    "AllGather", OP.bypass, replica_groups=RG,
                    ins=[bdi.opt()], outs=[bdo.opt()])

                # ---- halo extraction via one-hot rank masks ----
                # halo[kind][ct]: top rows (from rank k-1 bottom seg) & bottom rows
                halo = work.tile([128, 2, 2, NF, 2, 96], f32, tag="halo", bufs=1, name="halo")
                for kind in range(2):
                    for ct in range(2):
                        for rp in range(4):
                            ch = work.tile([128, 2, NF, 192], bf16, tag="hchk", bufs=1, name="hchk")
                            for _rr in range(2):
                                nc.sync.dma_start(
                                    ch[:, _rr, :, :],
                                    bdo[rp * 2 + _rr, kind, :, ct * 128:(ct + 1) * 128, :]
                                    .rearrange("f c x -> c f x"))
                            for rr in range(2):
                                r = rp * 2 + rr
                                for tb in range(2):
                                    sel = seltop_t if tb == 0 else selbot_t
                                    seg = ch[:, rr, :, 96:192] if tb == 0 else ch[:, rr, :, 0:96]
                                    dst = halo[:, kind, ct, :, tb, :]
                                    if r == 0:
                                        nc.vector.tensor_scalar(dst, seg, sel[:, 0:1],
                                                                None, OP.mult)
                                    else:
                                        nc.vector.scalar_tensor_tensor(
                                            dst, seg, sel[:, r:r + 1], dst,
                                            OP.mult, OP.add)

                # ---- ConvGRU ----
                mh = []
                for f in range(NF):
                    m_ = work.tile([128, 4, 512], f32r, tag="mh", bufs=3, name="mh")
                    nc.gpsimd.memset(m_[...].bitcast(f32), 0.0)
                    for ct in range(2):
                        rows = m_[:, ct, 6:506].rearrange("p (r w) -> p r w", w=50)
                        nc.vector.tensor_copy(
                            rows[:, 2:8, 1:49],
                            magg[:, ct, f, :].rearrange("p (r w) -> p r w", w=48))
                        nc.vector.tensor_copy(
                            rows[:, 0:2, 1:49],
                            halo[:, 0, ct, f, 0, :].rearrange("p (r w) -> p r w", w=48))
                        nc.vector.tensor_copy(
                            rows[:, 8:10, 1:49],
                            halo[:, 0, ct, f, 1, :].rearrange("p (r w) -> p r w", w=48))
                        hrows = m_[:, 2 + ct, 6:506].rearrange("p (r w) -> p r w", w=50)
                        nc.vector.tensor_copy(
                            hrows[:, 2:8, 1:49],
                            h_loc[:, ct, f, :].rearrange("p (r w) -> p r w", w=48))
                        nc.vector.tensor_copy(
                            hrows[:, 0:2, 1:49],
                            halo[:, 1, ct, f, 0, :].rearrange("p (r w) -> p r w", w=48))
                        nc.vector.tensor_copy(
                            hrows[:, 8:10, 1:49],
                            halo[:, 1, ct, f, 1, :].rearrange("p (r w) -> p r w", w=48))
                    mh.append(m_)

                def conv(wext, psums, NOUT, row0, src_of):
                    """9-tap conv: psums[f][:, mt, 0:NOUT] += taps."""
                    for tap in range(9):
                        dy, dx = tap // 3 - 1, tap % 3 - 1
                        wt = work.tile([128, 2, 4, 128], f32r, tag="wtap", bufs=2, name="wtap")
                        nc.sync.dma_start(wt[...], wext[tap])
                        for f in range(NF):
                            for kt in range(4):
                                mvs = src_of(f, kt, dy, dx)
                                if mvs is None:
                                    continue
                                for mt in range(2):
                                    nc.tensor.matmul(
                                        psums[f][:, mt, 0:NOUT],
                                        wt[:, mt, kt, :], r32(mvs),
                                        start=(tap == 0 and kt == 0),
                                        stop=(tap == 8 and kt == 3))

                def conv_psums():
                    ps = []
                    for f in range(NF):
                        tag = "pe" if f < 2 else "mu0"
                        ps.append(psum.tile([128, 2, 512], f32, tag=tag, bufs=2 if f < 2 else 1,
                                            name=f"cps{f}"))
                    return ps

                # z conv: out rows W2..W7 (own), N=300
                zps = conv_psums()
                conv(wz, zps, 300, 2,
                     lambda f, kt, dy, dx: mh[f][:, kt, 6 + (2 + dy) * 50 + dx:
                                                 6 + (2 + dy) * 50 + dx + 300])
                zgs = []
                for f in range(NF):
                    z_ = work.tile([128, 2, 300], f32, tag=f"zgs{f}", bufs=1, name="zgs")
                    for mt in range(2):
                        nc.scalar.activation(z_[:, mt, :], zps[f][:, mt, 0:300],
                                             AF.Sigmoid, bias=bz_t[:, mt:mt + 1])
                    zgs.append(z_)
                # r conv: out rows W1..W8, N=400
                rps = conv_psums()
                conv(wr, rps, 400, 1,
                     lambda f, kt, dy, dx: mh[f][:, kt, 6 + (1 + dy) * 50 + dx:
                                                 6 + (1 + dy) * 50 + dx + 400])
                rgh = []
                for f in range(NF):
                    # rg sigmoid written at 6-offset, then rg*h in place
                    rh_ = work.tile([128, 2, 416], f32r, tag=f"rgh{f}", bufs=1, name="rgh")
                    nc.gpsimd.memset(rh_[...].bitcast(f32), 0.0)
                    for mt in range(2):
                        nc.scalar.activation(rh_[:, mt, 6:406], rps[f][:, mt, 0:400],
                                             AF.Sigmoid, bias=br_t[:, mt:mt + 1])
                    nc.vector.tensor_tensor(
                        rh_[:, :, 6:406], rh_[:, :, 6:406],
                        mh[f][:, 2:4, 56:456], OP.mult)
                    rgh.append(rh_)
                # candidate conv: out rows W2..W7, N=300; inputs kt0-1 magg, kt2-3 rg*h
                hps = conv_psums()

                def hc_src(f, kt, dy, dx):
                    if kt < 2:
                        o = 6 + (2 + dy) * 50 + dx
                        return mh[f][:, kt, o:o + 300]
                    o = 6 + (1 + dy) * 50 + dx
                    return rgh[f][:, kt - 2, o:o + 300]

                conv(wh, hps, 300, 2, hc_src)
                for f in range(NF):
                    hc_ = work.tile([128, 2, 300], f32, tag="hcs", bufs=2, name="hcs")
                    for mt in range(2):
                        nc.scalar.activation(hc_[:, mt, :], hps[f][:, mt, 0:300],
                                             AF.Tanh, bias=bh_t[:, mt:mt + 1])
                    # h_new = h + z*(hc - h)
                    hold = mh[f][:, 2:4, 106:406]
                    nc.vector.tensor_tensor(hc_[...], hc_[...], hold, OP.subtract)
                    nc.vector.tensor_tensor(hc_[...], hc_[...], zgs[f][...], OP.mult)
                    for ct in range(2):
                        nc.vector.tensor_tensor(
                            h_loc[:, ct, f, :].rearrange("p (r w) -> p r w", w=48),
                            mh[f][:, 2 + ct, 106:406].rearrange("p (r w) -> p r w", w=50)[:, :, 1:49],
                            hc_[:, ct, :].rearrange("p (r w) -> p r w", w=50)[:, :, 1:49],
                            OP.add)

            # ---------------- readout ----------------
            hloc16f = work.tile([128, 2, NF, PL], bf16, tag="hloc16", bufs=1,
                                name="hloc16f")
            nc.vector.tensor_copy(hloc16f[...], h_loc[...])
            bdi2 = dram.tile([NF, C, 4 * WF], bf16, tag="bdi", bufs=2, name="bdi2")
            for f in range(NF):
                for ct in range(2):
                    nc.sync.dma_start(bdi2[f, ct * 128:(ct + 1) * 128, 0:96],
                                      hloc16f[:, ct, f, 0:96])
                    nc.sync.dma_start(bdi2[f, ct * 128:(ct + 1) * 128, 96:192],
                                      hloc16f[:, ct, f, 192:288])
            bdo2 = dram.tile([NCORES, NF, C, 4 * WF], bf16, tag="bdo", bufs=2,
                             addr_space="Shared", name="bdo2")
            nc.gpsimd.collective_compute(
                "AllGather", OP.bypass, replica_groups=RG,
                ins=[bdi2.opt()], outs=[bdo2.opt()])
            halo2 = work.tile([128, 2, NF, 2, 96], f32, tag="halo", bufs=1, name="halo2")
            for ct in range(2):
                for rp in range(4):
                    ch = work.tile([128, 2, NF, 192], bf16, tag="hchk", bufs=1, name="hchk2")
                    for _rr in range(2):
                        nc.sync.dma_start(
                            ch[:, _rr, :, :],
                            bdo2[rp * 2 + _rr, :, ct * 128:(ct + 1) * 128, :]
                            .rearrange("f c x -> c f x"))
                    for rr in range(2):
                        r = rp * 2 + rr
                        for tb in range(2):
                            sel = seltop_t if tb == 0 else selbot_t
                            seg = ch[:, rr, :, 96:192] if tb == 0 else ch[:, rr, :, 0:96]
                            dst = halo2[:, ct, :, tb, :]
                            if r == 0:
                                nc.vector.tensor_scalar(dst, seg, sel[:, 0:1], None, OP.mult)
                            else:
                                nc.vector.scalar_tensor_tensor(
                                    dst, seg, sel[:, r:r + 1], dst, OP.mult, OP.add)

            mask_sb = pers.tile([1, NF, RW * WF], f32)
            for f in range(NF):
                ro_in = work.tile([128, 4, 512], f32r, tag="mh", bufs=3, name="ro_in")
                nc.gpsimd.memset(ro_in[...].bitcast(f32), 0.0)
                for ct in range(2):
                    hrows = ro_in[:, ct, 6:506].rearrange("p (r w) -> p r w", w=50)
                    nc.vector.tensor_copy(
                        hrows[:, 2:8, 1:49],
                        h_loc[:, ct, f, :].rearrange("p (r w) -> p r w", w=48))
                    nc.vector.tensor_copy(
                        hrows[:, 0:2, 1:49],
                        halo2[:, ct, f, 0, :].rearrange("p (r w) -> p r w", w=48))
                    nc.vector.tensor_copy(
                        hrows[:, 8:10, 1:49],
                        halo2[:, ct, f, 1, :].rearrange("p (r w) -> p r w", w=48))
                    vrows = ro_in[:, 2 + ct, 6:506].rearrange("p (r w) -> p r w", w=50)
                    nc.vector.tensor_copy(
                        vrows[:, 0:10, 1:49],
                        v_sb[:, ct, f, :].rearrange("p (r w) -> p r w", w=48))
                # y = relu(ro1 * cat) rows W1..W8 (N=400)
                yps = psum.tile([128, 2, 512], f32, tag="pe", bufs=2, name="yps")
                for tap in range(9):
                    dy, dx = tap // 3 - 1, tap % 3 - 1
                    wt = work.tile([128, 2, 4, 128], f32r, tag="wtap", bufs=2, name="wtap2")
                    nc.sync.dma_start(wt[...], ro1[tap])
                    o = 6 + (1 + dy) * 50 + dx
                    for kt in range(4):
                        for mt in range(2):
                            nc.tensor.matmul(
                                yps[:, mt, 0:400], wt[:, mt, kt, :],
                                r32(ro_in[:, kt, o:o + 400]),
                                start=(tap == 0 and kt == 0), stop=(tap == 8 and kt == 3))
                y_sb = work.tile([128, 2, 412], f32r, tag="y_sb", bufs=1, name="y_sb")
                nc.gpsimd.memset(y_sb[...].bitcast(f32), 0.0)
                for mt in range(2):
                    ypr = yps[:, mt, 0:400].rearrange("p (r w) -> p r w", w=50)
                    ydst = y_sb[:, mt, 6:406].rearrange("p (r w) -> p r w", w=50)
                    # y rows: 0 (global 6k-1, invalid on core 0), 1..7, 7 (invalid on core 7)
                    nc.scalar.activation(ydst[:, 0:1, 1:49], ypr[:, 0:1, 1:49], AF.Relu,
                                         bias=rb1top_t[:, mt:mt + 1], scale=sctop_t[:, :])
                    nc.scalar.activation(ydst[:, 1:7, 1:49], ypr[:, 1:7, 1:49], AF.Relu,
                                         bias=rb1_t[:, mt:mt + 1])
                    nc.scalar.activation(ydst[:, 7:8, 1:49], ypr[:, 7:8, 1:49], AF.Relu,
                                         bias=rb1bot_t[:, mt:mt + 1], scale=scbot_t[:, :])
                # mask = ro2 * y + b2, own rows (N=300 in 50-col layout; y pad
                # cols are zero so the windows are safe)
                mps = psum.tile([1, 300], f32, tag="aux", bufs=1, name="mps")
                for tap in range(9):
                    dy, dx = tap // 3 - 1, tap % 3 - 1
                    o = 6 + (1 + dy) * 50 + dx
                    for ct in range(2):
                        nc.tensor.matmul(
                            mps[0:1, 0:300],
                            r32(ro2_t[:, ct, tap:tap + 1]),
                            r32(y_sb[:, ct, o:o + 300]),
                            start=(tap == 0 and ct == 0), stop=(tap == 8 and ct == 1))
                nc.scalar.activation(
                    mask_sb[0:1, f, :].rearrange("p (r w) -> p r w", w=48),
                    mps[0:1, :].rearrange("p (r w) -> p r w", w=50)[:, :, 1:49],
                    AF.Identity, bias=rb2_t[0:1, :])
            nc.sync.dma_start(out_ext.ap(), mask_sb[0:1, :, :])

    nc.finalize()
    return nc


def _prep_inputs(inputs):
    """Host-side weight prep + per-core shards."""
    frames = np.ascontiguousarray(inputs['frames'], dtype=np.float32)  # (1,3,3,384,384)
    bb_w = np.asarray(inputs['backbone_w'], dtype=np.float32)
    bb_b = np.asarray(inputs['backbone_b'], dtype=np.float32).reshape(C, 1)
    W_intra = np.asarray(inputs['W_intra'], dtype=np.float32)
    W_inter = np.asarray(inputs['W_inter'], dtype=np.float32)
    gate_w = np.asarray(inputs['gate_w'], dtype=np.float32)[:, :, 0, 0]
    gate_b = np.asarray(inputs['gate_b'], dtype=np.float32).reshape(C, 1)

    def taps(w):
        return np.ascontiguousarray(
            np.asarray(w, dtype=np.float32).transpose(2, 3, 1, 0).reshape(9, 2 * C, C))

    def swz(w):
        """[c, d] (256x256) -> [128, 2ct, d]"""
        return np.ascontiguousarray(w.reshape(2, 128, C).transpose(1, 0, 2))

    def bias2(b):
        """(256,1) -> (128, 2)"""
        return np.ascontiguousarray(b.reshape(2, 128).T)

    def conv_taps(w):
        """(O,I,3,3) -> [9, kp(128), mt, kt, mp(128)]"""
        t = taps(w)                                    # (9, 512, 256)
        t = t.reshape(9, 4, 128, 2, 128)               # tap, kt, kp, mt, mp
        return np.ascontiguousarray(t.transpose(0, 2, 3, 1, 4))

    bbw192 = bb_w.transpose(1, 2, 3, 0).reshape(192, C)
    ro2_flat = np.asarray(inputs['ro_w2'], dtype=np.float32
                          ).transpose(2, 3, 1, 0).reshape(9, C)      # tap, c
    common = {
        'bbw0': np.ascontiguousarray(bbw192[0:128]),
        'bbw1': np.ascontiguousarray(bbw192[128:192]),
        'bbb': bias2(bb_b),
        'w_int': swz(W_inter),
        'w_inta': swz(W_intra),
        'gw_t': swz(np.ascontiguousarray(gate_w.T)),
        'gb_neg': bias2(-gate_b),
        'wz': conv_taps(inputs['Wz']), 'wr': conv_taps(inputs['Wr']),
        'wh': conv_taps(inputs['Wh']),
        'bz': bias2(np.asarray(inputs['bz'], dtype=np.float32)),
        'br': bias2(np.asarray(inputs['br'], dtype=np.float32)),
        'bh': bias2(np.asarray(inputs['bh'], dtype=np.float32)),
        'ro1': conv_taps(inputs['ro_w1']),
        'rb1': bias2(np.asarray(inputs['ro_b1'], dtype=np.float32)),
        'ro2': np.ascontiguousarray(
            ro2_flat.T.reshape(2, 128, 9).transpose(1, 0, 2)),
        'rb2': np.asarray(inputs['ro_b2'], dtype=np.float32).reshape(1, 1),
        'ident_in': np.eye(128, dtype=np.float32),
    }

    fp = np.zeros((NF, 3, 384 + 32, 384), np.float32)
    fp[:, :, 16:400] = frames[0]

    in_maps = []
    for k in range(NCORES):
        m = dict(common)
        # patches [192=(ch,dy,dx), f, 10 rows x 48]
        pc = fp[:, :, 48 * k:48 * k + 80, :].reshape(NF, 3, 10, 8, 48, 8)
        m['patches'] = np.ascontiguousarray(
            pc.transpose(1, 3, 5, 0, 2, 4).reshape(192, NF, 480))
        st = np.zeros((128, 8), np.float32)
        sb = np.zeros((128, 8), np.float32)
        if k > 0:
            st[:, k - 1] = 1.0
        if k < NCORES - 1:
            sb[:, k + 1] = 1.0
        m['sel_top'] = st
        m['sel_bot'] = sb
        sct = np.full((128, 1), 0.0 if k == 0 else 1.0, np.float32)
        scb = np.full((128, 1), 0.0 if k == NCORES - 1 else 1.0, np.float32)
        m['sc_top'] = sct
        m['sc_bot'] = scb
        m['bbb_top'] = common['bbb'] * sct[0, 0]
        m['bbb_bot'] = common['bbb'] * scb[0, 0]
        m['rb1_top'] = common['rb1'] * sct[0, 0]
        m['rb1_bot'] = common['rb1'] * scb[0, 0]
        in_maps.append(m)
    return in_maps


def run_cores(inputs, trace=False):
    """Returns (per_core_results, BassKernelResults)."""
    sys.path.insert(0, '/opt/trn_rl_repo')
    from concourse.bass_utils import run_bass_kernel_spmd
    if 'nc' not in _CACHE:
        _CACHE['nc'] = _build_graph()
    nc = _CACHE['nc']
    in_maps = _prep_inputs(inputs)
    res = run_bass_kernel_spmd(nc, in_maps, core_ids=list(range(NCORES)), trace=trace)
    return res


def kernel(**inputs):
    res = run_cores(inputs, trace=False)
    out = np.zeros((1, NF, 1, HF, WF), np.float32)
    for k in range(NCORES):
        out[0, :, 0, RW * k:RW * (k + 1), :] = res.results[k]['out']
    return out


if __name__ == '__main__':
    data = np.load('/tmp/ref_inputs.npz')
    inputs = {k: data[k] for k in data.files}
    out = kernel(**inputs)
    ref = np.load('/tmp/ref_out.npy')
    rel = np.linalg.norm(out - ref) / np.linalg.norm(ref)
    print('rel l2 err:', rel)
